# revision 1
# baseline (speedup 1.0000x reference)
"""Trainium2 Bass kernel for a Deformable-DETR style encoder block.

Sharding: 8 NeuronCores = 4 batch samples x 2 query-halves.

Per core:
  - value projection over the full sample -> fp16 "patch table" in DRAM:
    for cell (y,x) and head h the 2x2 neighborhood [V[y,x], V[y,x+1],
    V[y+1,x], V[y+1,x+1]] is packed contiguously (4*32 fp16 = 256B), so one
    dma_gather descriptor fetches a complete bilinear patch.
  - offset/attention projections, softmax, bilinear weights and cell
    indices computed query-major (PE transposes feed the matmuls).
  - gpsimd.dma_gather (mlp ucode library) fetches patches; ACT expands the
    4 patch-slot weights over head_dim; DVE multiplies and tree-reduces.
  - output projection + LayerNorm + FFN + LayerNorm, then DMA out.
"""

import numpy as np
from contextlib import ExitStack

EMB = 256
NH = 8
NL = 4
NPT = 4
HD = 32
DFFN = 1024
P = 128


def make_cfg(shapes, n_blk_q, grp):
    L = sum(h * w for h, w in shapes)
    starts = np.cumsum([0] + [h * w for h, w in shapes])[:-1].tolist()
    n_blk_full = -(-L // P)
    assert n_blk_q % grp == 0
    return dict(
        shapes=[tuple(s) for s in shapes], starts=starts, L=L,
        LPAD=n_blk_full * P, NBF=n_blk_full, NBQ=n_blk_q, HQ=n_blk_q * P,
        GRP=grp, NGRP=n_blk_q // grp,
    )


CFG_FULL = make_cfg([(100, 100), (50, 50), (25, 25), (13, 13)], 52, 1)
HALF = 6647


# ------------------------------------------------------- host-side consts ---

def host_constants(cfg):
    shapes, starts = cfg["shapes"], cfg["starts"]
    invnorm = np.zeros(EMB, np.float32)
    pixscale = np.zeros(EMB, np.float32)
    clipmax = np.zeros(EMB, np.float32)
    vmax = np.zeros(EMB, np.float32)
    for h in range(NH):
        for l, (H_, W_) in enumerate(shapes):
            for pt in range(NPT):
                base = h * (NL * NPT * 2) + l * (NPT * 2) + pt * 2
                invnorm[base + 0] = 1.0 / W_
                invnorm[base + 1] = 1.0 / H_
                pixscale[base + 0] = W_
                pixscale[base + 1] = H_
                clipmax[base + 0] = W_ - 2
                clipmax[base + 1] = H_ - 2
                vmax[base + 0] = W_ - 1
                vmax[base + 1] = H_ - 1
    cst_xy = np.stack([invnorm, pixscale, clipmax, vmax])

    wrow = np.zeros(P, np.float32)
    srow = np.zeros(P, np.float32)
    hrow = np.zeros(P, np.float32)
    L = cfg["L"]
    for h in range(NH):
        for l, (H_, W_) in enumerate(shapes):
            for pt in range(NPT):
                base = h * (NL * NPT) + l * NPT + pt
                wrow[base] = W_
                srow[base] = starts[l]
                hrow[base] = h * L
    cst_hlp = np.stack([wrow, srow, hrow])

    ident = np.eye(P, dtype=np.float32)
    ones_row = np.ones((1, P), np.float32)
    return dict(cst_xy=cst_xy, cst_hlp=cst_hlp, ident=ident,
                ones_row=ones_row)


# ------------------------------------------------------------- emission ---

def emit_kernel(tc, outs, ins, cfg):
    import concourse.bass as bass
    from concourse import mybir
    from concourse import library_config
    from concourse.tile_rust import add_dep_helper

    nc = tc.nc
    op = mybir.AluOpType
    act_f = mybir.ActivationFunctionType
    f32, f16 = mybir.dt.float32, mybir.dt.float16
    i16, i32 = mybir.dt.int16, mybir.dt.int32
    AX = mybir.AxisListType

    shapes, starts = cfg["shapes"], cfg["starts"]
    L, LPAD, NBF, NBQ, GRP, NGRP = (cfg[k] for k in
                                    ("L", "LPAD", "NBF", "NBQ", "GRP", "NGRP"))
    NIDX = GRP * P * NL * NPT
    ICOLS = NIDX // 16

    ctx = ExitStack()

    def dap(handle, offset, dims):
        return bass.AP(tensor=handle, offset=offset,
                       ap=[list(d) for d in dims])

    def sap(ap0, extra_off, dims):
        return bass.AP(tensor=ap0.tensor, offset=ap0.offset + extra_off,
                       ap=[list(d) for d in dims])

    # ---- internal DRAM ----
    valf16 = nc.dram_tensor("valf16", [LPAD, EMB], f16, kind="Internal")
    tableT = nc.dram_tensor("tableT", [NH * L, 4 * HD], f16, kind="Internal")
    idxdram = nc.dram_tensor("idxdram", [NGRP, 16, NH * ICOLS], i16,
                             kind="Internal")

    # ---- pools ----
    cpool = ctx.enter_context(tc.tile_pool(name="consts", bufs=1))
    apool = ctx.enter_context(tc.tile_pool(name="acts", bufs=3))
    wpool = ctx.enter_context(tc.tile_pool(name="wmath", bufs=1))
    gpool = ctx.enter_context(tc.tile_pool(name="gath", bufs=2))
    kpool = ctx.enter_context(tc.tile_pool(name="comb", bufs=2))
    opool = ctx.enter_context(tc.tile_pool(name="outp", bufs=2))
    ps_tr = ctx.enter_context(tc.tile_pool(name="ps_tr", bufs=2, space="PSUM"))
    ps_mm = ctx.enter_context(tc.tile_pool(name="ps_mm", bufs=2, space="PSUM"))
    ps_sm = ctx.enter_context(tc.tile_pool(name="ps_sm", bufs=2, space="PSUM"))
    ps_qh = ctx.enter_context(tc.tile_pool(name="ps_qh", bufs=1, space="PSUM"))

    def dma(out_ap, in_ap):
        nc.sync.dma_start(out=out_ap, in_=in_ap)

    # ---- constants / weights ----
    def load_w(name, k, n):
        t = cpool.tile([P, k // P, n], f32, name=f"s_{name}")
        dma(t, ins[name].rearrange("(a p) n -> p a n", p=P))
        return t

    Wval = load_w("W_val", EMB, EMB)
    Woff = load_w("W_off", EMB, EMB)
    Watt = load_w("W_attn", EMB, NH * NL * NPT)
    Wout = load_w("W_out", EMB, EMB)
    W1 = load_w("W1", EMB, DFFN)
    W2 = load_w("W2", DFFN, EMB)

    def load_row(name, n):
        t = cpool.tile([1, n], f32, name=f"r_{name}")
        dma(t, ins[name][:, :])
        return t

    bval = load_row("b_val", EMB)
    boff = load_row("b_off", EMB)
    batt = load_row("b_attn", NH * NL * NPT)
    bout = load_row("b_out", EMB)
    b1r = load_row("b1", DFFN)
    b2r = load_row("b2", EMB)
    onesr = load_row("ones_row", P)

    def load_bc(src_ap, n, name):
        t = cpool.tile([P, n], f32, name=f"b_{name}")
        dma(t, src_ap.to_broadcast([P, n]))
        return t

    ln1g = load_bc(ins["ln1_g"][:, :], EMB, "ln1g")
    ln1b = load_bc(ins["ln1_b"][:, :], EMB, "ln1b")
    ln2g = load_bc(ins["ln2_g"][:, :], EMB, "ln2g")
    ln2b = load_bc(ins["ln2_b"][:, :], EMB, "ln2b")
    c_invn = load_bc(ins["cst_xy"][0:1, :], EMB, "invn")
    c_pixs = load_bc(ins["cst_xy"][1:2, :], EMB, "pixs")
    c_clip = load_bc(ins["cst_xy"][2:3, :], EMB, "clip")
    c_vmax = load_bc(ins["cst_xy"][3:4, :], EMB, "vmax")
    c_W = load_bc(ins["cst_hlp"][0:1, :], P, "cw")
    c_S = load_bc(ins["cst_hlp"][1:2, :], P, "cs")
    c_HL = load_bc(ins["cst_hlp"][2:3, :], P, "chl")

    ident = cpool.tile([P, P], f32, name="ident")
    dma(ident, ins["ident"][:, :])
    eps_t = cpool.tile([P, 1], f32, name="eps_t")
    nc.vector.memset(eps_t[:, :], 1e-5)

    refr = cpool.tile([P, NBQ, 2 * NL], f32, name="refr")
    dma(refr, ins["ref_q"].rearrange("(b p) l c -> p b (l c)", p=P))

    def mm(psum_ap, pairs, bias=None):
        seq = list(pairs)
        if bias is not None:
            seq.append((onesr[:1, :psum_ap.shape[0]], bias))
        for i, (lt, rt) in enumerate(seq):
            nc.tensor.matmul(psum_ap, lt, rt,
                             start=(i == 0), stop=(i == len(seq) - 1))

    # ======================= P1: value projection =======================
    idf16 = cpool.tile([P, P], f16, name="idf16")
    nc.vector.tensor_copy(idf16[:, :], ident[:, :])
    for blk in range(NBF):
        fv = apool.tile([P, EMB], f16, name="fv", tag="fv")
        dma(fv, ins["feat_val"][blk * P:(blk + 1) * P, :])
        ftp = ps_tr.tile([P, 2, P], f16, name="ftp", tag="tr")
        nc.tensor.transpose(ftp[:, 0, :], fv[:, 0:P], idf16[:, :])
        nc.tensor.transpose(ftp[:, 1, :], fv[:, P:EMB], idf16[:, :])
        fts = apool.tile([P, 2, P], f32, name="fts", tag="fts")
        nc.vector.tensor_copy(fts[:, :, :], ftp[:, :, :])
        vp = ps_mm.tile([P, EMB], f32, name="vp", tag="mm")
        mm(vp, [(fts[:, 0, :], Wval[:, 0, :]), (fts[:, 1, :], Wval[:, 1, :])],
           bias=bval[:1, :])
        vf = apool.tile([P, EMB], f16, name="vf", tag="vf")
        nc.vector.tensor_copy(vf[:, :], vp[:, :])
        dma(valf16.ap()[blk * P:(blk + 1) * P, :], vf)

    # ======================= P2: patch-table build ======================
    for h in range(NH):
        for l, (H_, W_) in enumerate(shapes):
            s = starts[l]
            for cy in (0, 1):
                for cx in (0, 1):
                    c = cy * 2 + cx
                    src = dap(valf16, (s + cy * W_ + cx) * EMB + h * HD,
                              [[W_ * EMB, H_ - 1], [EMB, W_ - 1], [1, HD]])
                    dst = dap(tableT, (h * L + s) * 4 * HD + c * HD,
                              [[W_ * 4 * HD, H_ - 1], [4 * HD, W_ - 1],
                               [1, HD]])
                    dma(dst, src)
            # fill never-gathered edge records (x=W-1 col, y=H-1 row) so the
            # table contains no uninitialized (possibly non-finite) bytes
            dma(dap(tableT, (h * L + s + W_ - 1) * 4 * HD,
                    [[W_ * 4 * HD, H_], [HD, 4], [1, HD]]),
                dap(valf16, (s + W_ - 1) * EMB + h * HD,
                    [[W_ * EMB, H_], [0, 4], [1, HD]]))
            dma(dap(tableT, (h * L + s + (H_ - 1) * W_) * 4 * HD,
                    [[4 * HD, W_ - 1], [HD, 4], [1, HD]]),
                dap(valf16, (s + (H_ - 1) * W_) * EMB + h * HD,
                    [[EMB, W_ - 1], [0, 4], [1, HD]]))

    # ==================== per-block frontend ====================
    def emit_frontend(blk):
        fq = apool.tile([P, EMB], f32, name="fq", tag="fq", bufs=GRP + 2)
        dma(fq, ins["feat_q"][blk * P:(blk + 1) * P, :])
        pq = apool.tile([P, EMB], f16, name="pq", tag="pq")
        dma(pq, ins["pos_q"][blk * P:(blk + 1) * P, :])
        qb = apool.tile([P, EMB], f32, name="qb", tag="qb")
        nc.vector.tensor_add(qb[:, :], fq[:, :], pq[:, :])

        qtp = ps_tr.tile([P, 2, P], f32, name="qtp", tag="tr")
        nc.tensor.transpose(qtp[:, 0, :], qb[:, 0:P], ident[:, :])
        nc.tensor.transpose(qtp[:, 1, :], qb[:, P:EMB], ident[:, :])
        qts = apool.tile([P, 2, P], f32, name="qts", tag="qts", bufs=2)
        nc.vector.tensor_copy(qts[:, :, :], qtp[:, :, :])

        offp = ps_mm.tile([P, EMB], f32, name="offp", tag="mm")
        mm(offp, [(qts[:, 0, :], Woff[:, 0, :]), (qts[:, 1, :], Woff[:, 1, :])],
           bias=boff[:1, :])
        off = wpool.tile([P, EMB], f32, name="off", tag="off")
        nc.vector.tensor_copy(off[:, :], offp[:, :])

        attp = ps_sm.tile([P, NH * 16], f32, name="attp", tag="sm")
        mm(attp, [(qts[:, 0, :], Watt[:, 0, :]), (qts[:, 1, :], Watt[:, 1, :])],
           bias=batt[:1, :])
        att = wpool.tile([P, NH, 16], f32, name="att", tag="att")
        nc.vector.tensor_copy(att[:, :, :], attp[:, :].rearrange(
            "p (h l) -> p h l", h=NH))

        # softmax over (l,pt) per head
        rmax = wpool.tile([P, NH], f32, name="rmax", tag="rmax")
        nc.vector.reduce_max(rmax[:, :], att[:, :, :], axis=AX.X)
        exv = wpool.tile([P, NH, 16], f32, name="exv", tag="exv")
        rmaxa = rmax[:, :]
        nc.vector.tensor_sub(exv[:, :, :], att[:, :, :],
                             sap(rmaxa, 0, [rmaxa.ap[0], [1, NH], [0, 16]]))
        nc.scalar.activation(exv[:, :, :], exv[:, :, :], act_f.Exp)
        ssum = wpool.tile([P, NH], f32, name="ssum", tag="ssum")
        nc.vector.reduce_sum(ssum[:, :], exv[:, :, :], axis=AX.X)
        rsum = wpool.tile([P, NH], f32, name="rsum", tag="rsum")
        nc.vector.reciprocal(rsum[:, :], ssum[:, :])
        aw = wpool.tile([P, NH, 16], f32, name="aw", tag="aw")
        rsuma = rsum[:, :]
        nc.vector.tensor_mul(aw[:, :, :], exv[:, :, :],
                             sap(rsuma, 0, [rsuma.ap[0], [1, NH], [0, 16]]))

        def wt(name):
            return wpool.tile([P, EMB], f32, name=name, tag=name)

        loc = wt("loc")
        nc.vector.tensor_mul(loc[:, :], off[:, :], c_invn[:, :])
        refa = refr[:, blk, :]
        for xy in (0, 1):
            lvh = sap(loc[:, :], xy, [loc[:, :].ap[0], [32, NH], [8, NL],
                                      [2, NPT]])
            nc.vector.tensor_add(lvh, lvh,
                                 sap(refa, xy, [refa.ap[0], [0, NH], [2, NL],
                                                [0, NPT]]))
        pix = wt("pix")
        nc.vector.tensor_mul(pix[:, :], loc[:, :], c_pixs[:, :])
        nc.vector.tensor_scalar_add(pix[:, :], pix[:, :], -0.5)

        # floor(pix) robust to cast rounding mode
        xi = wpool.tile([P, EMB], i32, name="xi", tag="xi")
        nc.vector.tensor_copy(xi[:, :], pix[:, :])
        base = wt("base")
        nc.vector.tensor_copy(base[:, :], xi[:, :])
        fixm = wt("fixm")
        nc.vector.tensor_tensor(fixm[:, :], pix[:, :], base[:, :], op=op.is_lt)
        nc.vector.tensor_sub(base[:, :], base[:, :], fixm[:, :])
        wfrac = wt("wfrac")
        nc.vector.tensor_sub(wfrac[:, :], pix[:, :], base[:, :])

        basec = wt("basec")
        nc.vector.tensor_scalar_max(basec[:, :], base[:, :], 0.0)
        nc.vector.tensor_tensor(basec[:, :], basec[:, :], c_clip[:, :],
                                op=op.min)

        v0b = wt("v0b")
        nc.vector.tensor_tensor(v0b[:, :], base[:, :], c_vmax[:, :],
                                op=op.is_le)
        vld0 = wt("vld0")
        nc.vector.scalar_tensor_tensor(vld0[:, :], base[:, :], 0.0, v0b[:, :],
                                       op0=op.is_ge, op1=op.mult)
        v1b = wt("v1b")
        nc.vector.tensor_tensor(v1b[:, :], base[:, :], c_clip[:, :],
                                op=op.is_le)
        vld1 = wt("vld1")
        nc.vector.scalar_tensor_tensor(vld1[:, :], base[:, :], -1.0, v1b[:, :],
                                       op0=op.is_ge, op1=op.mult)

        tsh = wt("tsh")
        nc.vector.tensor_sub(tsh[:, :], base[:, :], basec[:, :])
        e0 = wt("e0")
        nc.vector.tensor_scalar(e0[:, :], tsh[:, :], 0.0, None,
                                op0=op.is_equal)
        em1 = wt("em1")
        nc.vector.tensor_scalar(em1[:, :], tsh[:, :], -1.0, None,
                                op0=op.is_equal)
        ep1 = wt("ep1")
        nc.vector.tensor_scalar(ep1[:, :], tsh[:, :], 1.0, None,
                                op0=op.is_equal)

        u0 = wt("u0")
        nc.vector.tensor_scalar(u0[:, :], wfrac[:, :], -1.0, 1.0, op0=op.mult,
                                op1=op.add)
        nc.vector.tensor_mul(u0[:, :], u0[:, :], vld0[:, :])
        u1 = wt("u1")
        nc.vector.tensor_mul(u1[:, :], wfrac[:, :], vld1[:, :])

        a0 = wt("a0")
        nc.vector.tensor_mul(a0[:, :], u0[:, :], e0[:, :])
        t1 = wt("t1")
        nc.vector.tensor_mul(t1[:, :], u1[:, :], em1[:, :])
        nc.vector.tensor_add(a0[:, :], a0[:, :], t1[:, :])
        a1 = wt("a1")
        nc.vector.tensor_mul(a1[:, :], u0[:, :], ep1[:, :])
        nc.vector.tensor_mul(t1[:, :], u1[:, :], e0[:, :])
        nc.vector.tensor_add(a1[:, :], a1[:, :], t1[:, :])

        def ycols(t):
            return sap(t[:, :], 1, [[t[:, :].ap[0][0], P], [2, P]])

        def xcols(t):
            return sap(t[:, :], 0, [[t[:, :].ap[0][0], P], [2, P]])

        awf = aw.rearrange("p h l -> p (h l)")
        ay0 = wpool.tile([P, P], f32, name="ay0", tag="ay0")
        nc.vector.tensor_mul(ay0[:, :], ycols(a0), awf)
        ay1 = wpool.tile([P, P], f32, name="ay1", tag="ay1")
        nc.vector.tensor_mul(ay1[:, :], ycols(a1), awf)

        w4 = wpool.tile([P, P, 4], f16, name="w4", tag="w4", bufs=GRP + 1)
        nc.vector.tensor_mul(w4[:, :, 0], ay0[:, :], xcols(a0))
        nc.vector.tensor_mul(w4[:, :, 1], ay0[:, :], xcols(a1))
        nc.vector.tensor_mul(w4[:, :, 2], ay1[:, :], xcols(a0))
        nc.vector.tensor_mul(w4[:, :, 3], ay1[:, :], xcols(a1))

        cell = wpool.tile([P, P], f32, name="cell", tag="cell")
        nc.vector.tensor_mul(cell[:, :], ycols(basec), c_W[:, :])
        nc.vector.tensor_add(cell[:, :], cell[:, :], xcols(basec))
        nc.vector.tensor_add(cell[:, :], cell[:, :], c_S[:, :])

        nc.vector.tensor_add(cell[:, :], cell[:, :], c_HL[:, :])
        offs = wpool.tile([P, P], i32, name="offs", tag="offs", bufs=2)
        nc.vector.tensor_copy(offs[:, :], cell[:, :])
        return fq, w4, offs

    # ==================== LayerNorm ====================
    def emit_ln(r, gt, bt, pfx):
        nsum = opool.tile([P, 1], f32, name=f"{pfx}ns", tag=f"{pfx}ns")
        nc.vector.tensor_reduce(nsum[:, :], r[:, :], axis=AX.X, op=op.add,
                                negate=True)
        nmean = opool.tile([P, 1], f32, name=f"{pfx}nm", tag=f"{pfx}nm")
        nc.scalar.mul(nmean[:, :], nsum[:, :], 1.0 / EMB)
        c = opool.tile([P, EMB], f32, name=f"{pfx}c", tag=f"{pfx}c")
        nc.vector.tensor_scalar_add(c[:, :], r[:, :], nmean[:, :])
        csq = opool.tile([P, EMB], f32, name=f"{pfx}sq", tag=f"{pfx}sq")
        ssq = opool.tile([P, 1], f32, name=f"{pfx}ssq", tag=f"{pfx}ssq")
        nc.scalar.activation(csq[:, :], c[:, :], act_f.Square,
                             accum_out=ssq[:, :])
        std = opool.tile([P, 1], f32, name=f"{pfx}std", tag=f"{pfx}std")
        nc.scalar.activation(std[:, :], ssq[:, :], act_f.Sqrt,
                             bias=eps_t[:, :], scale=1.0 / EMB)
        rstd = opool.tile([P, 1], f32, name=f"{pfx}rs", tag=f"{pfx}rs")
        nc.vector.reciprocal(rstd[:, :], std[:, :])
        x = opool.tile([P, EMB], f32, name=f"{pfx}x", tag=f"{pfx}x")
        nc.vector.scalar_tensor_tensor(x[:, :], c[:, :], rstd[:, :], gt[:, :],
                                       op0=op.mult, op1=op.mult)
        nc.vector.tensor_add(x[:, :], x[:, :], bt[:, :])
        return x

    # ==================== per-group pipeline ====================
    def emit_group(g):
        blk = g
        fq, w4, offs = emit_frontend(blk)
        gb = gpool.tile([P, P, 4 * HD], f16, name="gb", tag="gb", bufs=2)
        for s in range(P):
            nc.gpsimd.indirect_dma_start(
                out=gb[:, s, :], out_offset=None,
                in_=tableT.ap()[:, :],
                in_offset=bass.IndirectOffsetOnAxis(ap=offs[:, s:s + 1],
                                                    axis=0))

        acat = kpool.tile([P, EMB], f32, name="acat", tag="acat")
        # all-heads combine, reduction tree folded in place inside gb
        gba = gb[:, :, :]
        pstr = gba.ap[0][0]

        def gsl(off, dims):
            return sap(gba, off, [[pstr, P]] + dims)

        # weights: w4 [P, (h,lp), 4] broadcast over head_dim (0-stride)
        w4b = sap(w4[:, :, :], 0,
                  [[w4[:, :, :].ap[0][0], P], [4, P], [1, 4], [0, HD]])
        gall = gsl(0, [[128, P], [HD, 4], [1, HD]])
        nc.vector.tensor_mul(gall, gall, w4b)
        # corner folds: c0+=c1, c2+=c3, c0+=c2
        d2 = [[128, P], [1, HD]]
        nc.vector.tensor_add(gsl(0, d2), gsl(0, d2), gsl(HD, d2))
        nc.vector.tensor_add(gsl(2 * HD, d2), gsl(2 * HD, d2), gsl(3 * HD, d2))
        nc.vector.tensor_add(gsl(0, d2), gsl(0, d2), gsl(2 * HD, d2))
        # lp folds: 16 -> 8 -> 4 -> 2 (per head; h stride 16*128)
        for w in (8, 4, 2):
            dh = [[16 * 128, NH], [128, w], [1, HD]]
            nc.vector.tensor_add(gsl(0, dh), gsl(0, dh), gsl(w * 128, dh))
        # final fold writes the fp32 attention output slice layout
        acv = sap(acat[:, :], 0, [[acat[:, :].ap[0][0], P], [HD, NH], [1, HD]])
        dh1 = [[16 * 128, NH], [1, HD]]
        nc.vector.tensor_add(acv, gsl(0, dh1), gsl(128, dh1))
        acats = [(blk, fq, acat)]

        # ---- output projection + LN + FFN + LN ----
        for blk, fq, acat in acats:
            atp = ps_tr.tile([P, 2, P], f32, name="atp", tag="tr")
            nc.tensor.transpose(atp[:, 0, :], acat[:, 0:P], ident[:, :])
            nc.tensor.transpose(atp[:, 1, :], acat[:, P:EMB], ident[:, :])
            ats = opool.tile([P, 2, P], f32, name="ats", tag="ats")
            nc.vector.tensor_copy(ats[:, :, :], atp[:, :, :])
            oprj = ps_mm.tile([P, EMB], f32, name="oprj", tag="mm")
            mm(oprj, [(ats[:, 0, :], Wout[:, 0, :]),
                      (ats[:, 1, :], Wout[:, 1, :])], bias=bout[:1, :])

            r1 = opool.tile([P, EMB], f32, name="r1", tag="r1")
            nc.vector.tensor_add(r1[:, :], oprj[:, :], fq[:, :])
            x1 = emit_ln(r1, ln1g, ln1b, "la")

            xtp = ps_tr.tile([P, 2, P], f32, name="xtp", tag="tr")
            nc.tensor.transpose(xtp[:, 0, :], x1[:, 0:P], ident[:, :])
            nc.tensor.transpose(xtp[:, 1, :], x1[:, P:EMB], ident[:, :])
            xts = opool.tile([P, 2, P], f32, name="xts", tag="xts")
            nc.vector.tensor_copy(xts[:, :, :], xtp[:, :, :])

            h1s = opool.tile([P, DFFN // P, P], f32, name="h1s", tag="h1s")
            hp = ps_mm.tile([P, DFFN // P, P], f32, name="hp", tag="hpw", bufs=1)
            for mt in range(DFFN // P):
                nc.tensor.matmul(hp[:, mt, :], W1[:, 0, mt * P:(mt + 1) * P],
                                 xts[:, 0, :], start=True, stop=False)
                nc.tensor.matmul(hp[:, mt, :], W1[:, 1, mt * P:(mt + 1) * P],
                                 xts[:, 1, :], start=False, stop=False)
                nc.tensor.matmul(hp[:, mt, :], b1r[:1, mt * P:(mt + 1) * P],
                                 onesr[:1, :], start=False, stop=True)
            nc.scalar.activation(h1s[:, :, :], hp[:, :, :], act_f.Relu)

            yp = ps_mm.tile([P, EMB], f32, name="yp", tag="mm")
            for mt in range(DFFN // P):
                nc.tensor.matmul(yp[:, :], h1s[:, mt, :], W2[:, mt, :],
                                 start=(mt == 0), stop=False)
            nc.tensor.matmul(yp[:, :], onesr[:1, :], b2r[:1, :],
                             start=False, stop=True)

            r2 = opool.tile([P, EMB], f32, name="r2", tag="r2")
            nc.vector.tensor_add(r2[:, :], yp[:, :], x1[:, :])
            x2 = emit_ln(r2, ln2g, ln2b, "lb")
            dma(outs["out_q"][blk * P:(blk + 1) * P, :], x2)

    for g in range(NGRP):
        emit_group(g)

    ctx.close()


# ------------------------------------------------------------ host entry ---

_CACHE = {}


def build_nc(cfg):
    import concourse.bass as bass
    from concourse import bacc, mybir, tile

    nc = bacc.Bacc("TRN2", debug=False)
    f32 = mybir.dt.float32

    f16 = mybir.dt.float16

    def di(name, shape, dt=None):
        return nc.dram_tensor(name, list(shape), dt or f32,
                              kind="ExternalInput").ap()

    HQ, LPAD = cfg["HQ"], cfg["LPAD"]
    ins = dict(
        feat_val=di("feat_val", [LPAD, EMB], f16),
        feat_q=di("feat_q", [HQ, EMB]),
        pos_q=di("pos_q", [HQ, EMB], f16),
        ref_q=di("ref_q", [HQ, NL, 2]),
        W_val=di("W_val", [EMB, EMB]), b_val=di("b_val", [1, EMB]),
        W_off=di("W_off", [EMB, EMB]), b_off=di("b_off", [1, EMB]),
        W_attn=di("W_attn", [EMB, NH * NL * NPT]),
        b_attn=di("b_attn", [1, NH * NL * NPT]),
        W_out=di("W_out", [EMB, EMB]), b_out=di("b_out", [1, EMB]),
        W1=di("W1", [EMB, DFFN]), b1=di("b1", [1, DFFN]),
        W2=di("W2", [DFFN, EMB]), b2=di("b2", [1, EMB]),
        ln1_g=di("ln1_g", [1, EMB]), ln1_b=di("ln1_b", [1, EMB]),
        ln2_g=di("ln2_g", [1, EMB]), ln2_b=di("ln2_b", [1, EMB]),
        cst_xy=di("cst_xy", [4, EMB]),
        cst_hlp=di("cst_hlp", [3, P]),
        ident=di("ident", [P, P]),
        ones_row=di("ones_row", [1, P]),
    )
    outs = dict(
        out_q=nc.dram_tensor("out_q", [HQ, EMB], f32,
                             kind="ExternalOutput").ap(),
    )
    with tile.TileContext(nc) as tc:
        emit_kernel(tc, outs, ins, cfg)
    nc.compile()
    return nc


def make_in_maps(inputs, cfg):
    feats = np.asarray(inputs["features"], np.float32)
    pos = np.asarray(inputs["pos"], np.float32)
    refp = np.asarray(inputs["reference_points"], np.float32)
    B = feats.shape[0]
    HQ, LPAD, L = cfg["HQ"], cfg["LPAD"], cfg["L"]
    half = L // 2

    consts = host_constants(cfg)
    wkeys = dict(
        W_val=inputs["W_val"], b_val=np.reshape(inputs["b_val"], (1, -1)),
        W_off=inputs["W_off"], b_off=np.reshape(inputs["b_off"], (1, -1)),
        W_attn=inputs["W_attn"], b_attn=np.reshape(inputs["b_attn"], (1, -1)),
        W_out=inputs["W_out"], b_out=np.reshape(inputs["b_out"], (1, -1)),
        W1=inputs["W1"], b1=np.reshape(inputs["b1"], (1, -1)),
        W2=inputs["W2"], b2=np.reshape(inputs["b2"], (1, -1)),
        ln1_g=np.reshape(inputs["ln1_g"], (1, -1)),
        ln1_b=np.reshape(inputs["ln1_b"], (1, -1)),
        ln2_g=np.reshape(inputs["ln2_g"], (1, -1)),
        ln2_b=np.reshape(inputs["ln2_b"], (1, -1)),
    )
    wkeys = {k: np.ascontiguousarray(np.asarray(v, np.float32))
             for k, v in wkeys.items()}

    halves = [(0, half), (half, L)]
    in_maps = []
    for core in range(2 * B):
        b, hf = core // 2, core % 2
        s, e = halves[hf]
        fv = np.zeros((LPAD, EMB), np.float16)
        fv[:L] = feats[b].astype(np.float16)
        fq = np.zeros((HQ, EMB), np.float32)
        fq[:e - s] = feats[b, s:e]
        pq = np.zeros((HQ, EMB), np.float16)
        pq[:e - s] = pos[b, s:e].astype(np.float16)
        rq = np.zeros((HQ, NL, 2), np.float32)
        rq[:e - s] = refp[b, s:e]
        m = dict(feat_val=fv, feat_q=fq, pos_q=pq, ref_q=rq)
        m.update(wkeys)
        m.update({k: np.ascontiguousarray(np.asarray(v, np.float32))
                  for k, v in consts.items()})
        in_maps.append(m)
    return in_maps, halves


def kernel(**inputs):
    from concourse import bass_utils

    cfg = CFG_FULL
    in_maps, halves = make_in_maps(inputs, cfg)
    B = np.asarray(inputs["features"]).shape[0]
    L = cfg["L"]

    if "nc" not in _CACHE:
        _CACHE["nc"] = build_nc(cfg)
    nc = _CACHE["nc"]

    res = bass_utils.run_bass_kernel_spmd(nc, in_maps,
                                          core_ids=list(range(2 * B)))
    out = np.zeros((B, L, EMB), np.float32)
    for core in range(2 * B):
        b, hf = core // 2, core % 2
        s, e = halves[hf]
        out[b, s:e] = res.results[core]["out_q"][:e - s]
    return out



# revision 5
# speedup vs baseline: 836.6328x; 836.6328x over previous
"""Trainium2 Bass kernel for a Deformable-DETR style encoder block.

Sharding: 8 NeuronCores = 4 batch samples x 2 query-halves.

Per core:
  - value projection over the full sample -> fp16 "patch table" in DRAM
    (one DRAM tensor per pyramid level, small levels first so their
    tables finish early): for cell (y,x) and head h the 2x2 neighborhood
    [V[y,x], V[y,x+1], V[y+1,x], V[y+1,x+1]] is packed contiguously
    (4*32 fp16 = 256B), so one dma_gather descriptor fetches a complete
    bilinear patch.
  - offset/attention projections, softmax, bilinear weights and cell
    indices computed query-major (PE transposes feed the matmuls).
  - gpsimd.indirect_dma_start fetches patches; the frontend for block
    g+1 is emitted ahead of the combine/backend of block g so the
    gather stream on GpSimd never waits on the PE/DVE tail of the
    previous block.
  - DVE multiplies by bilinear*attention weights and tree-reduces.
  - output projection + LayerNorm + FFN + LayerNorm, then DMA out.
"""

import numpy as np
from contextlib import ExitStack

EMB = 256
NH = 8
NL = 4
NPT = 4
HD = 32
DFFN = 1024
P = 128

# value/table build order: small levels first so tables are ready early
LVL_ORDER = [3, 2, 1, 0]


def make_cfg(shapes, n_blk_q):
    L = sum(h * w for h, w in shapes)
    hw = [h * w for h, w in shapes]
    # per-level block-aligned padded lengths, laid out in LVL_ORDER
    pblk = {l: -(-hw[l] // P) for l in range(NL)}
    vstart = {}
    off = 0
    for l in LVL_ORDER:
        vstart[l] = off
        off += pblk[l] * P
    return dict(
        shapes=[tuple(s) for s in shapes], hw=hw, L=L,
        pblk=pblk, vstart=vstart, LPAD=off, NBF=off // P,
        NBQ=n_blk_q, HQ=n_blk_q * P,
    )


CFG_FULL = make_cfg([(100, 100), (50, 50), (25, 25), (13, 13)], 52)


# ------------------------------------------------------- host-side consts ---

def host_constants(cfg):
    shapes, hw = cfg["shapes"], cfg["hw"]
    invnorm = np.zeros(EMB, np.float32)
    pixscale = np.zeros(EMB, np.float32)
    clipmax = np.zeros(EMB, np.float32)
    vmax = np.zeros(EMB, np.float32)
    for h in range(NH):
        for l, (H_, W_) in enumerate(shapes):
            for pt in range(NPT):
                base = h * (NL * NPT * 2) + l * (NPT * 2) + pt * 2
                invnorm[base + 0] = 1.0 / W_
                invnorm[base + 1] = 1.0 / H_
                pixscale[base + 0] = W_
                pixscale[base + 1] = H_
                clipmax[base + 0] = W_ - 2
                clipmax[base + 1] = H_ - 2
                vmax[base + 0] = W_ - 1
                vmax[base + 1] = H_ - 1
    cst_xy = np.stack([invnorm, pixscale, clipmax, vmax])

    wrow = np.zeros(P, np.float32)
    srow = np.zeros(P, np.float32)
    hrow = np.zeros(P, np.float32)
    for h in range(NH):
        for l, (H_, W_) in enumerate(shapes):
            for pt in range(NPT):
                base = h * (NL * NPT) + l * NPT + pt
                wrow[base] = W_
                srow[base] = 0.0
                hrow[base] = h * hw[l]
    cst_hlp = np.stack([wrow, srow, hrow])

    ident = np.eye(P, dtype=np.float32)
    ones_row = np.ones((1, P), np.float32)
    return dict(cst_xy=cst_xy, cst_hlp=cst_hlp, ident=ident,
                ones_row=ones_row)


# ------------------------------------------------------------- emission ---

def emit_kernel(tc, outs, ins, cfg):
    import concourse.bass as bass
    from concourse import mybir

    assert bass is not None
    nc = tc.nc
    op = mybir.AluOpType
    act_f = mybir.ActivationFunctionType
    f32, f16 = mybir.dt.float32, mybir.dt.float16
    i32 = mybir.dt.int32
    AX = mybir.AxisListType

    shapes, hw = cfg["shapes"], cfg["hw"]
    NBF, NBQ = cfg["NBF"], cfg["NBQ"]
    pblk, vstart = cfg["pblk"], cfg["vstart"]

    ctx = ExitStack()

    def dap(handle, offset, dims):
        return bass.AP(tensor=handle, offset=offset,
                       ap=[list(d) for d in dims])

    def sap(ap0, extra_off, dims):
        return bass.AP(tensor=ap0.tensor, offset=ap0.offset + extra_off,
                       ap=[list(d) for d in dims])

    # ---- internal DRAM: per-level value + patch table ----
    valf = {l: nc.dram_tensor(f"valf{l}", [pblk[l] * P, EMB], f16,
                              kind="Internal") for l in range(NL)}
    tabl = {l: nc.dram_tensor(f"tabl{l}", [NH * hw[l], 4 * HD], f16,
                              kind="Internal") for l in range(NL)}

    # ---- pools ----
    cpool = ctx.enter_context(tc.tile_pool(name="consts", bufs=1))
    apool = ctx.enter_context(tc.tile_pool(name="acts", bufs=3))
    wpool = ctx.enter_context(tc.tile_pool(name="wmath", bufs=1))
    gpool = ctx.enter_context(tc.tile_pool(name="gath", bufs=2))
    kpool = ctx.enter_context(tc.tile_pool(name="comb", bufs=2))
    opool = ctx.enter_context(tc.tile_pool(name="outp", bufs=2))
    ps_tr = ctx.enter_context(tc.tile_pool(name="ps_tr", bufs=2, space="PSUM"))
    ps_mm = ctx.enter_context(tc.tile_pool(name="ps_mm", bufs=2, space="PSUM"))
    ps_sm = ctx.enter_context(tc.tile_pool(name="ps_sm", bufs=1, space="PSUM"))

    def dma(out_ap, in_ap):
        nc.sync.dma_start(out=out_ap, in_=in_ap)

    # ---- constants / weights ----
    def load_w(name, k, n):
        t = cpool.tile([P, k // P, n], f32, name=f"s_{name}")
        dma(t, ins[name].rearrange("(a p) n -> p a n", p=P))
        return t

    Wval = load_w("W_val", EMB, EMB)
    Woff = load_w("W_off", EMB, EMB)
    Watt = load_w("W_attn", EMB, NH * NL * NPT)
    Wout = load_w("W_out", EMB, EMB)
    W1 = load_w("W1", EMB, DFFN)
    W2 = load_w("W2", DFFN, EMB)

    def load_row(name, n):
        t = cpool.tile([1, n], f32, name=f"r_{name}")
        dma(t, ins[name][:, :])
        return t

    bval = load_row("b_val", EMB)
    boff = load_row("b_off", EMB)
    batt = load_row("b_attn", NH * NL * NPT)
    bout = load_row("b_out", EMB)
    b1r = load_row("b1", DFFN)
    b2r = load_row("b2", EMB)
    onesr = load_row("ones_row", P)

    def load_bc(src_ap, n, name):
        t = cpool.tile([P, n], f32, name=f"b_{name}")
        dma(t, src_ap.to_broadcast([P, n]))
        return t

    ln1g = load_bc(ins["ln1_g"][:, :], EMB, "ln1g")
    ln1b = load_bc(ins["ln1_b"][:, :], EMB, "ln1b")
    ln2g = load_bc(ins["ln2_g"][:, :], EMB, "ln2g")
    ln2b = load_bc(ins["ln2_b"][:, :], EMB, "ln2b")
    c_invn = load_bc(ins["cst_xy"][0:1, :], EMB, "invn")
    c_pixs = load_bc(ins["cst_xy"][1:2, :], EMB, "pixs")
    c_clip = load_bc(ins["cst_xy"][2:3, :], EMB, "clip")
    c_vmax = load_bc(ins["cst_xy"][3:4, :], EMB, "vmax")
    c_W = load_bc(ins["cst_hlp"][0:1, :], P, "cw")
    c_S = load_bc(ins["cst_hlp"][1:2, :], P, "cs")
    c_HL = load_bc(ins["cst_hlp"][2:3, :], P, "chl")

    ident = cpool.tile([P, P], f32, name="ident")
    dma(ident, ins["ident"][:, :])
    eps_t = cpool.tile([P, 1], f32, name="eps_t")
    nc.vector.memset(eps_t[:, :], 1e-5)

    refr = cpool.tile([P, NBQ, 2 * NL], f32, name="refr")
    dma(refr, ins["ref_q"].rearrange("(b p) l c -> p b (l c)", p=P))

    def mm(psum_ap, pairs, bias=None):
        seq = list(pairs)
        if bias is not None:
            seq.append((onesr[:1, :psum_ap.shape[0]], bias))
        for i, (lt, rt) in enumerate(seq):
            nc.tensor.matmul(psum_ap, lt, rt,
                             start=(i == 0), stop=(i == len(seq) - 1))

    # ======================= patch-table build (per level) ==============
    def emit_table(l):
        H_, W_ = shapes[l]
        vt, tt = valf[l], tabl[l]
        for h in range(NH):
            for cy in (0, 1):
                for cx in (0, 1):
                    c = cy * 2 + cx
                    src = dap(vt, (cy * W_ + cx) * EMB + h * HD,
                              [[W_ * EMB, H_ - 1], [EMB, W_ - 1], [1, HD]])
                    dst = dap(tt, (h * hw[l]) * 4 * HD + c * HD,
                              [[W_ * 4 * HD, H_ - 1], [4 * HD, W_ - 1],
                               [1, HD]])
                    dma(dst, src)
            # fill never-gathered edge records (x=W-1 col, y=H-1 row) so the
            # table contains no uninitialized (possibly non-finite) bytes
            dma(dap(tt, (h * hw[l] + W_ - 1) * 4 * HD,
                    [[W_ * 4 * HD, H_], [HD, 4], [1, HD]]),
                dap(vt, (W_ - 1) * EMB + h * HD,
                    [[W_ * EMB, H_], [0, 4], [1, HD]]))
            dma(dap(tt, (h * hw[l] + (H_ - 1) * W_) * 4 * HD,
                    [[4 * HD, W_ - 1], [HD, 4], [1, HD]]),
                dap(vt, ((H_ - 1) * W_) * EMB + h * HD,
                    [[EMB, W_ - 1], [0, 4], [1, HD]]))

    # ======================= value projection ===========================
    def emit_value():
        idf16 = cpool.tile([P, P], f16, name="idf16")
        nc.vector.tensor_copy(idf16[:, :], ident[:, :])
        # global block -> (level, local block)
        sched = []
        for l in LVL_ORDER:
            for lb in range(pblk[l]):
                sched.append((l, lb, lb == pblk[l] - 1))
        for blk, (l, lb, last) in enumerate(sched):
            fv = apool.tile([P, EMB], f16, name="fv", tag="fv")
            dma(fv, ins["feat_val"][blk * P:(blk + 1) * P, :])
            ftp = ps_tr.tile([P, 2, P], f16, name="ftp", tag="tr")
            nc.tensor.transpose(ftp[:, 0, :], fv[:, 0:P], idf16[:, :])
            nc.tensor.transpose(ftp[:, 1, :], fv[:, P:EMB], idf16[:, :])
            fts = apool.tile([P, 2, P], f32, name="fts", tag="fts")
            nc.vector.tensor_copy(fts[:, :, :], ftp[:, :, :])
            vp = ps_mm.tile([P, EMB], f32, name="vp", tag="mm")
            mm(vp, [(fts[:, 0, :], Wval[:, 0, :]),
                    (fts[:, 1, :], Wval[:, 1, :])], bias=bval[:1, :])
            vf = apool.tile([P, EMB], f16, name="vf", tag="vf")
            nc.vector.tensor_copy(vf[:, :], vp[:, :])
            dma(valf[l].ap()[lb * P:(lb + 1) * P, :], vf)
            if last:
                emit_table(l)

    # ==================== per-block frontend ====================
    def emit_frontend(blk):
        fq = apool.tile([P, EMB], f32, name="fq", tag="fq", bufs=4)
        dma(fq, ins["feat_q"][blk * P:(blk + 1) * P, :])
        pq = apool.tile([P, EMB], f16, name="pq", tag="pq")
        dma(pq, ins["pos_q"][blk * P:(blk + 1) * P, :])
        qb = apool.tile([P, EMB], f32, name="qb", tag="qb")
        nc.vector.tensor_add(qb[:, :], fq[:, :], pq[:, :])

        qtp = ps_tr.tile([P, 2, P], f32, name="qtp", tag="tr")
        nc.tensor.transpose(qtp[:, 0, :], qb[:, 0:P], ident[:, :])
        nc.tensor.transpose(qtp[:, 1, :], qb[:, P:EMB], ident[:, :])
        qts = apool.tile([P, 2, P], f32, name="qts", tag="qts", bufs=2)
        nc.vector.tensor_copy(qts[:, :, :], qtp[:, :, :])

        offp = ps_mm.tile([P, EMB], f32, name="offp", tag="mm")
        mm(offp, [(qts[:, 0, :], Woff[:, 0, :]), (qts[:, 1, :], Woff[:, 1, :])],
           bias=boff[:1, :])
        off = wpool.tile([P, EMB], f32, name="off", tag="off")
        nc.vector.tensor_copy(off[:, :], offp[:, :])

        attp = ps_sm.tile([P, NH * 16], f32, name="attp", tag="sm")
        mm(attp, [(qts[:, 0, :], Watt[:, 0, :]), (qts[:, 1, :], Watt[:, 1, :])],
           bias=batt[:1, :])
        att = wpool.tile([P, NH, 16], f32, name="att", tag="att")
        nc.vector.tensor_copy(att[:, :, :], attp[:, :].rearrange(
            "p (h l) -> p h l", h=NH))

        # softmax over (l,pt) per head
        rmax = wpool.tile([P, NH], f32, name="rmax", tag="rmax")
        nc.vector.reduce_max(rmax[:, :], att[:, :, :], axis=AX.X)
        exv = wpool.tile([P, NH, 16], f32, name="exv", tag="exv")
        rmaxa = rmax[:, :]
        nc.vector.tensor_sub(exv[:, :, :], att[:, :, :],
                             sap(rmaxa, 0, [rmaxa.ap[0], [1, NH], [0, 16]]))
        nc.scalar.activation(exv[:, :, :], exv[:, :, :], act_f.Exp)
        ssum = wpool.tile([P, NH], f32, name="ssum", tag="ssum")
        nc.vector.reduce_sum(ssum[:, :], exv[:, :, :], axis=AX.X)
        rsum = wpool.tile([P, NH], f32, name="rsum", tag="rsum")
        nc.vector.reciprocal(rsum[:, :], ssum[:, :])
        aw = wpool.tile([P, NH, 16], f32, name="aw", tag="aw")
        rsuma = rsum[:, :]
        nc.vector.tensor_mul(aw[:, :, :], exv[:, :, :],
                             sap(rsuma, 0, [rsuma.ap[0], [1, NH], [0, 16]]))

        def wt(name):
            return wpool.tile([P, EMB], f32, name=name, tag=name)

        loc = wt("loc")
        nc.vector.tensor_mul(loc[:, :], off[:, :], c_invn[:, :])
        refa = refr[:, blk, :]
        for xy in (0, 1):
            lvh = sap(loc[:, :], xy, [loc[:, :].ap[0], [32, NH], [8, NL],
                                      [2, NPT]])
            nc.vector.tensor_add(lvh, lvh,
                                 sap(refa, xy, [refa.ap[0], [0, NH], [2, NL],
                                                [0, NPT]]))
        pix = wt("pix")
        nc.vector.tensor_mul(pix[:, :], loc[:, :], c_pixs[:, :])
        nc.vector.tensor_scalar_add(pix[:, :], pix[:, :], -0.5)

        # floor(pix) robust to cast rounding mode
        xi = wpool.tile([P, EMB], i32, name="xi", tag="xi")
        nc.vector.tensor_copy(xi[:, :], pix[:, :])
        base = wt("base")
        nc.vector.tensor_copy(base[:, :], xi[:, :])
        fixm = wt("fixm")
        nc.vector.tensor_tensor(fixm[:, :], pix[:, :], base[:, :], op=op.is_lt)
        nc.vector.tensor_sub(base[:, :], base[:, :], fixm[:, :])
        wfrac = wt("wfrac")
        nc.vector.tensor_sub(wfrac[:, :], pix[:, :], base[:, :])

        basec = wt("basec")
        nc.vector.tensor_scalar_max(basec[:, :], base[:, :], 0.0)
        nc.vector.tensor_tensor(basec[:, :], basec[:, :], c_clip[:, :],
                                op=op.min)

        v0b = wt("v0b")
        nc.vector.tensor_tensor(v0b[:, :], base[:, :], c_vmax[:, :],
                                op=op.is_le)
        vld0 = wt("vld0")
        nc.vector.scalar_tensor_tensor(vld0[:, :], base[:, :], 0.0, v0b[:, :],
                                       op0=op.is_ge, op1=op.mult)
        v1b = wt("v1b")
        nc.vector.tensor_tensor(v1b[:, :], base[:, :], c_clip[:, :],
                                op=op.is_le)
        vld1 = wt("vld1")
        nc.vector.scalar_tensor_tensor(vld1[:, :], base[:, :], -1.0, v1b[:, :],
                                       op0=op.is_ge, op1=op.mult)

        tsh = wt("tsh")
        nc.vector.tensor_sub(tsh[:, :], base[:, :], basec[:, :])
        e0 = wt("e0")
        nc.vector.tensor_scalar(e0[:, :], tsh[:, :], 0.0, None,
                                op0=op.is_equal)
        em1 = wt("em1")
        nc.vector.tensor_scalar(em1[:, :], tsh[:, :], -1.0, None,
                                op0=op.is_equal)
        ep1 = wt("ep1")
        nc.vector.tensor_scalar(ep1[:, :], tsh[:, :], 1.0, None,
                                op0=op.is_equal)

        u0 = wt("u0")
        nc.vector.tensor_scalar(u0[:, :], wfrac[:, :], -1.0, 1.0, op0=op.mult,
                                op1=op.add)
        nc.vector.tensor_mul(u0[:, :], u0[:, :], vld0[:, :])
        u1 = wt("u1")
        nc.vector.tensor_mul(u1[:, :], wfrac[:, :], vld1[:, :])

        a0 = wt("a0")
        nc.vector.tensor_mul(a0[:, :], u0[:, :], e0[:, :])
        t1 = wt("t1")
        nc.vector.tensor_mul(t1[:, :], u1[:, :], em1[:, :])
        nc.vector.tensor_add(a0[:, :], a0[:, :], t1[:, :])
        a1 = wt("a1")
        nc.vector.tensor_mul(a1[:, :], u0[:, :], ep1[:, :])
        nc.vector.tensor_mul(t1[:, :], u1[:, :], e0[:, :])
        nc.vector.tensor_add(a1[:, :], a1[:, :], t1[:, :])

        def ycols(t):
            return sap(t[:, :], 1, [[t[:, :].ap[0][0], P], [2, P]])

        def xcols(t):
            return sap(t[:, :], 0, [[t[:, :].ap[0][0], P], [2, P]])

        awf = aw.rearrange("p h l -> p (h l)")
        ay0 = wpool.tile([P, P], f32, name="ay0", tag="ay0")
        nc.vector.tensor_mul(ay0[:, :], ycols(a0), awf)
        ay1 = wpool.tile([P, P], f32, name="ay1", tag="ay1")
        nc.vector.tensor_mul(ay1[:, :], ycols(a1), awf)

        w4 = wpool.tile([P, P, 4], f16, name="w4", tag="w4", bufs=3)
        nc.vector.tensor_mul(w4[:, :, 0], ay0[:, :], xcols(a0))
        nc.vector.tensor_mul(w4[:, :, 1], ay0[:, :], xcols(a1))
        nc.vector.tensor_mul(w4[:, :, 2], ay1[:, :], xcols(a0))
        nc.vector.tensor_mul(w4[:, :, 3], ay1[:, :], xcols(a1))

        cell = wpool.tile([P, P], f32, name="cell", tag="cell")
        nc.vector.tensor_mul(cell[:, :], ycols(basec), c_W[:, :])
        nc.vector.tensor_add(cell[:, :], cell[:, :], xcols(basec))
        nc.vector.tensor_add(cell[:, :], cell[:, :], c_S[:, :])

        nc.vector.tensor_add(cell[:, :], cell[:, :], c_HL[:, :])
        offs = wpool.tile([P, P], i32, name="offs", tag="offs", bufs=3)
        nc.vector.tensor_copy(offs[:, :], cell[:, :])
        return fq, w4, offs

    # ==================== LayerNorm ====================
    def emit_ln(r, gt, bt, pfx):
        nsum = opool.tile([P, 1], f32, name=f"{pfx}ns", tag=f"{pfx}ns")
        nc.vector.tensor_reduce(nsum[:, :], r[:, :], axis=AX.X, op=op.add,
                                negate=True)
        nmean = opool.tile([P, 1], f32, name=f"{pfx}nm", tag=f"{pfx}nm")
        nc.scalar.mul(nmean[:, :], nsum[:, :], 1.0 / EMB)
        c = opool.tile([P, EMB], f32, name=f"{pfx}c", tag=f"{pfx}c")
        nc.vector.tensor_scalar_add(c[:, :], r[:, :], nmean[:, :])
        csq = opool.tile([P, EMB], f32, name=f"{pfx}sq", tag=f"{pfx}sq")
        ssq = opool.tile([P, 1], f32, name=f"{pfx}ssq", tag=f"{pfx}ssq")
        nc.scalar.activation(csq[:, :], c[:, :], act_f.Square,
                             accum_out=ssq[:, :])
        std = opool.tile([P, 1], f32, name=f"{pfx}std", tag=f"{pfx}std")
        nc.scalar.activation(std[:, :], ssq[:, :], act_f.Sqrt,
                             bias=eps_t[:, :], scale=1.0 / EMB)
        rstd = opool.tile([P, 1], f32, name=f"{pfx}rs", tag=f"{pfx}rs")
        nc.vector.reciprocal(rstd[:, :], std[:, :])
        x = opool.tile([P, EMB], f32, name=f"{pfx}x", tag=f"{pfx}x")
        nc.vector.scalar_tensor_tensor(x[:, :], c[:, :], rstd[:, :], gt[:, :],
                                       op0=op.mult, op1=op.mult)
        nc.vector.tensor_add(x[:, :], x[:, :], bt[:, :])
        return x

    # ==================== gathers for one block ====================
    def emit_gathers(offs):
        gb = gpool.tile([P, P, 4 * HD], f16, name="gb", tag="gb", bufs=2)
        for l in LVL_ORDER:
            tt = tabl[l]
            for h in range(NH):
                for pt in range(NPT):
                    s = h * 16 + l * NPT + pt
                    nc.gpsimd.indirect_dma_start(
                        out=gb[:, s, :], out_offset=None,
                        in_=tt.ap()[:, :],
                        in_offset=bass.IndirectOffsetOnAxis(
                            ap=offs[:, s:s + 1], axis=0))
        return gb

    # ==================== combine + backend for one block ================
    def emit_backend(blk, fq, w4, gb):
        acat = kpool.tile([P, EMB], f32, name="acat", tag="acat")
        gba = gb[:, :, :]
        pstr = gba.ap[0][0]

        def gsl(off, dims):
            return sap(gba, off, [[pstr, P]] + dims)

        # weights: w4 [P, (h,lp), 4] broadcast over head_dim (0-stride)
        w4b = sap(w4[:, :, :], 0,
                  [[w4[:, :, :].ap[0][0], P], [4, P], [1, 4], [0, HD]])
        gall = gsl(0, [[128, P], [HD, 4], [1, HD]])
        nc.vector.tensor_mul(gall, gall, w4b)
        # corner folds: c0+=c1, c2+=c3, c0+=c2
        d2 = [[128, P], [1, HD]]
        nc.vector.tensor_add(gsl(0, d2), gsl(0, d2), gsl(HD, d2))
        nc.vector.tensor_add(gsl(2 * HD, d2), gsl(2 * HD, d2), gsl(3 * HD, d2))
        nc.vector.tensor_add(gsl(0, d2), gsl(0, d2), gsl(2 * HD, d2))
        # lp folds: 16 -> 8 -> 4 -> 2 (per head; h stride 16*128)
        for w in (8, 4, 2):
            dh = [[16 * 128, NH], [128, w], [1, HD]]
            nc.vector.tensor_add(gsl(0, dh), gsl(0, dh), gsl(w * 128, dh))
        # final fold writes the fp32 attention output slice layout
        acv = sap(acat[:, :], 0, [[acat[:, :].ap[0][0], P], [HD, NH], [1, HD]])
        dh1 = [[16 * 128, NH], [1, HD]]
        nc.vector.tensor_add(acv, gsl(0, dh1), gsl(128, dh1))

        # ---- output projection + LN + FFN + LN ----
        atp = ps_tr.tile([P, 2, P], f32, name="atp", tag="tr")
        nc.tensor.transpose(atp[:, 0, :], acat[:, 0:P], ident[:, :])
        nc.tensor.transpose(atp[:, 1, :], acat[:, P:EMB], ident[:, :])
        ats = opool.tile([P, 2, P], f32, name="ats", tag="ats")
        nc.vector.tensor_copy(ats[:, :, :], atp[:, :, :])
        oprj = ps_mm.tile([P, EMB], f32, name="oprj", tag="mm")
        mm(oprj, [(ats[:, 0, :], Wout[:, 0, :]),
                  (ats[:, 1, :], Wout[:, 1, :])], bias=bout[:1, :])

        r1 = opool.tile([P, EMB], f32, name="r1", tag="r1")
        nc.vector.tensor_add(r1[:, :], oprj[:, :], fq[:, :])
        x1 = emit_ln(r1, ln1g, ln1b, "la")

        xtp = ps_tr.tile([P, 2, P], f32, name="xtp", tag="tr")
        nc.tensor.transpose(xtp[:, 0, :], x1[:, 0:P], ident[:, :])
        nc.tensor.transpose(xtp[:, 1, :], x1[:, P:EMB], ident[:, :])
        xts = opool.tile([P, 2, P], f32, name="xts", tag="xts")
        nc.vector.tensor_copy(xts[:, :, :], xtp[:, :, :])

        h1s = opool.tile([P, DFFN // P, P], f32, name="h1s", tag="h1s")
        hp = ps_mm.tile([P, DFFN // P, P], f32, name="hp", tag="hpw", bufs=1)
        for mt in range(DFFN // P):
            nc.tensor.matmul(hp[:, mt, :], W1[:, 0, mt * P:(mt + 1) * P],
                             xts[:, 0, :], start=True, stop=False)
            nc.tensor.matmul(hp[:, mt, :], W1[:, 1, mt * P:(mt + 1) * P],
                             xts[:, 1, :], start=False, stop=False)
            nc.tensor.matmul(hp[:, mt, :], b1r[:1, mt * P:(mt + 1) * P],
                             onesr[:1, :], start=False, stop=True)
        nc.scalar.activation(h1s[:, :, :], hp[:, :, :], act_f.Relu)

        yp = ps_mm.tile([P, EMB], f32, name="yp", tag="mm")
        for mt in range(DFFN // P):
            nc.tensor.matmul(yp[:, :], h1s[:, mt, :], W2[:, mt, :],
                             start=(mt == 0), stop=False)
        nc.tensor.matmul(yp[:, :], onesr[:1, :], b2r[:1, :],
                         start=False, stop=True)

        r2 = opool.tile([P, EMB], f32, name="r2", tag="r2")
        nc.vector.tensor_add(r2[:, :], yp[:, :], x1[:, :])
        x2 = emit_ln(r2, ln2g, ln2b, "lb")
        dma(outs["out_q"][blk * P:(blk + 1) * P, :], x2)

    # ==================== top-level schedule ====================
    # frontends for the first two blocks run during the value projection;
    # thereafter frontend(g+1) is emitted ahead of combine/backend(g) so
    # the gpsimd gather stream never waits on the previous block's tail.
    F = {0: emit_frontend(0), 1: emit_frontend(1)}
    emit_value()
    for g in range(NBQ):
        if g + 2 < NBQ:
            F[g + 2] = emit_frontend(g + 2)
        fq, w4, offs = F.pop(g)
        gb = emit_gathers(offs)
        emit_backend(g, fq, w4, gb)

    ctx.close()


# ------------------------------------------------------------ host entry ---

_CACHE = {}


def build_nc(cfg):
    import concourse.bass as bass
    from concourse import bacc, mybir, tile

    nc = bacc.Bacc("TRN2", debug=False)
    f32 = mybir.dt.float32

    f16 = mybir.dt.float16

    def di(name, shape, dt=None):
        return nc.dram_tensor(name, list(shape), dt or f32,
                              kind="ExternalInput").ap()

    HQ, LPAD = cfg["HQ"], cfg["LPAD"]
    ins = dict(
        feat_val=di("feat_val", [LPAD, EMB], f16),
        feat_q=di("feat_q", [HQ, EMB]),
        pos_q=di("pos_q", [HQ, EMB], f16),
        ref_q=di("ref_q", [HQ, NL, 2]),
        W_val=di("W_val", [EMB, EMB]), b_val=di("b_val", [1, EMB]),
        W_off=di("W_off", [EMB, EMB]), b_off=di("b_off", [1, EMB]),
        W_attn=di("W_attn", [EMB, NH * NL * NPT]),
        b_attn=di("b_attn", [1, NH * NL * NPT]),
        W_out=di("W_out", [EMB, EMB]), b_out=di("b_out", [1, EMB]),
        W1=di("W1", [EMB, DFFN]), b1=di("b1", [1, DFFN]),
        W2=di("W2", [DFFN, EMB]), b2=di("b2", [1, EMB]),
        ln1_g=di("ln1_g", [1, EMB]), ln1_b=di("ln1_b", [1, EMB]),
        ln2_g=di("ln2_g", [1, EMB]), ln2_b=di("ln2_b", [1, EMB]),
        cst_xy=di("cst_xy", [4, EMB]),
        cst_hlp=di("cst_hlp", [3, P]),
        ident=di("ident", [P, P]),
        ones_row=di("ones_row", [1, P]),
    )
    outs = dict(
        out_q=nc.dram_tensor("out_q", [HQ, EMB], f32,
                             kind="ExternalOutput").ap(),
    )
    with tile.TileContext(nc) as tc:
        emit_kernel(tc, outs, ins, cfg)
    nc.compile()
    return nc


def make_in_maps(inputs, cfg):
    feats = np.asarray(inputs["features"], np.float32)
    pos = np.asarray(inputs["pos"], np.float32)
    refp = np.asarray(inputs["reference_points"], np.float32)
    B = feats.shape[0]
    HQ, LPAD, L = cfg["HQ"], cfg["LPAD"], cfg["L"]
    hw, vstart = cfg["hw"], cfg["vstart"]
    starts = np.cumsum([0] + hw)[:-1]
    half = L // 2

    consts = host_constants(cfg)
    wkeys = dict(
        W_val=inputs["W_val"], b_val=np.reshape(inputs["b_val"], (1, -1)),
        W_off=inputs["W_off"], b_off=np.reshape(inputs["b_off"], (1, -1)),
        W_attn=inputs["W_attn"], b_attn=np.reshape(inputs["b_attn"], (1, -1)),
        W_out=inputs["W_out"], b_out=np.reshape(inputs["b_out"], (1, -1)),
        W1=inputs["W1"], b1=np.reshape(inputs["b1"], (1, -1)),
        W2=inputs["W2"], b2=np.reshape(inputs["b2"], (1, -1)),
        ln1_g=np.reshape(inputs["ln1_g"], (1, -1)),
        ln1_b=np.reshape(inputs["ln1_b"], (1, -1)),
        ln2_g=np.reshape(inputs["ln2_g"], (1, -1)),
        ln2_b=np.reshape(inputs["ln2_b"], (1, -1)),
    )
    wkeys = {k: np.ascontiguousarray(np.asarray(v, np.float32))
             for k, v in wkeys.items()}

    halves = [(0, half), (half, L)]
    in_maps = []
    for core in range(2 * B):
        b, hf = core // 2, core % 2
        s, e = halves[hf]
        fv = np.zeros((LPAD, EMB), np.float16)
        f16b = feats[b].astype(np.float16)
        for l in range(NL):
            fv[vstart[l]:vstart[l] + hw[l]] = \
                f16b[starts[l]:starts[l] + hw[l]]
        fq = np.zeros((HQ, EMB), np.float32)
        fq[:e - s] = feats[b, s:e]
        pq = np.zeros((HQ, EMB), np.float16)
        pq[:e - s] = pos[b, s:e].astype(np.float16)
        rq = np.zeros((HQ, NL, 2), np.float32)
        rq[:e - s] = refp[b, s:e]
        m = dict(feat_val=fv, feat_q=fq, pos_q=pq, ref_q=rq)
        m.update(wkeys)
        m.update({k: np.ascontiguousarray(np.asarray(v, np.float32))
                  for k, v in consts.items()})
        in_maps.append(m)
    return in_maps, halves


def kernel(**inputs):
    from concourse import bass_utils

    cfg = CFG_FULL
    in_maps, halves = make_in_maps(inputs, cfg)
    B = np.asarray(inputs["features"]).shape[0]
    L = cfg["L"]

    if "nc" not in _CACHE:
        _CACHE["nc"] = build_nc(cfg)
    nc = _CACHE["nc"]

    res = bass_utils.run_bass_kernel_spmd(nc, in_maps,
                                          core_ids=list(range(2 * B)))
    out = np.zeros((B, L, EMB), np.float32)
    for core in range(2 * B):
        b, hf = core // 2, core % 2
        s, e = halves[hf]
        out[b, s:e] = res.results[core]["out_q"][:e - s]
    return out


# revision 16
# speedup vs baseline: 847.7236x; 1.0133x over previous
"""Trainium2 Bass kernel for a Deformable-DETR style encoder block.

Sharding: 8 NeuronCores = 4 batch samples x 2 query-halves.

Per core:
  - value projection over the full sample -> fp16 "patch table" in DRAM:
    for cell (y,x) and head h the 2x2 neighborhood [V[y,x], V[y,x+1],
    V[y+1,x], V[y+1,x+1]] is packed contiguously (4*32 fp16 = 256B).
    Levels 0-2 go into one head-major table (rows h*13125 + lvloff + cell)
    so a single gpsimd.dma_gather per (block, head) fetches all 12
    (level, point) patches for 128 queries (1536 records/call, int16
    indices wrapped 16-way and replicated for the 8 Q7 cores).
  - level 3 (13x13) skips the gather entirely: its patch table lives in
    SBUF and a one-hot [cells x queries] matrix from the PE selects
    patches into PSUM (one matmul pair per head/point).
  - offset/attention projections, softmax, bilinear weights and cell
    indices computed query-major; cell indices are PE-transposed into the
    wrapped int16 index layout dma_gather wants.
  - DVE multiplies by bilinear*attention weights and tree-reduces.
  - output projection + LayerNorm + FFN + LayerNorm, then DMA out.
  - frontends are emitted ahead of the previous block's combine/backend
    so the gpsimd gather stream never stalls.
"""

import numpy as np
from contextlib import ExitStack

EMB = 256
NH = 8
NL = 4
NPT = 4
HD = 32
DFFN = 1024
P = 128

# value/table build order: small levels first so tables are ready early
LVL_ORDER = [3, 2, 1, 0]
# levels 0-2 combined per-head table: row = h*CROWS + LVLOFF[l] + cell
LVLOFF = {0: 0, 1: 10000, 2: 12500}
CROWS = 13125  # 10000 + 2500 + 625
L3 = 169       # 13*13 cells in level 3


def make_cfg(shapes, n_blk_q):
    L = sum(h * w for h, w in shapes)
    hw = [h * w for h, w in shapes]
    pblk = {l: -(-hw[l] // P) for l in range(NL)}
    vstart = {}
    off = 0
    for l in LVL_ORDER:
        vstart[l] = off
        off += pblk[l] * P
    return dict(
        shapes=[tuple(s) for s in shapes], hw=hw, L=L,
        pblk=pblk, vstart=vstart, LPAD=off, NBF=off // P,
        NBQ=n_blk_q, HQ=n_blk_q * P,
    )


CFG_FULL = make_cfg([(100, 100), (50, 50), (25, 25), (13, 13)], 52)


# ------------------------------------------------------- host-side consts ---

def host_constants(cfg):
    shapes = cfg["shapes"]
    invnorm = np.zeros(EMB, np.float32)
    pixscale = np.zeros(EMB, np.float32)
    clipmax = np.zeros(EMB, np.float32)
    vmax = np.zeros(EMB, np.float32)
    for h in range(NH):
        for l, (H_, W_) in enumerate(shapes):
            for pt in range(NPT):
                base = h * (NL * NPT * 2) + l * (NPT * 2) + pt * 2
                invnorm[base + 0] = 1.0 / W_
                invnorm[base + 1] = 1.0 / H_
                pixscale[base + 0] = W_
                pixscale[base + 1] = H_
                clipmax[base + 0] = W_ - 2
                clipmax[base + 1] = H_ - 2
                vmax[base + 0] = W_ - 1
                vmax[base + 1] = H_ - 1
    cst_xy = np.stack([invnorm, pixscale, clipmax, vmax])

    wrow = np.zeros(P, np.float32)
    srow = np.zeros(P, np.float32)
    for h in range(NH):
        for l, (H_, W_) in enumerate(shapes):
            for pt in range(NPT):
                base = h * (NL * NPT) + l * NPT + pt
                wrow[base] = W_
                srow[base] = float(LVLOFF.get(l, 0))
    cst_hlp = np.stack([wrow, srow, np.zeros(P, np.float32)])

    ident = np.eye(P, dtype=np.float32)
    ones_row = np.ones((1, P), np.float32)
    iota = np.arange(L3, dtype=np.float32).reshape(L3, 1)
    return dict(cst_xy=cst_xy, cst_hlp=cst_hlp, ident=ident,
                ones_row=ones_row, iota=iota)


# ------------------------------------------------------------- emission ---

def emit_kernel(tc, outs, ins, cfg):
    import concourse.bass as bass
    from concourse import mybir

    nc = tc.nc
    op = mybir.AluOpType
    act_f = mybir.ActivationFunctionType
    f32, f16 = mybir.dt.float32, mybir.dt.float16
    i16 = mybir.dt.int16
    AX = mybir.AxisListType

    shapes, hw = cfg["shapes"], cfg["hw"]
    NBQ = cfg["NBQ"]
    pblk = cfg["pblk"]

    ctx = ExitStack()

    def dap(handle, offset, dims):
        return bass.AP(tensor=handle, offset=offset,
                       ap=[list(d) for d in dims])

    def sap(ap0, extra_off, dims):
        return bass.AP(tensor=ap0.tensor, offset=ap0.offset + extra_off,
                       ap=[list(d) for d in dims])

    # ---- internal DRAM: per-level value + patch tables ----
    valf = {l: nc.dram_tensor(f"valf{l}", [pblk[l] * P, EMB], f16,
                              kind="Internal") for l in range(NL)}
    tabC = nc.dram_tensor("tabC", [NH * CROWS, 4 * HD], f16, kind="Internal")
    tab3 = nc.dram_tensor("tab3", [NH * L3, 4 * HD], f16, kind="Internal")
    # partition-replication bounce buffers (DMA broadcast goes via DRAM)
    NW = NH * 12 * 8
    cwd = [nc.dram_tensor(f"cwd{i}", [16 * NW], i16, kind="Internal")
           for i in range(2)]
    c3d = [nc.dram_tensor(f"c3d{i}", [1, NH * NPT * P], f16, kind="Internal")
           for i in range(2)]

    # ---- pools ----
    cpool = ctx.enter_context(tc.tile_pool(name="consts", bufs=1))
    apool = ctx.enter_context(tc.tile_pool(name="acts", bufs=3))
    wpool = ctx.enter_context(tc.tile_pool(name="wmath", bufs=1))
    gpool = ctx.enter_context(tc.tile_pool(name="gath", bufs=2))
    g3pool = ctx.enter_context(tc.tile_pool(name="gath3", bufs=2))
    bcpool = ctx.enter_context(tc.tile_pool(name="bcast", bufs=1))
    kpool = ctx.enter_context(tc.tile_pool(name="comb", bufs=2))
    opool = ctx.enter_context(tc.tile_pool(name="outp", bufs=2))
    ps_tr = ctx.enter_context(tc.tile_pool(name="ps_tr", bufs=2, space="PSUM"))
    ps_mm = ctx.enter_context(tc.tile_pool(name="ps_mm", bufs=2, space="PSUM"))
    ps_sm = ctx.enter_context(tc.tile_pool(name="ps_sm", bufs=1, space="PSUM"))

    def dma(out_ap, in_ap):
        nc.sync.dma_start(out=out_ap, in_=in_ap)

    # ---- constants / weights ----
    def load_w(name, k, n):
        tmp = apool.tile([P, 2 * DFFN], f32, name="wtmp", tag="wtmp",
                         bufs=1)
        tv = sap(tmp[:, :], 0, [tmp[:, :].ap[0], [n, k // P], [1, n]])
        dma(tv, ins[name].rearrange("(a p) n -> p a n", p=P))
        t = cpool.tile([P, k // P, n], f16, name=f"s_{name}")
        nc.vector.tensor_copy(t[:, :, :], tv)
        return t

    Wval = load_w("W_val", EMB, EMB)
    Woff = load_w("W_off", EMB, EMB)
    Watt = load_w("W_attn", EMB, NH * NL * NPT)
    Wout = load_w("W_out", EMB, EMB)
    W1 = load_w("W1", EMB, DFFN)
    W2 = load_w("W2", DFFN, EMB)

    def load_row(name, n):
        t = cpool.tile([1, n], f32, name=f"r_{name}")
        dma(t, ins[name][:, :])
        return t

    bval = load_row("b_val", EMB)
    boff = load_row("b_off", EMB)
    batt = load_row("b_attn", NH * NL * NPT)
    bout = load_row("b_out", EMB)
    b1r = load_row("b1", DFFN)
    b2r = load_row("b2", EMB)
    onesr = load_row("ones_row", P)

    def load_bc(src_ap, n, name):
        t = cpool.tile([P, n], f32, name=f"b_{name}")
        dma(t, src_ap.to_broadcast([P, n]))
        return t

    ln1g = load_bc(ins["ln1_g"][:, :], EMB, "ln1g")
    ln1b = load_bc(ins["ln1_b"][:, :], EMB, "ln1b")
    ln2g = load_bc(ins["ln2_g"][:, :], EMB, "ln2g")
    ln2b = load_bc(ins["ln2_b"][:, :], EMB, "ln2b")
    c_invn = load_bc(ins["cst_xy"][0:1, :], EMB, "invn")
    c_pixs = load_bc(ins["cst_xy"][1:2, :], EMB, "pixs")
    c_clip = load_bc(ins["cst_xy"][2:3, :], EMB, "clip")
    c_vmax = load_bc(ins["cst_xy"][3:4, :], EMB, "vmax")
    c_W = load_bc(ins["cst_hlp"][0:1, :], P, "cw")
    c_S = load_bc(ins["cst_hlp"][1:2, :], P, "cs")

    ident = cpool.tile([P, P], f32, name="ident")
    dma(ident, ins["ident"][:, :])
    eps_t = cpool.tile([P, 1], f32, name="eps_t")
    nc.vector.memset(eps_t[:, :], 1e-5)

    iota_a = cpool.tile([P, 1], f32, name="iota_a")
    dma(iota_a, ins["iota"][0:P, :])
    iota_b = cpool.tile([L3 - P, 1], f32, name="iota_b")
    dma(iota_b, ins["iota"][P:L3, :])

    refr = cpool.tile([P, NBQ, 2 * NL], f32, name="refr")
    dma(refr, ins["ref_q"].rearrange("(b p) l c -> p b (l c)", p=P))

    # level-3 patch table resident in SBUF: [cell, head, 128]
    t3a = cpool.tile([P, NH, 4 * HD], f16, name="t3a")
    t3b = cpool.tile([L3 - P, NH, 4 * HD], f16, name="t3b")

    def mm(psum_ap, pairs, bias=None):
        seq = list(pairs)
        if bias is not None:
            seq.append((onesr[:1, :psum_ap.shape[0]], bias))
        for i, (lt, rt) in enumerate(seq):
            nc.tensor.matmul(psum_ap, lt, rt,
                             start=(i == 0), stop=(i == len(seq) - 1))

    # ======================= patch-table build (per level) ==============
    def table_dst(l, h):
        if l == 3:
            return tab3, (h * L3) * 4 * HD
        return tabC, (h * CROWS + LVLOFF[l]) * 4 * HD

    def emit_table(l):
        H_, W_ = shapes[l]
        vt = valf[l]
        for h in range(NH):
            tt, tbase = table_dst(l, h)
            for cy in (0, 1):
                for cx in (0, 1):
                    c = cy * 2 + cx
                    src = dap(vt, (cy * W_ + cx) * EMB + h * HD,
                              [[W_ * EMB, H_ - 1], [EMB, W_ - 1], [1, HD]])
                    dst = dap(tt, tbase + c * HD,
                              [[W_ * 4 * HD, H_ - 1], [4 * HD, W_ - 1],
                               [1, HD]])
                    dma(dst, src)
            # fill never-gathered edge records (x=W-1 col, y=H-1 row) so the
            # table contains no uninitialized (possibly non-finite) bytes
            dma(dap(tt, tbase + (W_ - 1) * 4 * HD,
                    [[W_ * 4 * HD, H_], [HD, 4], [1, HD]]),
                dap(vt, (W_ - 1) * EMB + h * HD,
                    [[W_ * EMB, H_], [0, 4], [1, HD]]))
            dma(dap(tt, tbase + ((H_ - 1) * W_) * 4 * HD,
                    [[4 * HD, W_ - 1], [HD, 4], [1, HD]]),
                dap(vt, ((H_ - 1) * W_) * EMB + h * HD,
                    [[EMB, W_ - 1], [0, 4], [1, HD]]))
        if l == 3:
            # SBUF copy for the PE-side gather: [cell, head, 128]
            dma(t3a, tab3.ap().rearrange("(h c) d -> c h d", h=NH)[0:P])
            dma(t3b, tab3.ap().rearrange("(h c) d -> c h d", h=NH)[P:L3])

    # ======================= value projection ===========================
    def emit_value():
        idf16 = cpool.tile([P, P], f16, name="idf16")
        nc.vector.tensor_copy(idf16[:, :], ident[:, :])
        sched = []
        for l in LVL_ORDER:
            for lb in range(pblk[l]):
                sched.append((l, lb, lb == pblk[l] - 1))
        for blk, (l, lb, last) in enumerate(sched):
            fv = apool.tile([P, EMB], f16, name="fv", tag="fv")
            dma(fv, ins["feat_val"][blk * P:(blk + 1) * P, :])
            ftp = ps_tr.tile([P, 2, P], f16, name="ftp", tag="tr")
            nc.tensor.transpose(ftp[:, 0, :], fv[:, 0:P], idf16[:, :])
            nc.tensor.transpose(ftp[:, 1, :], fv[:, P:EMB], idf16[:, :])
            fts = apool.tile([P, 2, P], f16, name="fts", tag="fts")
            nc.vector.tensor_copy(fts[:, :, :], ftp[:, :, :])
            vp = ps_mm.tile([P, EMB], f32, name="vp", tag="mm")
            mm(vp, [(fts[:, 0, :], Wval[:, 0, :]),
                    (fts[:, 1, :], Wval[:, 1, :])], bias=bval[:1, :])
            vf = apool.tile([P, EMB], f16, name="vf", tag="vf")
            nc.vector.tensor_copy(vf[:, :], vp[:, :])
            dma(valf[l].ap()[lb * P:(lb + 1) * P, :], vf)
            if last:
                emit_table(l)

    # ==================== per-block frontend ====================
    def emit_frontend(blk):
        fq = apool.tile([P, EMB], f32, name="fq", tag="fq", bufs=4)
        dma(fq, ins["feat_q"][blk * P:(blk + 1) * P, :])
        pq = apool.tile([P, EMB], f16, name="pq", tag="pq")
        dma(pq, ins["pos_q"][blk * P:(blk + 1) * P, :])
        qb = apool.tile([P, EMB], f32, name="qb", tag="qb")
        nc.vector.tensor_add(qb[:, :], fq[:, :], pq[:, :])

        qtp = ps_tr.tile([P, 2, P], f32, name="qtp", tag="tr")
        nc.tensor.transpose(qtp[:, 0, :], qb[:, 0:P], ident[:, :])
        nc.tensor.transpose(qtp[:, 1, :], qb[:, P:EMB], ident[:, :])
        qts = apool.tile([P, 2, P], f16, name="qts", tag="qts", bufs=2)
        nc.vector.tensor_copy(qts[:, :, :], qtp[:, :, :])

        offp = ps_mm.tile([P, EMB], f32, name="offp", tag="mm")
        mm(offp, [(qts[:, 0, :], Woff[:, 0, :]), (qts[:, 1, :], Woff[:, 1, :])],
           bias=boff[:1, :])
        off = wpool.tile([P, EMB], f32, name="off", tag="off")
        nc.vector.tensor_copy(off[:, :], offp[:, :])

        attp = ps_sm.tile([P, NH * 16], f32, name="attp", tag="sm")
        mm(attp, [(qts[:, 0, :], Watt[:, 0, :]), (qts[:, 1, :], Watt[:, 1, :])],
           bias=batt[:1, :])
        att = wpool.tile([P, NH, 16], f32, name="att", tag="att")
        nc.vector.tensor_copy(att[:, :, :], attp[:, :].rearrange(
            "p (h l) -> p h l", h=NH))

        # softmax over (l,pt) per head
        rmax = wpool.tile([P, NH], f32, name="rmax", tag="rmax")
        nc.vector.reduce_max(rmax[:, :], att[:, :, :], axis=AX.X)
        exv = wpool.tile([P, NH, 16], f32, name="exv", tag="exv")
        rmaxa = rmax[:, :]
        nc.vector.tensor_sub(exv[:, :, :], att[:, :, :],
                             sap(rmaxa, 0, [rmaxa.ap[0], [1, NH], [0, 16]]))
        nc.scalar.activation(exv[:, :, :], exv[:, :, :], act_f.Exp)
        ssum = wpool.tile([P, NH], f32, name="ssum", tag="ssum")
        nc.vector.reduce_sum(ssum[:, :], exv[:, :, :], axis=AX.X)
        rsum = wpool.tile([P, NH], f32, name="rsum", tag="rsum")
        nc.vector.reciprocal(rsum[:, :], ssum[:, :])
        aw = wpool.tile([P, NH, 16], f32, name="aw", tag="aw")
        rsuma = rsum[:, :]
        nc.vector.tensor_mul(aw[:, :, :], exv[:, :, :],
                             sap(rsuma, 0, [rsuma.ap[0], [1, NH], [0, 16]]))

        def wt(name):
            return wpool.tile([P, EMB], f32, name=name, tag=name)

        loc = wt("loc")
        nc.vector.tensor_mul(loc[:, :], off[:, :], c_invn[:, :])
        refa = refr[:, blk, :]
        for xy in (0, 1):
            lvh = sap(loc[:, :], xy, [loc[:, :].ap[0], [32, NH], [8, NL],
                                      [2, NPT]])
            nc.vector.tensor_add(lvh, lvh,
                                 sap(refa, xy, [refa.ap[0], [0, NH], [2, NL],
                                                [0, NPT]]))
        pix = wt("pix")
        nc.vector.tensor_mul(pix[:, :], loc[:, :], c_pixs[:, :])
        nc.vector.tensor_scalar_add(pix[:, :], pix[:, :], -0.5)

        # floor(pix) robust to cast rounding mode
        xi = wpool.tile([P, EMB], mybir.dt.int32, name="xi", tag="xi")
        nc.vector.tensor_copy(xi[:, :], pix[:, :])
        base = wt("base")
        nc.vector.tensor_copy(base[:, :], xi[:, :])
        fixm = wt("fixm")
        nc.vector.tensor_tensor(fixm[:, :], pix[:, :], base[:, :], op=op.is_lt)
        nc.vector.tensor_sub(base[:, :], base[:, :], fixm[:, :])
        wfrac = wt("wfrac")
        nc.vector.tensor_sub(wfrac[:, :], pix[:, :], base[:, :])

        basec = wt("basec")
        nc.vector.tensor_scalar_max(basec[:, :], base[:, :], 0.0)
        nc.vector.tensor_tensor(basec[:, :], basec[:, :], c_clip[:, :],
                                op=op.min)

        v0b = wt("v0b")
        nc.vector.tensor_tensor(v0b[:, :], base[:, :], c_vmax[:, :],
                                op=op.is_le)
        vld0 = wt("vld0")
        nc.vector.scalar_tensor_tensor(vld0[:, :], base[:, :], 0.0, v0b[:, :],
                                       op0=op.is_ge, op1=op.mult)
        v1b = wt("v1b")
        nc.vector.tensor_tensor(v1b[:, :], base[:, :], c_clip[:, :],
                                op=op.is_le)
        vld1 = wt("vld1")
        nc.vector.scalar_tensor_tensor(vld1[:, :], base[:, :], -1.0, v1b[:, :],
                                       op0=op.is_ge, op1=op.mult)

        tsh = wt("tsh")
        nc.vector.tensor_sub(tsh[:, :], base[:, :], basec[:, :])
        e0 = wt("e0")
        nc.vector.tensor_scalar(e0[:, :], tsh[:, :], 0.0, None,
                                op0=op.is_equal)
        em1 = wt("em1")
        nc.vector.tensor_scalar(em1[:, :], tsh[:, :], -1.0, None,
                                op0=op.is_equal)
        ep1 = wt("ep1")
        nc.vector.tensor_scalar(ep1[:, :], tsh[:, :], 1.0, None,
                                op0=op.is_equal)

        u0 = wt("u0")
        nc.vector.tensor_scalar(u0[:, :], wfrac[:, :], -1.0, 1.0, op0=op.mult,
                                op1=op.add)
        nc.vector.tensor_mul(u0[:, :], u0[:, :], vld0[:, :])
        u1 = wt("u1")
        nc.vector.tensor_mul(u1[:, :], wfrac[:, :], vld1[:, :])

        a0 = wt("a0")
        nc.vector.tensor_mul(a0[:, :], u0[:, :], e0[:, :])
        t1 = wt("t1")
        nc.vector.tensor_mul(t1[:, :], u1[:, :], em1[:, :])
        nc.vector.tensor_add(a0[:, :], a0[:, :], t1[:, :])
        a1 = wt("a1")
        nc.vector.tensor_mul(a1[:, :], u0[:, :], ep1[:, :])
        nc.vector.tensor_mul(t1[:, :], u1[:, :], e0[:, :])
        nc.vector.tensor_add(a1[:, :], a1[:, :], t1[:, :])

        def ycols(t):
            return sap(t[:, :], 1, [[t[:, :].ap[0][0], P], [2, P]])

        def xcols(t):
            return sap(t[:, :], 0, [[t[:, :].ap[0][0], P], [2, P]])

        awf = aw.rearrange("p h l -> p (h l)")
        ay0 = wpool.tile([P, P], f32, name="ay0", tag="ay0")
        nc.vector.tensor_mul(ay0[:, :], ycols(a0), awf)
        ay1 = wpool.tile([P, P], f32, name="ay1", tag="ay1")
        nc.vector.tensor_mul(ay1[:, :], ycols(a1), awf)

        w4 = wpool.tile([P, P, 4], f16, name="w4", tag="w4", bufs=3)
        nc.vector.tensor_mul(w4[:, :, 0], ay0[:, :], xcols(a0))
        nc.vector.tensor_mul(w4[:, :, 1], ay0[:, :], xcols(a1))
        nc.vector.tensor_mul(w4[:, :, 2], ay1[:, :], xcols(a0))
        nc.vector.tensor_mul(w4[:, :, 3], ay1[:, :], xcols(a1))

        # cell index within level (+ level offset for the combined table)
        cell = wpool.tile([P, P], f32, name="cell", tag="cell")
        nc.vector.tensor_mul(cell[:, :], ycols(basec), c_W[:, :])
        nc.vector.tensor_add(cell[:, :], cell[:, :], xcols(basec))
        nc.vector.tensor_add(cell[:, :], cell[:, :], c_S[:, :])

        # transpose -> cellT [slot, q]
        ctp = ps_tr.tile([P, P], f32, name="ctp", tag="tr")
        nc.tensor.transpose(ctp[:, :], cell[:, :], ident[:, :])
        cellT = wpool.tile([P, P], f32, name="cellT", tag="cellT", bufs=2)
        nc.vector.tensor_copy(cellT[:, :], ctp[:, :])

        # level-3 index rows [32, q] as f16 (cells <= 168, exact):
        # transpose of the 32 level-3 columns of `cell`.
        cella = cell[:, :]
        c3c = wpool.tile([P, NH * NPT], f32, name="c3c", tag="c3c")
        nc.vector.tensor_copy(
            c3c[:, :], sap(cella, 12, [cella.ap[0], [16, NH], [1, NPT]]))
        c3p = ps_tr.tile([NH * NPT, P], f32, name="c3p", tag="tr3", bufs=1)
        nc.tensor.transpose(c3p[:, :], c3c[:, :], ident[:, :])
        cT3 = wpool.tile([NH * NPT, P], f16, name="cT3", tag="cT3", bufs=2)
        nc.vector.tensor_copy(cT3[:, :], c3p[:, :])
        # bounce through DRAM; broadcast-read happens in emit_l3
        dma(dap(c3d[blk % 2], 0, [[P, NH * NPT], [1, P]]), cT3[:, :])

        # wrapped int16 index layout for dma_gather: positions i = s*128+q
        # live at [i%16, i//16]; build via 8 [128,16]->[16,128] transposes.
        cW0 = wpool.tile([16, NH, 12, 8], i16, name="cW0", tag="cW0", bufs=2)
        for qhi in range(8):
            stp = ps_tr.tile([16, P], f32, name="stp", tag="tr3", bufs=1)
            nc.tensor.transpose(stp[:, :],
                                cellT[:, qhi * 16:(qhi + 1) * 16],
                                ident[:, :])
            pstr = stp[:, :].ap[0][0]
            src = sap(stp[:, :], 0, [[pstr, 16], [16, NH], [1, 12]])
            d0 = cW0[:, :, :, :]
            dst = sap(d0, qhi, [d0.ap[0], [12 * 8, NH], [8, 12]])
            nc.vector.tensor_copy(dst, src)
        # bounce through DRAM, replicating the 16 wrapped partitions x8
        dma(dap(cwd[blk % 2], 0, [[NW, 16], [1, NW]]),
            cW0[:, :, :, :].rearrange("p a b c -> p (a b c)"))
        cW = wpool.tile([P, NH, 12, 8], i16, name="cW", tag="cW", bufs=3)
        dma(cW[:, :, :, :].rearrange("p a b c -> p (a b c)"),
            dap(cwd[blk % 2], 0, [[0, 8], [NW, 16], [1, NW]]))
        return fq, w4, cW

    # ==================== LayerNorm ====================
    def emit_ln(r, gt, bt, pfx):
        nsum = opool.tile([P, 1], f32, name=f"{pfx}ns", tag=f"{pfx}ns")
        nc.vector.tensor_reduce(nsum[:, :], r[:, :], axis=AX.X, op=op.add,
                                negate=True)
        nmean = opool.tile([P, 1], f32, name=f"{pfx}nm", tag=f"{pfx}nm")
        nc.scalar.mul(nmean[:, :], nsum[:, :], 1.0 / EMB)
        c = opool.tile([P, EMB], f32, name=f"{pfx}c", tag=f"{pfx}c")
        nc.vector.tensor_scalar_add(c[:, :], r[:, :], nmean[:, :])
        csq = opool.tile([P, EMB], f32, name=f"{pfx}sq", tag=f"{pfx}sq")
        ssq = opool.tile([P, 1], f32, name=f"{pfx}ssq", tag=f"{pfx}ssq")
        nc.scalar.activation(csq[:, :], c[:, :], act_f.Square,
                             accum_out=ssq[:, :])
        std = opool.tile([P, 1], f32, name=f"{pfx}std", tag=f"{pfx}std")
        nc.scalar.activation(std[:, :], ssq[:, :], act_f.Sqrt,
                             bias=eps_t[:, :], scale=1.0 / EMB)
        rstd = opool.tile([P, 1], f32, name=f"{pfx}rs", tag=f"{pfx}rs")
        nc.vector.reciprocal(rstd[:, :], std[:, :])
        x = opool.tile([P, EMB], f32, name=f"{pfx}x", tag=f"{pfx}x")
        nc.vector.scalar_tensor_tensor(x[:, :], c[:, :], rstd[:, :], gt[:, :],
                                       op0=op.mult, op1=op.mult)
        nc.vector.tensor_add(x[:, :], x[:, :], bt[:, :])
        return x

    # ==================== gathers for one block (levels 0-2) ============
    def emit_gathers(cW):
        gb = gpool.tile([P, NH * 12, 4 * HD], f16, name="gb", tag="gb",
                        bufs=2)
        for h in range(NH):
            nc.gpsimd.dma_gather(
                out_ap=gb[:, h * 12:(h + 1) * 12, :],
                in_ap=tabC.ap()[h * CROWS:(h + 1) * CROWS, :],
                idxs_ap=cW[:, h, :, :],
                num_idxs=12 * P,
                num_idxs_reg=12 * P,
                elem_size=4 * HD,
                single_packet=False,
            )
        return gb

    # ==================== level-3 via PE one-hot ========================
    def emit_l3(blk):
        gb3 = g3pool.tile([P, NH * NPT, 4 * HD], f16, name="gb3", tag="gb3",
                          bufs=2)
        # broadcast all 32 level-3 index rows across cell-partitions
        nidx = NH * NPT * P
        bca = bcpool.tile([P, NH * NPT, P], f16, name="bca", tag="bca")
        dma(bca, c3d[blk % 2].ap().to_broadcast([P, nidx]).rearrange(
            "p (a b) -> p a b", a=NH * NPT))
        bcb = bcpool.tile([L3 - P, NH * NPT, P], f16, name="bcb", tag="bcb")
        dma(bcb, c3d[blk % 2].ap().to_broadcast([L3 - P, nidx]).rearrange(
            "p (a b) -> p a b", a=NH * NPT))
        for h in range(NH):
            for pt in range(NPT):
                s3 = h * NPT + pt
                oha = wpool.tile([P, P], f16, name="oha", tag="oha", bufs=2)
                nc.vector.tensor_scalar(oha[:, :], bca[:, s3, :],
                                        iota_a[:, :], None, op0=op.is_equal)
                ohb = wpool.tile([L3 - P, P], f16, name="ohb", tag="ohb",
                                 bufs=2)
                nc.vector.tensor_scalar(ohb[:, :], bcb[:, s3, :],
                                        iota_b[:, :], None, op0=op.is_equal)
                ps3 = ps_sm.tile([P, 4 * HD], f32, name="ps3", tag="sm")
                nc.tensor.matmul(ps3[:, :], oha[:, :], t3a[:, h, :],
                                 start=True, stop=False)
                nc.tensor.matmul(ps3[:, :], ohb[:, :], t3b[:, h, :],
                                 start=False, stop=True)
                nc.scalar.mul(gb3[:, s3, :], ps3[:, :], 1.0)
        return gb3

    # ==================== combine + backend for one block ================
    def emit_backend(blk, fq, w4, gb, gb3):
        acat = kpool.tile([P, EMB], f32, name="acat", tag="acat")
        gba = gb[:, :, :]
        pstr = gba.ap[0][0]
        g3a = gb3[:, :, :]
        p3str = g3a.ap[0][0]
        wstr = w4[:, :, :].ap[0][0]

        def gsl(off, dims):
            return sap(gba, off, [[pstr, P]] + dims)

        def g3l(off, dims):
            return sap(g3a, off, [[p3str, P]] + dims)

        # multiply by bilinear*attention weights (broadcast over head_dim)
        w4g = sap(w4[:, :, :], 0,
                  [[wstr, P], [64, NH], [4, 12], [1, 4], [0, HD]])
        gall = gsl(0, [[12 * 128, NH], [128, 12], [HD, 4], [1, HD]])
        nc.vector.tensor_mul(gall, gall, w4g)
        w43 = sap(w4[:, :, :], 48,
                  [[wstr, P], [64, NH], [4, NPT], [1, 4], [0, HD]])
        g3ll = g3l(0, [[4 * 128, NH], [128, NPT], [HD, 4], [1, HD]])
        nc.vector.tensor_mul(g3ll, g3ll, w43)

        # corner folds: c0+=c1, c2+=c3, c0+=c2
        d2 = [[128, NH * 12], [1, HD]]
        nc.vector.tensor_add(gsl(0, d2), gsl(0, d2), gsl(HD, d2))
        nc.vector.tensor_add(gsl(2 * HD, d2), gsl(2 * HD, d2), gsl(3 * HD, d2))
        nc.vector.tensor_add(gsl(0, d2), gsl(0, d2), gsl(2 * HD, d2))
        d3 = [[128, NH * NPT], [1, HD]]
        nc.vector.tensor_add(g3l(0, d3), g3l(0, d3), g3l(HD, d3))
        nc.vector.tensor_add(g3l(2 * HD, d3), g3l(2 * HD, d3),
                             g3l(3 * HD, d3))
        nc.vector.tensor_add(g3l(0, d3), g3l(0, d3), g3l(2 * HD, d3))

        # level folds within each head: lp[0..4) += lp[4..8), lp[8..12), l3
        dl = [[12 * 128, NH], [128, NPT], [1, HD]]
        nc.vector.tensor_add(gsl(0, dl), gsl(0, dl), gsl(4 * 128, dl))
        nc.vector.tensor_add(gsl(0, dl), gsl(0, dl), gsl(8 * 128, dl))
        d3l = [[4 * 128, NH], [128, NPT], [1, HD]]
        nc.vector.tensor_add(gsl(0, dl), gsl(0, dl), g3l(0, d3l))
        # point folds: 4 -> 2 -> 1 (final fold writes acat slices)
        dp = [[12 * 128, NH], [128, 2], [1, HD]]
        nc.vector.tensor_add(gsl(0, dp), gsl(0, dp), gsl(2 * 128, dp))
        acv = sap(acat[:, :], 0, [[acat[:, :].ap[0][0], P], [HD, NH], [1, HD]])
        dh1 = [[12 * 128, NH], [1, HD]]
        nc.vector.tensor_add(acv, gsl(0, dh1), gsl(128, dh1))

        # ---- output projection + LN + FFN + LN ----
        atp = ps_tr.tile([P, 2, P], f32, name="atp", tag="tr")
        nc.tensor.transpose(atp[:, 0, :], acat[:, 0:P], ident[:, :])
        nc.tensor.transpose(atp[:, 1, :], acat[:, P:EMB], ident[:, :])
        ats = opool.tile([P, 2, P], f16, name="ats", tag="ats")
        nc.vector.tensor_copy(ats[:, :, :], atp[:, :, :])
        oprj = ps_mm.tile([P, EMB], f32, name="oprj", tag="mm")
        mm(oprj, [(ats[:, 0, :], Wout[:, 0, :]),
                  (ats[:, 1, :], Wout[:, 1, :])], bias=bout[:1, :])

        r1 = opool.tile([P, EMB], f32, name="r1", tag="r1")
        nc.vector.tensor_add(r1[:, :], oprj[:, :], fq[:, :])
        x1 = emit_ln(r1, ln1g, ln1b, "la")

        xtp = ps_tr.tile([P, 2, P], f32, name="xtp", tag="tr")
        nc.tensor.transpose(xtp[:, 0, :], x1[:, 0:P], ident[:, :])
        nc.tensor.transpose(xtp[:, 1, :], x1[:, P:EMB], ident[:, :])
        xts = opool.tile([P, 2, P], f16, name="xts", tag="xts")
        nc.vector.tensor_copy(xts[:, :, :], xtp[:, :, :])

        h1s = opool.tile([P, DFFN // P, P], f16, name="h1s", tag="h1s")
        hp = ps_mm.tile([P, DFFN // P, P], f32, name="hp", tag="hpw", bufs=1)
        for mt in range(DFFN // P):
            nc.tensor.matmul(hp[:, mt, :], W1[:, 0, mt * P:(mt + 1) * P],
                             xts[:, 0, :], start=True, stop=False)
            nc.tensor.matmul(hp[:, mt, :], W1[:, 1, mt * P:(mt + 1) * P],
                             xts[:, 1, :], start=False, stop=False)
            nc.tensor.matmul(hp[:, mt, :], b1r[:1, mt * P:(mt + 1) * P],
                             onesr[:1, :], start=False, stop=True)
        nc.scalar.activation(h1s[:, :, :], hp[:, :, :], act_f.Relu)

        yp = ps_mm.tile([P, EMB], f32, name="yp", tag="mm")
        for mt in range(DFFN // P):
            nc.tensor.matmul(yp[:, :], h1s[:, mt, :], W2[:, mt, :],
                             start=(mt == 0), stop=False)
        nc.tensor.matmul(yp[:, :], onesr[:1, :], b2r[:1, :],
                         start=False, stop=True)

        r2 = opool.tile([P, EMB], f32, name="r2", tag="r2")
        nc.vector.tensor_add(r2[:, :], yp[:, :], x1[:, :])
        x2 = emit_ln(r2, ln2g, ln2b, "lb")
        dma(outs["out_q"][blk * P:(blk + 1) * P, :], x2)

    # ==================== top-level schedule ====================
    F = {0: emit_frontend(0), 1: emit_frontend(1)}
    emit_value()
    for g in range(NBQ):
        if g + 2 < NBQ:
            F[g + 2] = emit_frontend(g + 2)
        fq, w4, cW = F.pop(g)
        gb = emit_gathers(cW)
        gb3 = emit_l3(g)
        emit_backend(g, fq, w4, gb, gb3)

    ctx.close()


# ------------------------------------------------------------ host entry ---

_CACHE = {}


def build_nc(cfg):
    from concourse import bacc, mybir, tile

    nc = bacc.Bacc("TRN2", debug=False)
    f32 = mybir.dt.float32
    f16 = mybir.dt.float16

    def di(name, shape, dt=None):
        return nc.dram_tensor(name, list(shape), dt or f32,
                              kind="ExternalInput").ap()

    HQ, LPAD = cfg["HQ"], cfg["LPAD"]
    ins = dict(
        feat_val=di("feat_val", [LPAD, EMB], f16),
        feat_q=di("feat_q", [HQ, EMB]),
        pos_q=di("pos_q", [HQ, EMB], f16),
        ref_q=di("ref_q", [HQ, NL, 2]),
        W_val=di("W_val", [EMB, EMB]), b_val=di("b_val", [1, EMB]),
        W_off=di("W_off", [EMB, EMB]), b_off=di("b_off", [1, EMB]),
        W_attn=di("W_attn", [EMB, NH * NL * NPT]),
        b_attn=di("b_attn", [1, NH * NL * NPT]),
        W_out=di("W_out", [EMB, EMB]), b_out=di("b_out", [1, EMB]),
        W1=di("W1", [EMB, DFFN]), b1=di("b1", [1, DFFN]),
        W2=di("W2", [DFFN, EMB]), b2=di("b2", [1, EMB]),
        ln1_g=di("ln1_g", [1, EMB]), ln1_b=di("ln1_b", [1, EMB]),
        ln2_g=di("ln2_g", [1, EMB]), ln2_b=di("ln2_b", [1, EMB]),
        cst_xy=di("cst_xy", [4, EMB]),
        cst_hlp=di("cst_hlp", [3, P]),
        ident=di("ident", [P, P]),
        ones_row=di("ones_row", [1, P]),
        iota=di("iota", [L3, 1]),
    )
    outs = dict(
        out_q=nc.dram_tensor("out_q", [HQ, EMB], f32,
                             kind="ExternalOutput").ap(),
    )
    with tile.TileContext(nc) as tc:
        emit_kernel(tc, outs, ins, cfg)
    nc.compile()
    return nc


def make_in_maps(inputs, cfg):
    feats = np.asarray(inputs["features"], np.float32)
    pos = np.asarray(inputs["pos"], np.float32)
    refp = np.asarray(inputs["reference_points"], np.float32)
    B = feats.shape[0]
    HQ, LPAD, L = cfg["HQ"], cfg["LPAD"], cfg["L"]
    hw, vstart = cfg["hw"], cfg["vstart"]
    starts = np.cumsum([0] + hw)[:-1]
    half = L // 2

    consts = host_constants(cfg)
    wkeys = dict(
        W_val=inputs["W_val"], b_val=np.reshape(inputs["b_val"], (1, -1)),
        W_off=inputs["W_off"], b_off=np.reshape(inputs["b_off"], (1, -1)),
        W_attn=inputs["W_attn"], b_attn=np.reshape(inputs["b_attn"], (1, -1)),
        W_out=inputs["W_out"], b_out=np.reshape(inputs["b_out"], (1, -1)),
        W1=inputs["W1"], b1=np.reshape(inputs["b1"], (1, -1)),
        W2=inputs["W2"], b2=np.reshape(inputs["b2"], (1, -1)),
        ln1_g=np.reshape(inputs["ln1_g"], (1, -1)),
        ln1_b=np.reshape(inputs["ln1_b"], (1, -1)),
        ln2_g=np.reshape(inputs["ln2_g"], (1, -1)),
        ln2_b=np.reshape(inputs["ln2_b"], (1, -1)),
    )
    wkeys = {k: np.ascontiguousarray(np.asarray(v, np.float32))
             for k, v in wkeys.items()}

    halves = [(0, half), (half, L)]
    in_maps = []
    for core in range(2 * B):
        b, hf = core // 2, core % 2
        s, e = halves[hf]
        fv = np.zeros((LPAD, EMB), np.float16)
        f16b = feats[b].astype(np.float16)
        for l in range(NL):
            fv[vstart[l]:vstart[l] + hw[l]] = \
                f16b[starts[l]:starts[l] + hw[l]]
        fq = np.zeros((HQ, EMB), np.float32)
        fq[:e - s] = feats[b, s:e]
        pq = np.zeros((HQ, EMB), np.float16)
        pq[:e - s] = pos[b, s:e].astype(np.float16)
        rq = np.zeros((HQ, NL, 2), np.float32)
        rq[:e - s] = refp[b, s:e]
        m = dict(feat_val=fv, feat_q=fq, pos_q=pq, ref_q=rq)
        m.update(wkeys)
        m.update({k: np.ascontiguousarray(np.asarray(v, np.float32))
                  for k, v in consts.items()})
        in_maps.append(m)
    return in_maps, halves


def kernel(**inputs):
    from concourse import bass_utils

    cfg = CFG_FULL
    in_maps, halves = make_in_maps(inputs, cfg)
    B = np.asarray(inputs["features"]).shape[0]
    L = cfg["L"]

    if "nc" not in _CACHE:
        _CACHE["nc"] = build_nc(cfg)
    nc = _CACHE["nc"]

    res = bass_utils.run_bass_kernel_spmd(nc, in_maps,
                                          core_ids=list(range(2 * B)))
    out = np.zeros((B, L, EMB), np.float32)
    for core in range(2 * B):
        b, hf = core // 2, core % 2
        s, e = halves[hf]
        out[b, s:e] = res.results[core]["out_q"][:e - s]
    return out


# revision 18
# speedup vs baseline: 955.9037x; 1.1276x over previous
"""Trainium2 Bass kernel for a Deformable-DETR style encoder block.

Sharding: 8 NeuronCores = 4 batch samples x 2 query-halves.

Per core:
  - value projection over the full sample -> fp16 "patch table" in DRAM:
    for cell (y,x) and head h the 2x2 neighborhood [V[y,x], V[y,x+1],
    V[y+1,x], V[y+1,x+1]] is packed contiguously (4*32 fp16 = 256B).
    Levels 0-2 go into one head-major table (rows h*13125 + lvloff + cell)
    so a single gpsimd.dma_gather per (block, head) fetches all 12
    (level, point) patches for 128 queries (1536 records/call, int16
    indices wrapped 16-way and replicated for the 8 Q7 cores).
  - level 3 (13x13) skips the gather entirely: its patch table lives in
    SBUF and a one-hot [cells x queries] matrix from the PE selects
    patches into PSUM (one matmul pair per head/point).
  - offset/attention projections, softmax, bilinear weights and cell
    indices computed query-major; cell indices are PE-transposed into the
    wrapped int16 index layout dma_gather wants.
  - DVE multiplies by bilinear*attention weights and tree-reduces.
  - output projection + LayerNorm + FFN + LayerNorm, then DMA out.
  - frontends are emitted ahead of the previous block's combine/backend
    so the gpsimd gather stream never stalls.
"""

import numpy as np
from contextlib import ExitStack

EMB = 256
NH = 8
NL = 4
NPT = 4
HD = 32
DFFN = 1024
P = 128

# value/table build order: small levels first so tables are ready early
LVL_ORDER = [3, 2, 1, 0]
# levels 0-2 combined per-head table: row = h*CROWS + LVLOFF[l] + cell
LVLOFF = {0: 0, 1: 10000, 2: 12500}
CROWS = 13125  # 10000 + 2500 + 625
L3 = 169       # 13*13 cells in level 3


def make_cfg(shapes, n_blk_q):
    L = sum(h * w for h, w in shapes)
    hw = [h * w for h, w in shapes]
    pblk = {l: -(-hw[l] // P) for l in range(NL)}
    vstart = {}
    off = 0
    for l in LVL_ORDER:
        vstart[l] = off
        off += pblk[l] * P
    return dict(
        shapes=[tuple(s) for s in shapes], hw=hw, L=L,
        pblk=pblk, vstart=vstart, LPAD=off, NBF=off // P,
        NBQ=n_blk_q, HQ=n_blk_q * P,
    )


CFG_FULL = make_cfg([(100, 100), (50, 50), (25, 25), (13, 13)], 52)


# ------------------------------------------------------- host-side consts ---

def host_constants(cfg):
    shapes = cfg["shapes"]
    invnorm = np.zeros(EMB, np.float32)
    pixscale = np.zeros(EMB, np.float32)
    clipmax = np.zeros(EMB, np.float32)
    vmax = np.zeros(EMB, np.float32)
    for h in range(NH):
        for l, (H_, W_) in enumerate(shapes):
            for pt in range(NPT):
                base = h * (NL * NPT * 2) + l * (NPT * 2) + pt * 2
                invnorm[base + 0] = 1.0 / W_
                invnorm[base + 1] = 1.0 / H_
                pixscale[base + 0] = W_
                pixscale[base + 1] = H_
                clipmax[base + 0] = W_ - 2
                clipmax[base + 1] = H_ - 2
                vmax[base + 0] = W_ - 1
                vmax[base + 1] = H_ - 1
    cst_xy = np.stack([invnorm, pixscale, clipmax, vmax])

    wrow = np.zeros(P, np.float32)
    srow = np.zeros(P, np.float32)
    for h in range(NH):
        for l, (H_, W_) in enumerate(shapes):
            for pt in range(NPT):
                base = h * (NL * NPT) + l * NPT + pt
                wrow[base] = W_
                srow[base] = float(LVLOFF.get(l, 0))
    cst_hlp = np.stack([wrow, srow, np.zeros(P, np.float32)])

    ident = np.eye(P, dtype=np.float32)
    ones_row = np.ones((1, P), np.float32)
    iota = np.arange(L3, dtype=np.float32).reshape(L3, 1)
    return dict(cst_xy=cst_xy, cst_hlp=cst_hlp, ident=ident,
                ones_row=ones_row, iota=iota)


# ------------------------------------------------------------- emission ---

def emit_kernel(tc, outs, ins, cfg):
    import concourse.bass as bass
    from concourse import mybir

    nc = tc.nc
    op = mybir.AluOpType
    act_f = mybir.ActivationFunctionType
    f32, f16 = mybir.dt.float32, mybir.dt.float16
    i16 = mybir.dt.int16
    AX = mybir.AxisListType

    shapes, hw = cfg["shapes"], cfg["hw"]
    NBQ = cfg["NBQ"]
    pblk = cfg["pblk"]

    ctx = ExitStack()

    def dap(handle, offset, dims):
        return bass.AP(tensor=handle, offset=offset,
                       ap=[list(d) for d in dims])

    def sap(ap0, extra_off, dims):
        return bass.AP(tensor=ap0.tensor, offset=ap0.offset + extra_off,
                       ap=[list(d) for d in dims])

    # ---- internal DRAM: per-level value + patch tables ----
    valf = {l: nc.dram_tensor(f"valf{l}", [pblk[l] * P, EMB], f16,
                              kind="Internal") for l in range(NL)}
    tabC = nc.dram_tensor("tabC", [NH * CROWS, 4 * HD], f16, kind="Internal")
    tab3 = nc.dram_tensor("tab3", [NH * L3, 4 * HD], f16, kind="Internal")
    # partition-replication bounce buffers (DMA broadcast goes via DRAM)
    NW = NH * 12 * 8
    cwd = [nc.dram_tensor(f"cwd{i}", [16 * NW], i16, kind="Internal")
           for i in range(4)]
    c3d = [nc.dram_tensor(f"c3d{i}", [1, NH * NPT * P], f16, kind="Internal")
           for i in range(4)]

    # ---- pools ----
    cpool = ctx.enter_context(tc.tile_pool(name="consts", bufs=1))
    apool = ctx.enter_context(tc.tile_pool(name="acts", bufs=3))
    wpool = ctx.enter_context(tc.tile_pool(name="wmath", bufs=1))
    gpool = ctx.enter_context(tc.tile_pool(name="gath", bufs=2))
    g3pool = ctx.enter_context(tc.tile_pool(name="gath3", bufs=2))
    bcpool = ctx.enter_context(tc.tile_pool(name="bcast", bufs=1))
    kpool = ctx.enter_context(tc.tile_pool(name="comb", bufs=2))
    opool = ctx.enter_context(tc.tile_pool(name="outp", bufs=2))
    ps_tr = ctx.enter_context(tc.tile_pool(name="ps_tr", bufs=2, space="PSUM"))
    ps_mm = ctx.enter_context(tc.tile_pool(name="ps_mm", bufs=2, space="PSUM"))
    ps_sm = ctx.enter_context(tc.tile_pool(name="ps_sm", bufs=1, space="PSUM"))

    def dma(out_ap, in_ap):
        nc.sync.dma_start(out=out_ap, in_=in_ap)

    # ---- constants / weights ----
    def load_w(name, k, n):
        tmp = apool.tile([P, 2 * DFFN], f32, name="wtmp", tag="wtmp",
                         bufs=1)
        tv = sap(tmp[:, :], 0, [tmp[:, :].ap[0], [n, k // P], [1, n]])
        dma(tv, ins[name].rearrange("(a p) n -> p a n", p=P))
        t = cpool.tile([P, k // P, n], f16, name=f"s_{name}")
        nc.vector.tensor_copy(t[:, :, :], tv)
        return t

    Wval = load_w("W_val", EMB, EMB)
    Woff = load_w("W_off", EMB, EMB)
    Watt = load_w("W_attn", EMB, NH * NL * NPT)
    Wout = load_w("W_out", EMB, EMB)
    W1 = load_w("W1", EMB, DFFN)
    W2 = load_w("W2", DFFN, EMB)

    def load_row(name, n):
        t = cpool.tile([1, n], f32, name=f"r_{name}")
        dma(t, ins[name][:, :])
        return t

    bval = load_row("b_val", EMB)
    boff = load_row("b_off", EMB)
    batt = load_row("b_attn", NH * NL * NPT)
    bout = load_row("b_out", EMB)
    b1r = load_row("b1", DFFN)
    b2r = load_row("b2", EMB)
    onesr = load_row("ones_row", P)

    def load_bc(src_ap, n, name):
        t = cpool.tile([P, n], f32, name=f"b_{name}")
        dma(t, src_ap.to_broadcast([P, n]))
        return t

    ln1g = load_bc(ins["ln1_g"][:, :], EMB, "ln1g")
    ln1b = load_bc(ins["ln1_b"][:, :], EMB, "ln1b")
    ln2g = load_bc(ins["ln2_g"][:, :], EMB, "ln2g")
    ln2b = load_bc(ins["ln2_b"][:, :], EMB, "ln2b")
    c_invn = load_bc(ins["cst_xy"][0:1, :], EMB, "invn")
    c_pixs = load_bc(ins["cst_xy"][1:2, :], EMB, "pixs")
    c_clip = load_bc(ins["cst_xy"][2:3, :], EMB, "clip")
    c_vmax = load_bc(ins["cst_xy"][3:4, :], EMB, "vmax")
    c_W = load_bc(ins["cst_hlp"][0:1, :], P, "cw")
    c_S = load_bc(ins["cst_hlp"][1:2, :], P, "cs")

    ident = cpool.tile([P, P], f32, name="ident")
    dma(ident, ins["ident"][:, :])
    eps_t = cpool.tile([P, 1], f32, name="eps_t")
    nc.vector.memset(eps_t[:, :], 1e-5)

    iota_a = cpool.tile([P, 1], f32, name="iota_a")
    dma(iota_a, ins["iota"][0:P, :])
    iota_b = cpool.tile([L3 - P, 1], f32, name="iota_b")
    dma(iota_b, ins["iota"][P:L3, :])

    refr = cpool.tile([P, NBQ, 2 * NL], f32, name="refr")
    dma(refr, ins["ref_q"].rearrange("(b p) l c -> p b (l c)", p=P))

    # level-3 patch table resident in SBUF: [cell, head, 128]
    t3a = cpool.tile([P, NH, 4 * HD], f16, name="t3a")
    t3b = cpool.tile([L3 - P, NH, 4 * HD], f16, name="t3b")

    def mm(psum_ap, pairs, bias=None):
        seq = list(pairs)
        if bias is not None:
            seq.append((onesr[:1, :psum_ap.shape[0]], bias))
        for i, (lt, rt) in enumerate(seq):
            nc.tensor.matmul(psum_ap, lt, rt,
                             start=(i == 0), stop=(i == len(seq) - 1))

    # ======================= patch-table build (per level) ==============
    def table_dst(l, h):
        if l == 3:
            return tab3, (h * L3) * 4 * HD
        return tabC, (h * CROWS + LVLOFF[l]) * 4 * HD

    def emit_table(l):
        H_, W_ = shapes[l]
        vt = valf[l]
        for h in range(NH):
            tt, tbase = table_dst(l, h)
            for cy in (0, 1):
                for cx in (0, 1):
                    c = cy * 2 + cx
                    src = dap(vt, (cy * W_ + cx) * EMB + h * HD,
                              [[W_ * EMB, H_ - 1], [EMB, W_ - 1], [1, HD]])
                    dst = dap(tt, tbase + c * HD,
                              [[W_ * 4 * HD, H_ - 1], [4 * HD, W_ - 1],
                               [1, HD]])
                    dma(dst, src)
            # fill never-gathered edge records (x=W-1 col, y=H-1 row) so the
            # table contains no uninitialized (possibly non-finite) bytes
            dma(dap(tt, tbase + (W_ - 1) * 4 * HD,
                    [[W_ * 4 * HD, H_], [HD, 4], [1, HD]]),
                dap(vt, (W_ - 1) * EMB + h * HD,
                    [[W_ * EMB, H_], [0, 4], [1, HD]]))
            dma(dap(tt, tbase + ((H_ - 1) * W_) * 4 * HD,
                    [[4 * HD, W_ - 1], [HD, 4], [1, HD]]),
                dap(vt, ((H_ - 1) * W_) * EMB + h * HD,
                    [[EMB, W_ - 1], [0, 4], [1, HD]]))
        if l == 3:
            # SBUF copy for the PE-side gather: [cell, head, 128]
            dma(t3a, tab3.ap().rearrange("(h c) d -> c h d", h=NH)[0:P])
            dma(t3b, tab3.ap().rearrange("(h c) d -> c h d", h=NH)[P:L3])

    # ======================= value projection ===========================
    def emit_value():
        idf16 = cpool.tile([P, P], f16, name="idf16")
        nc.vector.tensor_copy(idf16[:, :], ident[:, :])
        sched = []
        for l in LVL_ORDER:
            for lb in range(pblk[l]):
                sched.append((l, lb, lb == pblk[l] - 1))
        for blk, (l, lb, last) in enumerate(sched):
            fv = apool.tile([P, EMB], f16, name="fv", tag="fv")
            dma(fv, ins["feat_val"][blk * P:(blk + 1) * P, :])
            ftp = ps_tr.tile([P, 2, P], f16, name="ftp", tag="tr")
            nc.tensor.transpose(ftp[:, 0, :], fv[:, 0:P], idf16[:, :])
            nc.tensor.transpose(ftp[:, 1, :], fv[:, P:EMB], idf16[:, :])
            fts = apool.tile([P, 2, P], f16, name="fts", tag="fts")
            nc.vector.tensor_copy(fts[:, :, :], ftp[:, :, :])
            vp = ps_mm.tile([P, EMB], f32, name="vp", tag="mm")
            mm(vp, [(fts[:, 0, :], Wval[:, 0, :]),
                    (fts[:, 1, :], Wval[:, 1, :])], bias=bval[:1, :])
            vf = apool.tile([P, EMB], f16, name="vf", tag="vf")
            nc.vector.tensor_copy(vf[:, :], vp[:, :])
            dma(valf[l].ap()[lb * P:(lb + 1) * P, :], vf)
            if last:
                emit_table(l)

    # ==================== per-block frontend ====================
    def emit_frontend(blk):
        fq = apool.tile([P, EMB], f32, name="fq", tag="fq", bufs=4)
        dma(fq, ins["feat_q"][blk * P:(blk + 1) * P, :])
        pq = apool.tile([P, EMB], f16, name="pq", tag="pq")
        dma(pq, ins["pos_q"][blk * P:(blk + 1) * P, :])
        qb = apool.tile([P, EMB], f32, name="qb", tag="qb")
        nc.vector.tensor_add(qb[:, :], fq[:, :], pq[:, :])

        qtp = ps_tr.tile([P, 2, P], f32, name="qtp", tag="tr")
        nc.tensor.transpose(qtp[:, 0, :], qb[:, 0:P], ident[:, :])
        nc.tensor.transpose(qtp[:, 1, :], qb[:, P:EMB], ident[:, :])
        qts = apool.tile([P, 2, P], f16, name="qts", tag="qts", bufs=2)
        nc.vector.tensor_copy(qts[:, :, :], qtp[:, :, :])

        offp = ps_mm.tile([P, EMB], f32, name="offp", tag="mm")
        mm(offp, [(qts[:, 0, :], Woff[:, 0, :]), (qts[:, 1, :], Woff[:, 1, :])],
           bias=boff[:1, :])
        off = wpool.tile([P, EMB], f32, name="off", tag="off")
        nc.vector.tensor_copy(off[:, :], offp[:, :])

        attp = ps_sm.tile([P, NH * 16], f32, name="attp", tag="sm")
        mm(attp, [(qts[:, 0, :], Watt[:, 0, :]), (qts[:, 1, :], Watt[:, 1, :])],
           bias=batt[:1, :])
        att = wpool.tile([P, NH, 16], f32, name="att", tag="att")
        nc.vector.tensor_copy(att[:, :, :], attp[:, :].rearrange(
            "p (h l) -> p h l", h=NH))

        # softmax over (l,pt) per head
        rmax = wpool.tile([P, NH], f32, name="rmax", tag="rmax")
        nc.vector.reduce_max(rmax[:, :], att[:, :, :], axis=AX.X)
        exv = wpool.tile([P, NH, 16], f32, name="exv", tag="exv")
        rmaxa = rmax[:, :]
        nc.vector.tensor_sub(exv[:, :, :], att[:, :, :],
                             sap(rmaxa, 0, [rmaxa.ap[0], [1, NH], [0, 16]]))
        nc.scalar.activation(exv[:, :, :], exv[:, :, :], act_f.Exp)
        ssum = wpool.tile([P, NH], f32, name="ssum", tag="ssum")
        nc.vector.reduce_sum(ssum[:, :], exv[:, :, :], axis=AX.X)
        rsum = wpool.tile([P, NH], f32, name="rsum", tag="rsum")
        nc.vector.reciprocal(rsum[:, :], ssum[:, :])
        aw = wpool.tile([P, NH, 16], f32, name="aw", tag="aw")
        rsuma = rsum[:, :]
        nc.vector.tensor_mul(aw[:, :, :], exv[:, :, :],
                             sap(rsuma, 0, [rsuma.ap[0], [1, NH], [0, 16]]))

        def wt(name):
            return wpool.tile([P, EMB], f32, name=name, tag=name)

        loc = wt("loc")
        nc.vector.tensor_mul(loc[:, :], off[:, :], c_invn[:, :])
        refa = refr[:, blk, :]
        for xy in (0, 1):
            lvh = sap(loc[:, :], xy, [loc[:, :].ap[0], [32, NH], [8, NL],
                                      [2, NPT]])
            nc.vector.tensor_add(lvh, lvh,
                                 sap(refa, xy, [refa.ap[0], [0, NH], [2, NL],
                                                [0, NPT]]))
        pix = wt("pix")
        nc.vector.tensor_mul(pix[:, :], loc[:, :], c_pixs[:, :])
        nc.vector.tensor_scalar_add(pix[:, :], pix[:, :], -0.5)

        # floor(pix) robust to cast rounding mode
        xi = wpool.tile([P, EMB], mybir.dt.int32, name="xi", tag="xi")
        nc.vector.tensor_copy(xi[:, :], pix[:, :])
        base = wt("base")
        nc.vector.tensor_copy(base[:, :], xi[:, :])
        fixm = wt("fixm")
        nc.vector.tensor_tensor(fixm[:, :], pix[:, :], base[:, :], op=op.is_lt)
        nc.vector.tensor_sub(base[:, :], base[:, :], fixm[:, :])
        wfrac = wt("wfrac")
        nc.vector.tensor_sub(wfrac[:, :], pix[:, :], base[:, :])

        basec = wt("basec")
        nc.vector.tensor_scalar_max(basec[:, :], base[:, :], 0.0)
        nc.vector.tensor_tensor(basec[:, :], basec[:, :], c_clip[:, :],
                                op=op.min)

        v0b = wt("v0b")
        nc.vector.tensor_tensor(v0b[:, :], base[:, :], c_vmax[:, :],
                                op=op.is_le)
        vld0 = wt("vld0")
        nc.vector.scalar_tensor_tensor(vld0[:, :], base[:, :], 0.0, v0b[:, :],
                                       op0=op.is_ge, op1=op.mult)
        v1b = wt("v1b")
        nc.vector.tensor_tensor(v1b[:, :], base[:, :], c_clip[:, :],
                                op=op.is_le)
        vld1 = wt("vld1")
        nc.vector.scalar_tensor_tensor(vld1[:, :], base[:, :], -1.0, v1b[:, :],
                                       op0=op.is_ge, op1=op.mult)

        tsh = wt("tsh")
        nc.vector.tensor_sub(tsh[:, :], base[:, :], basec[:, :])
        e0 = wt("e0")
        nc.vector.tensor_scalar(e0[:, :], tsh[:, :], 0.0, None,
                                op0=op.is_equal)
        em1 = wt("em1")
        nc.vector.tensor_scalar(em1[:, :], tsh[:, :], -1.0, None,
                                op0=op.is_equal)
        ep1 = wt("ep1")
        nc.vector.tensor_scalar(ep1[:, :], tsh[:, :], 1.0, None,
                                op0=op.is_equal)

        u0 = wt("u0")
        nc.vector.tensor_scalar(u0[:, :], wfrac[:, :], -1.0, 1.0, op0=op.mult,
                                op1=op.add)
        nc.vector.tensor_mul(u0[:, :], u0[:, :], vld0[:, :])
        u1 = wt("u1")
        nc.vector.tensor_mul(u1[:, :], wfrac[:, :], vld1[:, :])

        a0 = wt("a0")
        nc.vector.tensor_mul(a0[:, :], u0[:, :], e0[:, :])
        t1 = wt("t1")
        nc.vector.tensor_mul(t1[:, :], u1[:, :], em1[:, :])
        nc.vector.tensor_add(a0[:, :], a0[:, :], t1[:, :])
        a1 = wt("a1")
        nc.vector.tensor_mul(a1[:, :], u0[:, :], ep1[:, :])
        nc.vector.tensor_mul(t1[:, :], u1[:, :], e0[:, :])
        nc.vector.tensor_add(a1[:, :], a1[:, :], t1[:, :])

        def ycols(t):
            return sap(t[:, :], 1, [[t[:, :].ap[0][0], P], [2, P]])

        def xcols(t):
            return sap(t[:, :], 0, [[t[:, :].ap[0][0], P], [2, P]])

        awf = aw.rearrange("p h l -> p (h l)")
        ay0 = wpool.tile([P, P], f32, name="ay0", tag="ay0")
        nc.vector.tensor_mul(ay0[:, :], ycols(a0), awf)
        ay1 = wpool.tile([P, P], f32, name="ay1", tag="ay1")
        nc.vector.tensor_mul(ay1[:, :], ycols(a1), awf)

        w4 = wpool.tile([P, P, 4], f16, name="w4", tag="w4", bufs=3)
        nc.vector.tensor_mul(w4[:, :, 0], ay0[:, :], xcols(a0))
        nc.vector.tensor_mul(w4[:, :, 1], ay0[:, :], xcols(a1))
        nc.vector.tensor_mul(w4[:, :, 2], ay1[:, :], xcols(a0))
        nc.vector.tensor_mul(w4[:, :, 3], ay1[:, :], xcols(a1))

        # cell index within level (+ level offset for the combined table)
        cell = wpool.tile([P, P], f32, name="cell", tag="cell")
        nc.vector.tensor_mul(cell[:, :], ycols(basec), c_W[:, :])
        nc.vector.tensor_add(cell[:, :], cell[:, :], xcols(basec))
        nc.vector.tensor_add(cell[:, :], cell[:, :], c_S[:, :])

        # transpose -> cellT [slot, q]
        ctp = ps_tr.tile([P, P], f32, name="ctp", tag="tr")
        nc.tensor.transpose(ctp[:, :], cell[:, :], ident[:, :])
        cellT = wpool.tile([P, P], f32, name="cellT", tag="cellT", bufs=2)
        nc.vector.tensor_copy(cellT[:, :], ctp[:, :])

        # level-3 index rows [32, q] as f16 (cells <= 168, exact):
        # transpose of the 32 level-3 columns of `cell`.
        cella = cell[:, :]
        c3c = wpool.tile([P, NH * NPT], f32, name="c3c", tag="c3c")
        nc.vector.tensor_copy(
            c3c[:, :], sap(cella, 12, [cella.ap[0], [16, NH], [1, NPT]]))
        c3p = ps_tr.tile([NH * NPT, P], f32, name="c3p", tag="tr3", bufs=1)
        nc.tensor.transpose(c3p[:, :], c3c[:, :], ident[:, :])
        cT3 = wpool.tile([NH * NPT, P], f16, name="cT3", tag="cT3", bufs=2)
        nc.vector.tensor_copy(cT3[:, :], c3p[:, :])
        # bounce through DRAM; broadcast-read happens in emit_l3
        dma(dap(c3d[blk % 4], 0, [[P, NH * NPT], [1, P]]), cT3[:, :])

        # wrapped int16 index layout for dma_gather: positions i = s*128+q
        # live at [i%16, i//16]; build via 8 [128,16]->[16,128] transposes.
        cW0 = wpool.tile([16, NH, 12, 8], i16, name="cW0", tag="cW0", bufs=2)
        for qhi in range(8):
            stp = ps_tr.tile([16, P], f32, name="stp", tag="tr3", bufs=1)
            nc.tensor.transpose(stp[:, :],
                                cellT[:, qhi * 16:(qhi + 1) * 16],
                                ident[:, :])
            pstr = stp[:, :].ap[0][0]
            src = sap(stp[:, :], 0, [[pstr, 16], [16, NH], [1, 12]])
            d0 = cW0[:, :, :, :]
            dst = sap(d0, qhi, [d0.ap[0], [12 * 8, NH], [8, 12]])
            nc.vector.tensor_copy(dst, src)
        # bounce through DRAM, replicating the 16 wrapped partitions x8
        dma(dap(cwd[blk % 4], 0, [[NW, 16], [1, NW]]),
            cW0[:, :, :, :].rearrange("p a b c -> p (a b c)"))
        cW = wpool.tile([P, NH, 12, 8], i16, name="cW", tag="cW", bufs=3)
        dma(cW[:, :, :, :].rearrange("p a b c -> p (a b c)"),
            dap(cwd[blk % 4], 0, [[0, 8], [NW, 16], [1, NW]]))
        return fq, w4, cW

    # ==================== LayerNorm ====================
    def emit_ln(r, gt, bt, pfx):
        nsum = opool.tile([P, 1], f32, name=f"{pfx}ns", tag=f"{pfx}ns")
        nc.vector.tensor_reduce(nsum[:, :], r[:, :], axis=AX.X, op=op.add,
                                negate=True)
        nmean = opool.tile([P, 1], f32, name=f"{pfx}nm", tag=f"{pfx}nm")
        nc.scalar.mul(nmean[:, :], nsum[:, :], 1.0 / EMB)
        c = opool.tile([P, EMB], f32, name=f"{pfx}c", tag=f"{pfx}c")
        nc.vector.tensor_scalar_add(c[:, :], r[:, :], nmean[:, :])
        csq = opool.tile([P, EMB], f32, name=f"{pfx}sq", tag=f"{pfx}sq")
        ssq = opool.tile([P, 1], f32, name=f"{pfx}ssq", tag=f"{pfx}ssq")
        nc.scalar.activation(csq[:, :], c[:, :], act_f.Square,
                             accum_out=ssq[:, :])
        std = opool.tile([P, 1], f32, name=f"{pfx}std", tag=f"{pfx}std")
        nc.scalar.activation(std[:, :], ssq[:, :], act_f.Sqrt,
                             bias=eps_t[:, :], scale=1.0 / EMB)
        rstd = opool.tile([P, 1], f32, name=f"{pfx}rs", tag=f"{pfx}rs")
        nc.vector.reciprocal(rstd[:, :], std[:, :])
        x = opool.tile([P, EMB], f32, name=f"{pfx}x", tag=f"{pfx}x")
        nc.vector.scalar_tensor_tensor(x[:, :], c[:, :], rstd[:, :], gt[:, :],
                                       op0=op.mult, op1=op.mult)
        nc.vector.tensor_add(x[:, :], x[:, :], bt[:, :])
        return x

    # ==================== gathers for one block (levels 0-2) ============
    def emit_gathers(cW):
        gb = gpool.tile([P, NH * 12, 4 * HD], f16, name="gb", tag="gb",
                        bufs=2)
        for h in range(NH):
            nc.gpsimd.dma_gather(
                out_ap=gb[:, h * 12:(h + 1) * 12, :],
                in_ap=tabC.ap()[h * CROWS:(h + 1) * CROWS, :],
                idxs_ap=cW[:, h, :, :],
                num_idxs=12 * P,
                num_idxs_reg=12 * P,
                elem_size=4 * HD,
                single_packet=False,
            )
        return gb

    # ==================== level-3 via PE one-hot ========================
    def emit_l3(blk):
        gb3 = g3pool.tile([P, NH * NPT, 4 * HD], f16, name="gb3", tag="gb3",
                          bufs=2)
        # broadcast all 32 level-3 index rows across cell-partitions
        nidx = NH * NPT * P
        bca = bcpool.tile([P, NH * NPT, P], f16, name="bca", tag="bca")
        dma(bca, c3d[blk % 4].ap().to_broadcast([P, nidx]).rearrange(
            "p (a b) -> p a b", a=NH * NPT))
        bcb = bcpool.tile([L3 - P, NH * NPT, P], f16, name="bcb", tag="bcb")
        dma(bcb, c3d[blk % 4].ap().to_broadcast([L3 - P, nidx]).rearrange(
            "p (a b) -> p a b", a=NH * NPT))
        for h in range(NH):
            for pt in range(NPT):
                s3 = h * NPT + pt
                oha = wpool.tile([P, P], f16, name="oha", tag="oha", bufs=2)
                nc.vector.tensor_scalar(oha[:, :], bca[:, s3, :],
                                        iota_a[:, :], None, op0=op.is_equal)
                ohb = wpool.tile([L3 - P, P], f16, name="ohb", tag="ohb",
                                 bufs=2)
                nc.vector.tensor_scalar(ohb[:, :], bcb[:, s3, :],
                                        iota_b[:, :], None, op0=op.is_equal)
                ps3 = ps_sm.tile([P, 4 * HD], f32, name="ps3", tag="sm")
                nc.tensor.matmul(ps3[:, :], oha[:, :], t3a[:, h, :],
                                 start=True, stop=False)
                nc.tensor.matmul(ps3[:, :], ohb[:, :], t3b[:, h, :],
                                 start=False, stop=True)
                nc.scalar.mul(gb3[:, s3, :], ps3[:, :], 1.0)
        return gb3

    # ==================== combine + backend for one block ================
    def emit_backend(blk, fq, w4, gb, gb3):
        acat = kpool.tile([P, EMB], f32, name="acat", tag="acat")
        gba = gb[:, :, :]
        pstr = gba.ap[0][0]
        g3a = gb3[:, :, :]
        p3str = g3a.ap[0][0]
        wstr = w4[:, :, :].ap[0][0]

        def gsl(off, dims):
            return sap(gba, off, [[pstr, P]] + dims)

        def g3l(off, dims):
            return sap(g3a, off, [[p3str, P]] + dims)

        # multiply by bilinear*attention weights (broadcast over head_dim)
        w4g = sap(w4[:, :, :], 0,
                  [[wstr, P], [64, NH], [4, 12], [1, 4], [0, HD]])
        gall = gsl(0, [[12 * 128, NH], [128, 12], [HD, 4], [1, HD]])
        nc.vector.tensor_mul(gall, gall, w4g)
        w43 = sap(w4[:, :, :], 48,
                  [[wstr, P], [64, NH], [4, NPT], [1, 4], [0, HD]])
        g3ll = g3l(0, [[4 * 128, NH], [128, NPT], [HD, 4], [1, HD]])
        nc.vector.tensor_mul(g3ll, g3ll, w43)

        # corner folds: c0+=c1, c2+=c3, c0+=c2
        d2 = [[128, NH * 12], [1, HD]]
        nc.vector.tensor_add(gsl(0, d2), gsl(0, d2), gsl(HD, d2))
        nc.vector.tensor_add(gsl(2 * HD, d2), gsl(2 * HD, d2), gsl(3 * HD, d2))
        nc.vector.tensor_add(gsl(0, d2), gsl(0, d2), gsl(2 * HD, d2))
        d3 = [[128, NH * NPT], [1, HD]]
        nc.vector.tensor_add(g3l(0, d3), g3l(0, d3), g3l(HD, d3))
        nc.vector.tensor_add(g3l(2 * HD, d3), g3l(2 * HD, d3),
                             g3l(3 * HD, d3))
        nc.vector.tensor_add(g3l(0, d3), g3l(0, d3), g3l(2 * HD, d3))

        # level folds within each head: lp[0..4) += lp[4..8), lp[8..12), l3
        dl = [[12 * 128, NH], [128, NPT], [1, HD]]
        nc.vector.tensor_add(gsl(0, dl), gsl(0, dl), gsl(4 * 128, dl))
        nc.vector.tensor_add(gsl(0, dl), gsl(0, dl), gsl(8 * 128, dl))
        d3l = [[4 * 128, NH], [128, NPT], [1, HD]]
        nc.vector.tensor_add(gsl(0, dl), gsl(0, dl), g3l(0, d3l))
        # point folds: 4 -> 2 -> 1 (final fold writes acat slices)
        dp = [[12 * 128, NH], [128, 2], [1, HD]]
        nc.vector.tensor_add(gsl(0, dp), gsl(0, dp), gsl(2 * 128, dp))
        acv = sap(acat[:, :], 0, [[acat[:, :].ap[0][0], P], [HD, NH], [1, HD]])
        dh1 = [[12 * 128, NH], [1, HD]]
        nc.vector.tensor_add(acv, gsl(0, dh1), gsl(128, dh1))

        # ---- output projection + LN + FFN + LN ----
        atp = ps_tr.tile([P, 2, P], f32, name="atp", tag="tr")
        nc.tensor.transpose(atp[:, 0, :], acat[:, 0:P], ident[:, :])
        nc.tensor.transpose(atp[:, 1, :], acat[:, P:EMB], ident[:, :])
        ats = opool.tile([P, 2, P], f16, name="ats", tag="ats")
        nc.vector.tensor_copy(ats[:, :, :], atp[:, :, :])
        oprj = ps_mm.tile([P, EMB], f32, name="oprj", tag="mm")
        mm(oprj, [(ats[:, 0, :], Wout[:, 0, :]),
                  (ats[:, 1, :], Wout[:, 1, :])], bias=bout[:1, :])

        r1 = opool.tile([P, EMB], f32, name="r1", tag="r1")
        nc.vector.tensor_add(r1[:, :], oprj[:, :], fq[:, :])
        x1 = emit_ln(r1, ln1g, ln1b, "la")

        xtp = ps_tr.tile([P, 2, P], f32, name="xtp", tag="tr")
        nc.tensor.transpose(xtp[:, 0, :], x1[:, 0:P], ident[:, :])
        nc.tensor.transpose(xtp[:, 1, :], x1[:, P:EMB], ident[:, :])
        xts = opool.tile([P, 2, P], f16, name="xts", tag="xts")
        nc.vector.tensor_copy(xts[:, :, :], xtp[:, :, :])

        h1s = opool.tile([P, DFFN // P, P], f16, name="h1s", tag="h1s")
        hp = ps_mm.tile([P, DFFN // P, P], f32, name="hp", tag="hpw", bufs=1)
        for mt in range(DFFN // P):
            nc.tensor.matmul(hp[:, mt, :], W1[:, 0, mt * P:(mt + 1) * P],
                             xts[:, 0, :], start=True, stop=False)
            nc.tensor.matmul(hp[:, mt, :], W1[:, 1, mt * P:(mt + 1) * P],
                             xts[:, 1, :], start=False, stop=False)
            nc.tensor.matmul(hp[:, mt, :], b1r[:1, mt * P:(mt + 1) * P],
                             onesr[:1, :], start=False, stop=True)
        nc.scalar.activation(h1s[:, :, :], hp[:, :, :], act_f.Relu)

        yp = ps_mm.tile([P, EMB], f32, name="yp", tag="mm")
        for mt in range(DFFN // P):
            nc.tensor.matmul(yp[:, :], h1s[:, mt, :], W2[:, mt, :],
                             start=(mt == 0), stop=False)
        nc.tensor.matmul(yp[:, :], onesr[:1, :], b2r[:1, :],
                         start=False, stop=True)

        r2 = opool.tile([P, EMB], f32, name="r2", tag="r2")
        nc.vector.tensor_add(r2[:, :], yp[:, :], x1[:, :])
        x2 = emit_ln(r2, ln2g, ln2b, "lb")
        dma(outs["out_q"][blk * P:(blk + 1) * P, :], x2)

    # ==================== top-level schedule ====================
    F = {0: emit_frontend(0), 1: emit_frontend(1)}
    emit_value()
    for g in range(NBQ):
        if g + 2 < NBQ:
            F[g + 2] = emit_frontend(g + 2)
        fq, w4, cW = F.pop(g)
        gb = emit_gathers(cW)
        gb3 = emit_l3(g)
        emit_backend(g, fq, w4, gb, gb3)

    ctx.close()


# ------------------------------------------------------------ host entry ---

_CACHE = {}


def build_nc(cfg):
    from concourse import bacc, mybir, tile

    nc = bacc.Bacc("TRN2", debug=False)
    f32 = mybir.dt.float32
    f16 = mybir.dt.float16

    def di(name, shape, dt=None):
        return nc.dram_tensor(name, list(shape), dt or f32,
                              kind="ExternalInput").ap()

    HQ, LPAD = cfg["HQ"], cfg["LPAD"]
    ins = dict(
        feat_val=di("feat_val", [LPAD, EMB], f16),
        feat_q=di("feat_q", [HQ, EMB]),
        pos_q=di("pos_q", [HQ, EMB], f16),
        ref_q=di("ref_q", [HQ, NL, 2]),
        W_val=di("W_val", [EMB, EMB]), b_val=di("b_val", [1, EMB]),
        W_off=di("W_off", [EMB, EMB]), b_off=di("b_off", [1, EMB]),
        W_attn=di("W_attn", [EMB, NH * NL * NPT]),
        b_attn=di("b_attn", [1, NH * NL * NPT]),
        W_out=di("W_out", [EMB, EMB]), b_out=di("b_out", [1, EMB]),
        W1=di("W1", [EMB, DFFN]), b1=di("b1", [1, DFFN]),
        W2=di("W2", [DFFN, EMB]), b2=di("b2", [1, EMB]),
        ln1_g=di("ln1_g", [1, EMB]), ln1_b=di("ln1_b", [1, EMB]),
        ln2_g=di("ln2_g", [1, EMB]), ln2_b=di("ln2_b", [1, EMB]),
        cst_xy=di("cst_xy", [4, EMB]),
        cst_hlp=di("cst_hlp", [3, P]),
        ident=di("ident", [P, P]),
        ones_row=di("ones_row", [1, P]),
        iota=di("iota", [L3, 1]),
    )
    outs = dict(
        out_q=nc.dram_tensor("out_q", [HQ, EMB], f32,
                             kind="ExternalOutput").ap(),
    )
    with tile.TileContext(nc) as tc:
        emit_kernel(tc, outs, ins, cfg)
    nc.compile()
    return nc


def make_in_maps(inputs, cfg):
    feats = np.asarray(inputs["features"], np.float32)
    pos = np.asarray(inputs["pos"], np.float32)
    refp = np.asarray(inputs["reference_points"], np.float32)
    B = feats.shape[0]
    HQ, LPAD, L = cfg["HQ"], cfg["LPAD"], cfg["L"]
    hw, vstart = cfg["hw"], cfg["vstart"]
    starts = np.cumsum([0] + hw)[:-1]
    half = L // 2

    consts = host_constants(cfg)
    wkeys = dict(
        W_val=inputs["W_val"], b_val=np.reshape(inputs["b_val"], (1, -1)),
        W_off=inputs["W_off"], b_off=np.reshape(inputs["b_off"], (1, -1)),
        W_attn=inputs["W_attn"], b_attn=np.reshape(inputs["b_attn"], (1, -1)),
        W_out=inputs["W_out"], b_out=np.reshape(inputs["b_out"], (1, -1)),
        W1=inputs["W1"], b1=np.reshape(inputs["b1"], (1, -1)),
        W2=inputs["W2"], b2=np.reshape(inputs["b2"], (1, -1)),
        ln1_g=np.reshape(inputs["ln1_g"], (1, -1)),
        ln1_b=np.reshape(inputs["ln1_b"], (1, -1)),
        ln2_g=np.reshape(inputs["ln2_g"], (1, -1)),
        ln2_b=np.reshape(inputs["ln2_b"], (1, -1)),
    )
    wkeys = {k: np.ascontiguousarray(np.asarray(v, np.float32))
             for k, v in wkeys.items()}

    halves = [(0, half), (half, L)]
    in_maps = []
    for core in range(2 * B):
        b, hf = core // 2, core % 2
        s, e = halves[hf]
        fv = np.zeros((LPAD, EMB), np.float16)
        f16b = feats[b].astype(np.float16)
        for l in range(NL):
            fv[vstart[l]:vstart[l] + hw[l]] = \
                f16b[starts[l]:starts[l] + hw[l]]
        fq = np.zeros((HQ, EMB), np.float32)
        fq[:e - s] = feats[b, s:e]
        pq = np.zeros((HQ, EMB), np.float16)
        pq[:e - s] = pos[b, s:e].astype(np.float16)
        rq = np.zeros((HQ, NL, 2), np.float32)
        rq[:e - s] = refp[b, s:e]
        m = dict(feat_val=fv, feat_q=fq, pos_q=pq, ref_q=rq)
        m.update(wkeys)
        m.update({k: np.ascontiguousarray(np.asarray(v, np.float32))
                  for k, v in consts.items()})
        in_maps.append(m)
    return in_maps, halves


def kernel(**inputs):
    from concourse import bass_utils

    cfg = CFG_FULL
    in_maps, halves = make_in_maps(inputs, cfg)
    B = np.asarray(inputs["features"]).shape[0]
    L = cfg["L"]

    if "nc" not in _CACHE:
        _CACHE["nc"] = build_nc(cfg)
    nc = _CACHE["nc"]

    res = bass_utils.run_bass_kernel_spmd(nc, in_maps,
                                          core_ids=list(range(2 * B)))
    out = np.zeros((B, L, EMB), np.float32)
    for core in range(2 * B):
        b, hf = core // 2, core % 2
        s, e = halves[hf]
        out[b, s:e] = res.results[core]["out_q"][:e - s]
    return out


# revision 19
# speedup vs baseline: 976.6987x; 1.0218x over previous
"""Trainium2 Bass kernel for a Deformable-DETR style encoder block.

Sharding: 8 NeuronCores = 4 batch samples x 2 query-halves.

Per core:
  - value projection over the full sample -> fp16 "patch table" in DRAM:
    for cell (y,x) and head h the 2x2 neighborhood [V[y,x], V[y,x+1],
    V[y+1,x], V[y+1,x+1]] is packed contiguously (4*32 fp16 = 256B).
    Levels 0-2 go into one head-major table (rows h*13125 + lvloff + cell)
    so a single gpsimd.dma_gather per (block, head) fetches all 12
    (level, point) patches for 128 queries (1536 records/call, int16
    indices wrapped 16-way and replicated for the 8 Q7 cores).
  - level 3 (13x13) skips the gather entirely: its patch table lives in
    SBUF and a one-hot [cells x queries] matrix from the PE selects
    patches into PSUM (one matmul pair per head/point).
  - offset/attention projections, softmax, bilinear weights and cell
    indices computed query-major; cell indices are PE-transposed into the
    wrapped int16 index layout dma_gather wants.
  - DVE multiplies by bilinear*attention weights and tree-reduces.
  - output projection + LayerNorm + FFN + LayerNorm, then DMA out.
  - frontends are emitted ahead of the previous block's combine/backend
    so the gpsimd gather stream never stalls.
"""

import numpy as np
from contextlib import ExitStack

EMB = 256
NH = 8
NL = 4
NPT = 4
HD = 32
DFFN = 1024
P = 128

# value/table build order: small levels first so tables are ready early
LVL_ORDER = [3, 2, 1, 0]
# levels 0-2 combined per-head table: row = h*CROWS + LVLOFF[l] + cell
LVLOFF = {0: 0, 1: 10000, 2: 12500}
CROWS = 13125  # 10000 + 2500 + 625
L3 = 169       # 13*13 cells in level 3


def make_cfg(shapes, n_blk_q):
    L = sum(h * w for h, w in shapes)
    hw = [h * w for h, w in shapes]
    pblk = {l: -(-hw[l] // P) for l in range(NL)}
    vstart = {}
    off = 0
    for l in LVL_ORDER:
        vstart[l] = off
        off += pblk[l] * P
    return dict(
        shapes=[tuple(s) for s in shapes], hw=hw, L=L,
        pblk=pblk, vstart=vstart, LPAD=off, NBF=off // P,
        NBQ=n_blk_q, HQ=n_blk_q * P,
    )


CFG_FULL = make_cfg([(100, 100), (50, 50), (25, 25), (13, 13)], 52)


# ------------------------------------------------------- host-side consts ---

def host_constants(cfg):
    shapes = cfg["shapes"]
    invnorm = np.zeros(EMB, np.float32)
    pixscale = np.zeros(EMB, np.float32)
    clipmax = np.zeros(EMB, np.float32)
    vmax = np.zeros(EMB, np.float32)
    for h in range(NH):
        for l, (H_, W_) in enumerate(shapes):
            for pt in range(NPT):
                base = h * (NL * NPT * 2) + l * (NPT * 2) + pt * 2
                invnorm[base + 0] = 1.0 / W_
                invnorm[base + 1] = 1.0 / H_
                pixscale[base + 0] = W_
                pixscale[base + 1] = H_
                clipmax[base + 0] = W_ - 2
                clipmax[base + 1] = H_ - 2
                vmax[base + 0] = W_ - 1
                vmax[base + 1] = H_ - 1
    cst_xy = np.stack([invnorm, pixscale, clipmax, vmax])

    wrow = np.zeros(P, np.float32)
    srow = np.zeros(P, np.float32)
    for h in range(NH):
        for l, (H_, W_) in enumerate(shapes):
            for pt in range(NPT):
                base = h * (NL * NPT) + l * NPT + pt
                wrow[base] = W_
                srow[base] = float(LVLOFF.get(l, 0))
    cst_hlp = np.stack([wrow, srow, np.zeros(P, np.float32)])

    ident = np.eye(P, dtype=np.float32)
    ones_row = np.ones((1, P), np.float32)
    iota = np.arange(L3, dtype=np.float32).reshape(L3, 1)
    return dict(cst_xy=cst_xy, cst_hlp=cst_hlp, ident=ident,
                ones_row=ones_row, iota=iota)


# ------------------------------------------------------------- emission ---

def emit_kernel(tc, outs, ins, cfg):
    import concourse.bass as bass
    from concourse import mybir

    nc = tc.nc
    op = mybir.AluOpType
    act_f = mybir.ActivationFunctionType
    f32, f16 = mybir.dt.float32, mybir.dt.float16
    i16 = mybir.dt.int16
    AX = mybir.AxisListType

    shapes, hw = cfg["shapes"], cfg["hw"]
    NBQ = cfg["NBQ"]
    pblk = cfg["pblk"]

    ctx = ExitStack()

    def dap(handle, offset, dims):
        return bass.AP(tensor=handle, offset=offset,
                       ap=[list(d) for d in dims])

    def sap(ap0, extra_off, dims):
        return bass.AP(tensor=ap0.tensor, offset=ap0.offset + extra_off,
                       ap=[list(d) for d in dims])

    # ---- internal DRAM: per-level value + patch tables ----
    valf = {l: nc.dram_tensor(f"valf{l}", [pblk[l] * P, EMB], f16,
                              kind="Internal") for l in range(NL)}
    tabC = nc.dram_tensor("tabC", [NH * CROWS, 4 * HD], f16, kind="Internal")
    tab3 = nc.dram_tensor("tab3", [NH * L3, 4 * HD], f16, kind="Internal")
    # partition-replication bounce buffers (DMA broadcast goes via DRAM)
    NW = NH * 12 * 8
    cwd = [nc.dram_tensor(f"cwd{i}", [16 * NW], i16, kind="Internal")
           for i in range(4)]
    c3d = [nc.dram_tensor(f"c3d{i}", [1, NH * NPT * P], f16, kind="Internal")
           for i in range(4)]

    # ---- pools ----
    cpool = ctx.enter_context(tc.tile_pool(name="consts", bufs=1))
    apool = ctx.enter_context(tc.tile_pool(name="acts", bufs=3))
    wpool = ctx.enter_context(tc.tile_pool(name="wmath", bufs=1))
    gpool = ctx.enter_context(tc.tile_pool(name="gath", bufs=2))
    g3pool = ctx.enter_context(tc.tile_pool(name="gath3", bufs=2))
    bcpool = ctx.enter_context(tc.tile_pool(name="bcast", bufs=1))
    kpool = ctx.enter_context(tc.tile_pool(name="comb", bufs=2))
    opool = ctx.enter_context(tc.tile_pool(name="outp", bufs=2))
    ps_tr = ctx.enter_context(tc.tile_pool(name="ps_tr", bufs=2, space="PSUM"))
    ps_mm = ctx.enter_context(tc.tile_pool(name="ps_mm", bufs=2, space="PSUM"))
    ps_sm = ctx.enter_context(tc.tile_pool(name="ps_sm", bufs=1, space="PSUM"))

    def dma(out_ap, in_ap):
        nc.sync.dma_start(out=out_ap, in_=in_ap)

    # ---- constants / weights ----
    def load_w(name, k, n):
        tmp = apool.tile([P, 2 * DFFN], f32, name="wtmp", tag="wtmp",
                         bufs=1)
        tv = sap(tmp[:, :], 0, [tmp[:, :].ap[0], [n, k // P], [1, n]])
        dma(tv, ins[name].rearrange("(a p) n -> p a n", p=P))
        t = cpool.tile([P, k // P, n], f16, name=f"s_{name}")
        nc.vector.tensor_copy(t[:, :, :], tv)
        return t

    Wval = load_w("W_val", EMB, EMB)
    Woff = load_w("W_off", EMB, EMB)
    Watt = load_w("W_attn", EMB, NH * NL * NPT)
    Wout = load_w("W_out", EMB, EMB)
    W1 = load_w("W1", EMB, DFFN)
    W2 = load_w("W2", DFFN, EMB)

    def load_row(name, n):
        t = cpool.tile([1, n], f32, name=f"r_{name}")
        dma(t, ins[name][:, :])
        return t

    bval = load_row("b_val", EMB)
    boff = load_row("b_off", EMB)
    batt = load_row("b_attn", NH * NL * NPT)
    bout = load_row("b_out", EMB)
    b1r = load_row("b1", DFFN)
    b2r = load_row("b2", EMB)
    onesr = load_row("ones_row", P)

    def load_bc(src_ap, n, name):
        t = cpool.tile([P, n], f32, name=f"b_{name}")
        dma(t, src_ap.to_broadcast([P, n]))
        return t

    ln1g = load_bc(ins["ln1_g"][:, :], EMB, "ln1g")
    ln1b = load_bc(ins["ln1_b"][:, :], EMB, "ln1b")
    ln2g = load_bc(ins["ln2_g"][:, :], EMB, "ln2g")
    ln2b = load_bc(ins["ln2_b"][:, :], EMB, "ln2b")
    c_invn = load_bc(ins["cst_xy"][0:1, :], EMB, "invn")
    c_pixs = load_bc(ins["cst_xy"][1:2, :], EMB, "pixs")
    c_clip = load_bc(ins["cst_xy"][2:3, :], EMB, "clip")
    c_vmax = load_bc(ins["cst_xy"][3:4, :], EMB, "vmax")
    c_W = load_bc(ins["cst_hlp"][0:1, :], P, "cw")
    c_S = load_bc(ins["cst_hlp"][1:2, :], P, "cs")

    ident = cpool.tile([P, P], f32, name="ident")
    dma(ident, ins["ident"][:, :])
    eps_t = cpool.tile([P, 1], f32, name="eps_t")
    nc.vector.memset(eps_t[:, :], 1e-5)

    iota_a = cpool.tile([P, 1], f32, name="iota_a")
    dma(iota_a, ins["iota"][0:P, :])
    iota_b = cpool.tile([L3 - P, 1], f32, name="iota_b")
    dma(iota_b, ins["iota"][P:L3, :])

    refr = cpool.tile([P, NBQ, 2 * NL], f32, name="refr")
    dma(refr, ins["ref_q"].rearrange("(b p) l c -> p b (l c)", p=P))

    # level-3 patch table resident in SBUF: [cell, head, 128]
    t3a = cpool.tile([P, NH, 4 * HD], f16, name="t3a")
    t3b = cpool.tile([L3 - P, NH, 4 * HD], f16, name="t3b")

    def mm(psum_ap, pairs, bias=None):
        seq = list(pairs)
        if bias is not None:
            seq.append((onesr[:1, :psum_ap.shape[0]], bias))
        for i, (lt, rt) in enumerate(seq):
            nc.tensor.matmul(psum_ap, lt, rt,
                             start=(i == 0), stop=(i == len(seq) - 1))

    # ======================= patch-table build (per level) ==============
    def table_dst(l, h):
        if l == 3:
            return tab3, (h * L3) * 4 * HD
        return tabC, (h * CROWS + LVLOFF[l]) * 4 * HD

    def emit_table(l):
        H_, W_ = shapes[l]
        vt = valf[l]
        for h in range(NH):
            tt, tbase = table_dst(l, h)
            for cy in (0, 1):
                for cx in (0, 1):
                    c = cy * 2 + cx
                    src = dap(vt, (cy * W_ + cx) * EMB + h * HD,
                              [[W_ * EMB, H_ - 1], [EMB, W_ - 1], [1, HD]])
                    dst = dap(tt, tbase + c * HD,
                              [[W_ * 4 * HD, H_ - 1], [4 * HD, W_ - 1],
                               [1, HD]])
                    dma(dst, src)
            # fill never-gathered edge records (x=W-1 col, y=H-1 row) so the
            # table contains no uninitialized (possibly non-finite) bytes
            dma(dap(tt, tbase + (W_ - 1) * 4 * HD,
                    [[W_ * 4 * HD, H_], [HD, 4], [1, HD]]),
                dap(vt, (W_ - 1) * EMB + h * HD,
                    [[W_ * EMB, H_], [0, 4], [1, HD]]))
            dma(dap(tt, tbase + ((H_ - 1) * W_) * 4 * HD,
                    [[4 * HD, W_ - 1], [HD, 4], [1, HD]]),
                dap(vt, ((H_ - 1) * W_) * EMB + h * HD,
                    [[EMB, W_ - 1], [0, 4], [1, HD]]))
        if l == 3:
            # SBUF copy for the PE-side gather: [cell, head, 128]
            dma(t3a, tab3.ap().rearrange("(h c) d -> c h d", h=NH)[0:P])
            dma(t3b, tab3.ap().rearrange("(h c) d -> c h d", h=NH)[P:L3])

    # ======================= value projection ===========================
    def emit_value():
        idf16 = cpool.tile([P, P], f16, name="idf16")
        nc.vector.tensor_copy(idf16[:, :], ident[:, :])
        sched = []
        for l in LVL_ORDER:
            for lb in range(pblk[l]):
                sched.append((l, lb, lb == pblk[l] - 1))
        for blk, (l, lb, last) in enumerate(sched):
            fv = apool.tile([P, EMB], f16, name="fv", tag="fv")
            dma(fv, ins["feat_val"][blk * P:(blk + 1) * P, :])
            ftp = ps_tr.tile([P, 2, P], f16, name="ftp", tag="tr")
            nc.tensor.transpose(ftp[:, 0, :], fv[:, 0:P], idf16[:, :])
            nc.tensor.transpose(ftp[:, 1, :], fv[:, P:EMB], idf16[:, :])
            fts = apool.tile([P, 2, P], f16, name="fts", tag="fts")
            nc.vector.tensor_copy(fts[:, :, :], ftp[:, :, :])
            vp = ps_mm.tile([P, EMB], f32, name="vp", tag="mm")
            mm(vp, [(fts[:, 0, :], Wval[:, 0, :]),
                    (fts[:, 1, :], Wval[:, 1, :])], bias=bval[:1, :])
            vf = apool.tile([P, EMB], f16, name="vf", tag="vf")
            nc.vector.tensor_copy(vf[:, :], vp[:, :])
            dma(valf[l].ap()[lb * P:(lb + 1) * P, :], vf)
            if last:
                emit_table(l)

    # ==================== per-block frontend ====================
    def emit_frontend(blk):
        fq = apool.tile([P, EMB], f32, name="fq", tag="fq", bufs=4)
        dma(fq, ins["feat_q"][blk * P:(blk + 1) * P, :])
        pq = apool.tile([P, EMB], f16, name="pq", tag="pq")
        dma(pq, ins["pos_q"][blk * P:(blk + 1) * P, :])
        qb = apool.tile([P, EMB], f32, name="qb", tag="qb")
        nc.vector.tensor_add(qb[:, :], fq[:, :], pq[:, :])

        qtp = ps_tr.tile([P, 2, P], f32, name="qtp", tag="tr")
        nc.tensor.transpose(qtp[:, 0, :], qb[:, 0:P], ident[:, :])
        nc.tensor.transpose(qtp[:, 1, :], qb[:, P:EMB], ident[:, :])
        qts = apool.tile([P, 2, P], f16, name="qts", tag="qts", bufs=2)
        nc.vector.tensor_copy(qts[:, :, :], qtp[:, :, :])

        offp = ps_mm.tile([P, EMB], f32, name="offp", tag="mm")
        mm(offp, [(qts[:, 0, :], Woff[:, 0, :]), (qts[:, 1, :], Woff[:, 1, :])],
           bias=boff[:1, :])
        off = wpool.tile([P, EMB], f32, name="off", tag="off")
        nc.vector.tensor_copy(off[:, :], offp[:, :])

        attp = ps_sm.tile([P, NH * 16], f32, name="attp", tag="sm")
        mm(attp, [(qts[:, 0, :], Watt[:, 0, :]), (qts[:, 1, :], Watt[:, 1, :])],
           bias=batt[:1, :])
        att = wpool.tile([P, NH, 16], f32, name="att", tag="att")
        nc.vector.tensor_copy(att[:, :, :], attp[:, :].rearrange(
            "p (h l) -> p h l", h=NH))

        # softmax over (l,pt) per head
        rmax = wpool.tile([P, NH], f32, name="rmax", tag="rmax")
        nc.vector.reduce_max(rmax[:, :], att[:, :, :], axis=AX.X)
        exv = wpool.tile([P, NH, 16], f32, name="exv", tag="exv")
        rmaxa = rmax[:, :]
        nc.vector.tensor_sub(exv[:, :, :], att[:, :, :],
                             sap(rmaxa, 0, [rmaxa.ap[0], [1, NH], [0, 16]]))
        nc.scalar.activation(exv[:, :, :], exv[:, :, :], act_f.Exp)
        ssum = wpool.tile([P, NH], f32, name="ssum", tag="ssum")
        nc.vector.reduce_sum(ssum[:, :], exv[:, :, :], axis=AX.X)
        rsum = wpool.tile([P, NH], f32, name="rsum", tag="rsum")
        nc.vector.reciprocal(rsum[:, :], ssum[:, :])
        aw = wpool.tile([P, NH, 16], f32, name="aw", tag="aw")
        rsuma = rsum[:, :]
        nc.vector.tensor_mul(aw[:, :, :], exv[:, :, :],
                             sap(rsuma, 0, [rsuma.ap[0], [1, NH], [0, 16]]))

        def wt(name):
            return wpool.tile([P, EMB], f32, name=name, tag=name)

        loc = wt("loc")
        nc.vector.tensor_mul(loc[:, :], off[:, :], c_invn[:, :])
        refa = refr[:, blk, :]
        for xy in (0, 1):
            lvh = sap(loc[:, :], xy, [loc[:, :].ap[0], [32, NH], [8, NL],
                                      [2, NPT]])
            nc.vector.tensor_add(lvh, lvh,
                                 sap(refa, xy, [refa.ap[0], [0, NH], [2, NL],
                                                [0, NPT]]))
        pix = wt("pix")
        nc.vector.tensor_mul(pix[:, :], loc[:, :], c_pixs[:, :])
        nc.vector.tensor_scalar_add(pix[:, :], pix[:, :], -0.5)

        # floor(pix) robust to cast rounding mode
        xi = wpool.tile([P, EMB], mybir.dt.int32, name="xi", tag="xi")
        nc.vector.tensor_copy(xi[:, :], pix[:, :])
        base = wt("base")
        nc.vector.tensor_copy(base[:, :], xi[:, :])
        fixm = wt("fixm")
        nc.vector.tensor_tensor(fixm[:, :], pix[:, :], base[:, :], op=op.is_lt)
        nc.vector.tensor_sub(base[:, :], base[:, :], fixm[:, :])
        wfrac = wt("wfrac")
        nc.vector.tensor_sub(wfrac[:, :], pix[:, :], base[:, :])

        basec = wt("basec")
        nc.vector.tensor_scalar_max(basec[:, :], base[:, :], 0.0)
        nc.vector.tensor_tensor(basec[:, :], basec[:, :], c_clip[:, :],
                                op=op.min)

        v0b = wt("v0b")
        nc.vector.tensor_tensor(v0b[:, :], base[:, :], c_vmax[:, :],
                                op=op.is_le)
        vld0 = wt("vld0")
        nc.vector.scalar_tensor_tensor(vld0[:, :], base[:, :], 0.0, v0b[:, :],
                                       op0=op.is_ge, op1=op.mult)
        v1b = wt("v1b")
        nc.vector.tensor_tensor(v1b[:, :], base[:, :], c_clip[:, :],
                                op=op.is_le)
        vld1 = wt("vld1")
        nc.vector.scalar_tensor_tensor(vld1[:, :], base[:, :], -1.0, v1b[:, :],
                                       op0=op.is_ge, op1=op.mult)

        tsh = wt("tsh")
        nc.vector.tensor_sub(tsh[:, :], base[:, :], basec[:, :])
        e0 = wt("e0")
        nc.vector.tensor_scalar(e0[:, :], tsh[:, :], 0.0, None,
                                op0=op.is_equal)
        em1 = wt("em1")
        nc.vector.tensor_scalar(em1[:, :], tsh[:, :], -1.0, None,
                                op0=op.is_equal)
        ep1 = wt("ep1")
        nc.vector.tensor_scalar(ep1[:, :], tsh[:, :], 1.0, None,
                                op0=op.is_equal)

        u0 = wt("u0")
        nc.vector.tensor_scalar(u0[:, :], wfrac[:, :], -1.0, 1.0, op0=op.mult,
                                op1=op.add)
        nc.vector.tensor_mul(u0[:, :], u0[:, :], vld0[:, :])
        u1 = wt("u1")
        nc.vector.tensor_mul(u1[:, :], wfrac[:, :], vld1[:, :])

        a0 = wt("a0")
        nc.vector.tensor_mul(a0[:, :], u0[:, :], e0[:, :])
        t1 = wt("t1")
        nc.vector.tensor_mul(t1[:, :], u1[:, :], em1[:, :])
        nc.vector.tensor_add(a0[:, :], a0[:, :], t1[:, :])
        a1 = wt("a1")
        nc.vector.tensor_mul(a1[:, :], u0[:, :], ep1[:, :])
        nc.vector.tensor_mul(t1[:, :], u1[:, :], e0[:, :])
        nc.vector.tensor_add(a1[:, :], a1[:, :], t1[:, :])

        def ycols(t):
            return sap(t[:, :], 1, [[t[:, :].ap[0][0], P], [2, P]])

        def xcols(t):
            return sap(t[:, :], 0, [[t[:, :].ap[0][0], P], [2, P]])

        awf = aw.rearrange("p h l -> p (h l)")
        ay0 = wpool.tile([P, P], f32, name="ay0", tag="ay0")
        nc.vector.tensor_mul(ay0[:, :], ycols(a0), awf)
        ay1 = wpool.tile([P, P], f32, name="ay1", tag="ay1")
        nc.vector.tensor_mul(ay1[:, :], ycols(a1), awf)

        w4 = wpool.tile([P, P, 4], f16, name="w4", tag="w4", bufs=3)
        nc.vector.tensor_mul(w4[:, :, 0], ay0[:, :], xcols(a0))
        nc.vector.tensor_mul(w4[:, :, 1], ay0[:, :], xcols(a1))
        nc.vector.tensor_mul(w4[:, :, 2], ay1[:, :], xcols(a0))
        nc.vector.tensor_mul(w4[:, :, 3], ay1[:, :], xcols(a1))

        # cell index within level (+ level offset for the combined table)
        cell = wpool.tile([P, P], f32, name="cell", tag="cell")
        nc.vector.tensor_mul(cell[:, :], ycols(basec), c_W[:, :])
        nc.vector.tensor_add(cell[:, :], cell[:, :], xcols(basec))
        nc.vector.tensor_add(cell[:, :], cell[:, :], c_S[:, :])

        # transpose -> cellT [slot, q]
        ctp = ps_tr.tile([P, P], f32, name="ctp", tag="tr")
        nc.tensor.transpose(ctp[:, :], cell[:, :], ident[:, :])
        cellT = wpool.tile([P, P], f32, name="cellT", tag="cellT", bufs=2)
        nc.vector.tensor_copy(cellT[:, :], ctp[:, :])

        # level-3 index rows [32, q] as f16 (cells <= 168, exact):
        # transpose of the 32 level-3 columns of `cell`.
        cella = cell[:, :]
        c3c = wpool.tile([P, NH * NPT], f32, name="c3c", tag="c3c")
        nc.vector.tensor_copy(
            c3c[:, :], sap(cella, 12, [cella.ap[0], [16, NH], [1, NPT]]))
        c3p = ps_tr.tile([NH * NPT, P], f32, name="c3p", tag="tr3", bufs=1)
        nc.tensor.transpose(c3p[:, :], c3c[:, :], ident[:, :])
        cT3 = wpool.tile([NH * NPT, P], f16, name="cT3", tag="cT3", bufs=2)
        nc.vector.tensor_copy(cT3[:, :], c3p[:, :])
        # bounce through DRAM; broadcast-read happens in emit_l3
        dma(dap(c3d[blk % 4], 0, [[P, NH * NPT], [1, P]]), cT3[:, :])

        # wrapped int16 index layout for dma_gather: positions i = s*128+q
        # live at [i%16, i//16]; build via 8 [128,16]->[16,128] transposes.
        cW0 = wpool.tile([16, NH, 12, 8], i16, name="cW0", tag="cW0", bufs=2)
        for qhi in range(8):
            stp = ps_tr.tile([16, P], f32, name="stp", tag="tr3", bufs=1)
            nc.tensor.transpose(stp[:, :],
                                cellT[:, qhi * 16:(qhi + 1) * 16],
                                ident[:, :])
            pstr = stp[:, :].ap[0][0]
            src = sap(stp[:, :], 0, [[pstr, 16], [16, NH], [1, 12]])
            d0 = cW0[:, :, :, :]
            dst = sap(d0, qhi, [d0.ap[0], [12 * 8, NH], [8, 12]])
            nc.vector.tensor_copy(dst, src)
        # bounce through DRAM, replicating the 16 wrapped partitions x8
        dma(dap(cwd[blk % 4], 0, [[NW, 16], [1, NW]]),
            cW0[:, :, :, :].rearrange("p a b c -> p (a b c)"))
        cW = wpool.tile([P, NH, 12, 8], i16, name="cW", tag="cW", bufs=3)
        dma(cW[:, :, :, :].rearrange("p a b c -> p (a b c)"),
            dap(cwd[blk % 4], 0, [[0, 8], [NW, 16], [1, NW]]))
        return fq, w4, cW

    # ==================== LayerNorm ====================
    def emit_ln(r, gt, bt, pfx):
        nsum = opool.tile([P, 1], f32, name=f"{pfx}ns", tag=f"{pfx}ns")
        nc.vector.tensor_reduce(nsum[:, :], r[:, :], axis=AX.X, op=op.add,
                                negate=True)
        nmean = opool.tile([P, 1], f32, name=f"{pfx}nm", tag=f"{pfx}nm")
        nc.scalar.mul(nmean[:, :], nsum[:, :], 1.0 / EMB)
        c = opool.tile([P, EMB], f32, name=f"{pfx}c", tag=f"{pfx}c")
        nc.vector.tensor_scalar_add(c[:, :], r[:, :], nmean[:, :])
        csq = opool.tile([P, EMB], f32, name=f"{pfx}sq", tag=f"{pfx}sq")
        ssq = opool.tile([P, 1], f32, name=f"{pfx}ssq", tag=f"{pfx}ssq")
        nc.scalar.activation(csq[:, :], c[:, :], act_f.Square,
                             accum_out=ssq[:, :])
        std = opool.tile([P, 1], f32, name=f"{pfx}std", tag=f"{pfx}std")
        nc.scalar.activation(std[:, :], ssq[:, :], act_f.Sqrt,
                             bias=eps_t[:, :], scale=1.0 / EMB)
        rstd = opool.tile([P, 1], f32, name=f"{pfx}rs", tag=f"{pfx}rs")
        nc.vector.reciprocal(rstd[:, :], std[:, :])
        x = opool.tile([P, EMB], f32, name=f"{pfx}x", tag=f"{pfx}x")
        nc.vector.scalar_tensor_tensor(x[:, :], c[:, :], rstd[:, :], gt[:, :],
                                       op0=op.mult, op1=op.mult)
        nc.vector.tensor_add(x[:, :], x[:, :], bt[:, :])
        return x

    # ==================== gathers for one block (levels 0-2) ============
    def emit_gathers(cW):
        gb = gpool.tile([P, NH * 12, 4 * HD], f16, name="gb", tag="gb",
                        bufs=2)
        for h in range(NH):
            nc.gpsimd.dma_gather(
                out_ap=gb[:, h * 12:(h + 1) * 12, :],
                in_ap=tabC.ap()[h * CROWS:(h + 1) * CROWS, :],
                idxs_ap=cW[:, h, :, :],
                num_idxs=12 * P,
                num_idxs_reg=12 * P,
                elem_size=4 * HD,
                single_packet=False,
            )
        return gb

    # ==================== level-3 via PE one-hot ========================
    def emit_l3(blk):
        gb3 = g3pool.tile([P, NH * NPT, 4 * HD], f16, name="gb3", tag="gb3",
                          bufs=2)
        # broadcast all 32 level-3 index rows across cell-partitions
        nidx = NH * NPT * P
        bca = bcpool.tile([P, NH * NPT, P], f16, name="bca", tag="bca")
        dma(bca, c3d[blk % 4].ap().to_broadcast([P, nidx]).rearrange(
            "p (a b) -> p a b", a=NH * NPT))
        bcb = bcpool.tile([L3 - P, NH * NPT, P], f16, name="bcb", tag="bcb")
        dma(bcb, c3d[blk % 4].ap().to_broadcast([L3 - P, nidx]).rearrange(
            "p (a b) -> p a b", a=NH * NPT))
        for h in range(NH):
            ps3 = ps_sm.tile([P, NPT, 4 * HD], f32, name="ps3", tag="sm")
            for pt in range(NPT):
                s3 = h * NPT + pt
                oha = wpool.tile([P, P], f16, name="oha", tag="oha", bufs=2)
                nc.vector.tensor_scalar(oha[:, :], bca[:, s3, :],
                                        iota_a[:, :], None, op0=op.is_equal)
                ohb = wpool.tile([L3 - P, P], f16, name="ohb", tag="ohb",
                                 bufs=2)
                nc.vector.tensor_scalar(ohb[:, :], bcb[:, s3, :],
                                        iota_b[:, :], None, op0=op.is_equal)
                nc.tensor.matmul(ps3[:, pt, :], oha[:, :], t3a[:, h, :],
                                 start=True, stop=False)
                nc.tensor.matmul(ps3[:, pt, :], ohb[:, :], t3b[:, h, :],
                                 start=False, stop=True)
            nc.scalar.mul(gb3[:, h * NPT:(h + 1) * NPT, :], ps3[:, :, :], 1.0)
        return gb3

    # ==================== combine + backend for one block ================
    def emit_backend(blk, fq, w4, gb, gb3):
        acat = kpool.tile([P, EMB], f32, name="acat", tag="acat")
        gba = gb[:, :, :]
        pstr = gba.ap[0][0]
        g3a = gb3[:, :, :]
        p3str = g3a.ap[0][0]
        wstr = w4[:, :, :].ap[0][0]

        def gsl(off, dims):
            return sap(gba, off, [[pstr, P]] + dims)

        def g3l(off, dims):
            return sap(g3a, off, [[p3str, P]] + dims)

        # multiply by bilinear*attention weights (broadcast over head_dim)
        w4g = sap(w4[:, :, :], 0,
                  [[wstr, P], [64, NH], [4, 12], [1, 4], [0, HD]])
        gall = gsl(0, [[12 * 128, NH], [128, 12], [HD, 4], [1, HD]])
        nc.vector.tensor_mul(gall, gall, w4g)
        w43 = sap(w4[:, :, :], 48,
                  [[wstr, P], [64, NH], [4, NPT], [1, 4], [0, HD]])
        g3ll = g3l(0, [[4 * 128, NH], [128, NPT], [HD, 4], [1, HD]])
        nc.vector.tensor_mul(g3ll, g3ll, w43)

        # corner folds: c0+=c1, c2+=c3, c0+=c2
        d2 = [[128, NH * 12], [1, HD]]
        nc.vector.tensor_add(gsl(0, d2), gsl(0, d2), gsl(HD, d2))
        nc.vector.tensor_add(gsl(2 * HD, d2), gsl(2 * HD, d2), gsl(3 * HD, d2))
        nc.vector.tensor_add(gsl(0, d2), gsl(0, d2), gsl(2 * HD, d2))
        d3 = [[128, NH * NPT], [1, HD]]
        nc.vector.tensor_add(g3l(0, d3), g3l(0, d3), g3l(HD, d3))
        nc.vector.tensor_add(g3l(2 * HD, d3), g3l(2 * HD, d3),
                             g3l(3 * HD, d3))
        nc.vector.tensor_add(g3l(0, d3), g3l(0, d3), g3l(2 * HD, d3))

        # level folds within each head: lp[0..4) += lp[4..8), lp[8..12), l3
        dl = [[12 * 128, NH], [128, NPT], [1, HD]]
        nc.vector.tensor_add(gsl(0, dl), gsl(0, dl), gsl(4 * 128, dl))
        nc.vector.tensor_add(gsl(0, dl), gsl(0, dl), gsl(8 * 128, dl))
        d3l = [[4 * 128, NH], [128, NPT], [1, HD]]
        nc.vector.tensor_add(gsl(0, dl), gsl(0, dl), g3l(0, d3l))
        # point folds: 4 -> 2 -> 1 (final fold writes acat slices)
        dp = [[12 * 128, NH], [128, 2], [1, HD]]
        nc.vector.tensor_add(gsl(0, dp), gsl(0, dp), gsl(2 * 128, dp))
        acv = sap(acat[:, :], 0, [[acat[:, :].ap[0][0], P], [HD, NH], [1, HD]])
        dh1 = [[12 * 128, NH], [1, HD]]
        nc.vector.tensor_add(acv, gsl(0, dh1), gsl(128, dh1))

        # ---- output projection + LN + FFN + LN ----
        atp = ps_tr.tile([P, 2, P], f32, name="atp", tag="tr")
        nc.tensor.transpose(atp[:, 0, :], acat[:, 0:P], ident[:, :])
        nc.tensor.transpose(atp[:, 1, :], acat[:, P:EMB], ident[:, :])
        ats = opool.tile([P, 2, P], f16, name="ats", tag="ats")
        nc.vector.tensor_copy(ats[:, :, :], atp[:, :, :])
        oprj = ps_mm.tile([P, EMB], f32, name="oprj", tag="mm")
        mm(oprj, [(ats[:, 0, :], Wout[:, 0, :]),
                  (ats[:, 1, :], Wout[:, 1, :])], bias=bout[:1, :])

        r1 = opool.tile([P, EMB], f32, name="r1", tag="r1")
        nc.vector.tensor_add(r1[:, :], oprj[:, :], fq[:, :])
        x1 = emit_ln(r1, ln1g, ln1b, "la")

        xtp = ps_tr.tile([P, 2, P], f32, name="xtp", tag="tr")
        nc.tensor.transpose(xtp[:, 0, :], x1[:, 0:P], ident[:, :])
        nc.tensor.transpose(xtp[:, 1, :], x1[:, P:EMB], ident[:, :])
        xts = opool.tile([P, 2, P], f16, name="xts", tag="xts")
        nc.vector.tensor_copy(xts[:, :, :], xtp[:, :, :])

        h1s = opool.tile([P, DFFN // P, P], f16, name="h1s", tag="h1s")
        hp = ps_mm.tile([P, DFFN // P, P], f32, name="hp", tag="hpw", bufs=1)
        for mt in range(DFFN // P):
            nc.tensor.matmul(hp[:, mt, :], W1[:, 0, mt * P:(mt + 1) * P],
                             xts[:, 0, :], start=True, stop=False)
            nc.tensor.matmul(hp[:, mt, :], W1[:, 1, mt * P:(mt + 1) * P],
                             xts[:, 1, :], start=False, stop=False)
            nc.tensor.matmul(hp[:, mt, :], b1r[:1, mt * P:(mt + 1) * P],
                             onesr[:1, :], start=False, stop=True)
        nc.scalar.activation(h1s[:, :, :], hp[:, :, :], act_f.Relu)

        yp = ps_mm.tile([P, EMB], f32, name="yp", tag="mm")
        for mt in range(DFFN // P):
            nc.tensor.matmul(yp[:, :], h1s[:, mt, :], W2[:, mt, :],
                             start=(mt == 0), stop=False)
        nc.tensor.matmul(yp[:, :], onesr[:1, :], b2r[:1, :],
                         start=False, stop=True)

        r2 = opool.tile([P, EMB], f32, name="r2", tag="r2")
        nc.vector.tensor_add(r2[:, :], yp[:, :], x1[:, :])
        x2 = emit_ln(r2, ln2g, ln2b, "lb")
        dma(outs["out_q"][blk * P:(blk + 1) * P, :], x2)

    # ==================== top-level schedule ====================
    F = {0: emit_frontend(0), 1: emit_frontend(1)}
    emit_value()
    G = {0: emit_gathers(F[0][2])}
    for g in range(NBQ):
        if g + 2 < NBQ:
            F[g + 2] = emit_frontend(g + 2)
        gb3 = emit_l3(g)
        if g + 1 < NBQ:
            G[g + 1] = emit_gathers(F[g + 1][2])
        fq, w4, _ = F.pop(g)
        emit_backend(g, fq, w4, G.pop(g), gb3)

    ctx.close()


# ------------------------------------------------------------ host entry ---

_CACHE = {}


def build_nc(cfg):
    from concourse import bacc, mybir, tile

    nc = bacc.Bacc("TRN2", debug=False)
    f32 = mybir.dt.float32
    f16 = mybir.dt.float16

    def di(name, shape, dt=None):
        return nc.dram_tensor(name, list(shape), dt or f32,
                              kind="ExternalInput").ap()

    HQ, LPAD = cfg["HQ"], cfg["LPAD"]
    ins = dict(
        feat_val=di("feat_val", [LPAD, EMB], f16),
        feat_q=di("feat_q", [HQ, EMB]),
        pos_q=di("pos_q", [HQ, EMB], f16),
        ref_q=di("ref_q", [HQ, NL, 2]),
        W_val=di("W_val", [EMB, EMB]), b_val=di("b_val", [1, EMB]),
        W_off=di("W_off", [EMB, EMB]), b_off=di("b_off", [1, EMB]),
        W_attn=di("W_attn", [EMB, NH * NL * NPT]),
        b_attn=di("b_attn", [1, NH * NL * NPT]),
        W_out=di("W_out", [EMB, EMB]), b_out=di("b_out", [1, EMB]),
        W1=di("W1", [EMB, DFFN]), b1=di("b1", [1, DFFN]),
        W2=di("W2", [DFFN, EMB]), b2=di("b2", [1, EMB]),
        ln1_g=di("ln1_g", [1, EMB]), ln1_b=di("ln1_b", [1, EMB]),
        ln2_g=di("ln2_g", [1, EMB]), ln2_b=di("ln2_b", [1, EMB]),
        cst_xy=di("cst_xy", [4, EMB]),
        cst_hlp=di("cst_hlp", [3, P]),
        ident=di("ident", [P, P]),
        ones_row=di("ones_row", [1, P]),
        iota=di("iota", [L3, 1]),
    )
    outs = dict(
        out_q=nc.dram_tensor("out_q", [HQ, EMB], f32,
                             kind="ExternalOutput").ap(),
    )
    with tile.TileContext(nc) as tc:
        emit_kernel(tc, outs, ins, cfg)
    nc.compile()
    return nc


def make_in_maps(inputs, cfg):
    feats = np.asarray(inputs["features"], np.float32)
    pos = np.asarray(inputs["pos"], np.float32)
    refp = np.asarray(inputs["reference_points"], np.float32)
    B = feats.shape[0]
    HQ, LPAD, L = cfg["HQ"], cfg["LPAD"], cfg["L"]
    hw, vstart = cfg["hw"], cfg["vstart"]
    starts = np.cumsum([0] + hw)[:-1]
    half = L // 2

    consts = host_constants(cfg)
    wkeys = dict(
        W_val=inputs["W_val"], b_val=np.reshape(inputs["b_val"], (1, -1)),
        W_off=inputs["W_off"], b_off=np.reshape(inputs["b_off"], (1, -1)),
        W_attn=inputs["W_attn"], b_attn=np.reshape(inputs["b_attn"], (1, -1)),
        W_out=inputs["W_out"], b_out=np.reshape(inputs["b_out"], (1, -1)),
        W1=inputs["W1"], b1=np.reshape(inputs["b1"], (1, -1)),
        W2=inputs["W2"], b2=np.reshape(inputs["b2"], (1, -1)),
        ln1_g=np.reshape(inputs["ln1_g"], (1, -1)),
        ln1_b=np.reshape(inputs["ln1_b"], (1, -1)),
        ln2_g=np.reshape(inputs["ln2_g"], (1, -1)),
        ln2_b=np.reshape(inputs["ln2_b"], (1, -1)),
    )
    wkeys = {k: np.ascontiguousarray(np.asarray(v, np.float32))
             for k, v in wkeys.items()}

    halves = [(0, half), (half, L)]
    in_maps = []
    for core in range(2 * B):
        b, hf = core // 2, core % 2
        s, e = halves[hf]
        fv = np.zeros((LPAD, EMB), np.float16)
        f16b = feats[b].astype(np.float16)
        for l in range(NL):
            fv[vstart[l]:vstart[l] + hw[l]] = \
                f16b[starts[l]:starts[l] + hw[l]]
        fq = np.zeros((HQ, EMB), np.float32)
        fq[:e - s] = feats[b, s:e]
        pq = np.zeros((HQ, EMB), np.float16)
        pq[:e - s] = pos[b, s:e].astype(np.float16)
        rq = np.zeros((HQ, NL, 2), np.float32)
        rq[:e - s] = refp[b, s:e]
        m = dict(feat_val=fv, feat_q=fq, pos_q=pq, ref_q=rq)
        m.update(wkeys)
        m.update({k: np.ascontiguousarray(np.asarray(v, np.float32))
                  for k, v in consts.items()})
        in_maps.append(m)
    return in_maps, halves


def kernel(**inputs):
    from concourse import bass_utils

    cfg = CFG_FULL
    in_maps, halves = make_in_maps(inputs, cfg)
    B = np.asarray(inputs["features"]).shape[0]
    L = cfg["L"]

    if "nc" not in _CACHE:
        _CACHE["nc"] = build_nc(cfg)
    nc = _CACHE["nc"]

    res = bass_utils.run_bass_kernel_spmd(nc, in_maps,
                                          core_ids=list(range(2 * B)))
    out = np.zeros((B, L, EMB), np.float32)
    for core in range(2 * B):
        b, hf = core // 2, core % 2
        s, e = halves[hf]
        out[b, s:e] = res.results[core]["out_q"][:e - s]
    return out


# revision 21
# speedup vs baseline: 982.8261x; 1.0063x over previous
"""Trainium2 Bass kernel for a Deformable-DETR style encoder block.

Sharding: 8 NeuronCores = 4 batch samples x 2 query-halves.

Per core:
  - value projection over the full sample -> fp16 "patch table" in DRAM:
    for cell (y,x) and head h the 2x2 neighborhood [V[y,x], V[y,x+1],
    V[y+1,x], V[y+1,x+1]] is packed contiguously (4*32 fp16 = 256B).
    Levels 0-2 go into one head-major table (rows h*13125 + lvloff + cell)
    so a single gpsimd.dma_gather per (block, head) fetches all 12
    (level, point) patches for 128 queries (1536 records/call, int16
    indices wrapped 16-way and replicated for the 8 Q7 cores).
  - level 3 (13x13) skips the gather entirely: its patch table lives in
    SBUF and a one-hot [cells x queries] matrix from the PE selects
    patches into PSUM (one matmul pair per head/point).
  - offset/attention projections, softmax, bilinear weights and cell
    indices computed query-major; cell indices are PE-transposed into the
    wrapped int16 index layout dma_gather wants.
  - DVE multiplies by bilinear*attention weights and tree-reduces.
  - output projection + LayerNorm + FFN + LayerNorm, then DMA out.
  - frontends are emitted ahead of the previous block's combine/backend
    so the gpsimd gather stream never stalls.
"""

import numpy as np
from contextlib import ExitStack

EMB = 256
NH = 8
NL = 4
NPT = 4
HD = 32
DFFN = 1024
P = 128

# value/table build order: small levels first so tables are ready early
LVL_ORDER = [3, 2, 1, 0]
# levels 0-2 combined per-head table: row = h*CROWS + LVLOFF[l] + cell
LVLOFF = {0: 0, 1: 10000, 2: 12500}
CROWS = 13125  # 10000 + 2500 + 625
L3 = 169       # 13*13 cells in level 3


def make_cfg(shapes, n_blk_q):
    L = sum(h * w for h, w in shapes)
    hw = [h * w for h, w in shapes]
    pblk = {l: -(-hw[l] // P) for l in range(NL)}
    vstart = {}
    off = 0
    for l in LVL_ORDER:
        vstart[l] = off
        off += pblk[l] * P
    return dict(
        shapes=[tuple(s) for s in shapes], hw=hw, L=L,
        pblk=pblk, vstart=vstart, LPAD=off, NBF=off // P,
        NBQ=n_blk_q, HQ=n_blk_q * P,
    )


CFG_FULL = make_cfg([(100, 100), (50, 50), (25, 25), (13, 13)], 52)


# ------------------------------------------------------- host-side consts ---

def host_constants(cfg):
    shapes = cfg["shapes"]
    invnorm = np.zeros(EMB, np.float32)
    pixscale = np.zeros(EMB, np.float32)
    clipmax = np.zeros(EMB, np.float32)
    vmax = np.zeros(EMB, np.float32)
    for h in range(NH):
        for l, (H_, W_) in enumerate(shapes):
            for pt in range(NPT):
                base = h * (NL * NPT * 2) + l * (NPT * 2) + pt * 2
                invnorm[base + 0] = 1.0 / W_
                invnorm[base + 1] = 1.0 / H_
                pixscale[base + 0] = W_
                pixscale[base + 1] = H_
                clipmax[base + 0] = W_ - 2
                clipmax[base + 1] = H_ - 2
                vmax[base + 0] = W_ - 1
                vmax[base + 1] = H_ - 1
    cst_xy = np.stack([invnorm, pixscale, clipmax, vmax])

    wrow = np.zeros(P, np.float32)
    srow = np.zeros(P, np.float32)
    for h in range(NH):
        for l, (H_, W_) in enumerate(shapes):
            for pt in range(NPT):
                base = h * (NL * NPT) + l * NPT + pt
                wrow[base] = W_
                srow[base] = float(LVLOFF.get(l, 0))
    cst_hlp = np.stack([wrow, srow, np.zeros(P, np.float32)])

    ident = np.eye(P, dtype=np.float32)
    ones_row = np.ones((1, P), np.float32)
    iota = np.arange(L3, dtype=np.float32).reshape(L3, 1)
    return dict(cst_xy=cst_xy, cst_hlp=cst_hlp, ident=ident,
                ones_row=ones_row, iota=iota)


# ------------------------------------------------------------- emission ---

def emit_kernel(tc, outs, ins, cfg):
    import concourse.bass as bass
    from concourse import mybir

    nc = tc.nc
    op = mybir.AluOpType
    act_f = mybir.ActivationFunctionType
    f32, f16 = mybir.dt.float32, mybir.dt.float16
    i16 = mybir.dt.int16
    AX = mybir.AxisListType

    shapes, hw = cfg["shapes"], cfg["hw"]
    NBQ = cfg["NBQ"]
    pblk = cfg["pblk"]

    ctx = ExitStack()

    def dap(handle, offset, dims):
        return bass.AP(tensor=handle, offset=offset,
                       ap=[list(d) for d in dims])

    def sap(ap0, extra_off, dims):
        return bass.AP(tensor=ap0.tensor, offset=ap0.offset + extra_off,
                       ap=[list(d) for d in dims])

    # ---- internal DRAM: per-level value + patch tables ----
    valf = {l: nc.dram_tensor(f"valf{l}", [pblk[l] * P, EMB], f16,
                              kind="Internal") for l in range(NL)}
    tabC = nc.dram_tensor("tabC", [NH * CROWS, 4 * HD], f16, kind="Internal")
    tab3 = nc.dram_tensor("tab3", [NH * L3, 4 * HD], f16, kind="Internal")
    # partition-replication bounce buffers (DMA broadcast goes via DRAM)
    NW = NH * 12 * 8
    cwd = [nc.dram_tensor(f"cwd{i}", [16 * NW], i16, kind="Internal")
           for i in range(8)]
    c3d = [nc.dram_tensor(f"c3d{i}", [1, NH * NPT * P], f16, kind="Internal")
           for i in range(8)]

    # ---- pools ----
    cpool = ctx.enter_context(tc.tile_pool(name="consts", bufs=1))
    apool = ctx.enter_context(tc.tile_pool(name="acts", bufs=3))
    wpool = ctx.enter_context(tc.tile_pool(name="wmath", bufs=1))
    gpool = ctx.enter_context(tc.tile_pool(name="gath", bufs=2))
    g3pool = ctx.enter_context(tc.tile_pool(name="gath3", bufs=2))
    bcpool = ctx.enter_context(tc.tile_pool(name="bcast", bufs=1))
    kpool = ctx.enter_context(tc.tile_pool(name="comb", bufs=2))
    opool = ctx.enter_context(tc.tile_pool(name="outp", bufs=2))
    ps_tr = ctx.enter_context(tc.tile_pool(name="ps_tr", bufs=2, space="PSUM"))
    ps_mm = ctx.enter_context(tc.tile_pool(name="ps_mm", bufs=2, space="PSUM"))
    ps_sm = ctx.enter_context(tc.tile_pool(name="ps_sm", bufs=1, space="PSUM"))

    def dma(out_ap, in_ap):
        nc.sync.dma_start(out=out_ap, in_=in_ap)

    # ---- constants / weights ----
    def load_w(name, k, n):
        t = cpool.tile([P, k // P, n], f16, name=f"s_{name}")
        src_r = ins[name].rearrange("(a p) n -> p a n", p=P)
        ka = k // P
        nch = -(-(ka * n) // DFFN)
        for c in range(nch):
            a0 = ka * c // nch
            a1 = ka * (c + 1) // nch
            n0, n1 = 0, n
            if ka == a1 - a0 == ka and nch > 1:
                pass
            if a1 == a0:  # chunk along n instead
                a0, a1 = 0, ka
                n0 = n * c // nch
                n1 = n * (c + 1) // nch
            tmp = apool.tile([P, DFFN], f32, name="wtmp", tag="wtmp",
                             bufs=1)
            na = n1 - n0
            tv = sap(tmp[:, :], 0, [tmp[:, :].ap[0], [na, a1 - a0], [1, na]])
            dma(tv, src_r[:, a0:a1, n0:n1])
            nc.vector.tensor_copy(t[:, a0:a1, n0:n1], tv)
        return t

    Wval = load_w("W_val", EMB, EMB)
    Woff = load_w("W_off", EMB, EMB)
    Watt = load_w("W_attn", EMB, NH * NL * NPT)
    Wout = load_w("W_out", EMB, EMB)
    W1 = load_w("W1", EMB, DFFN)
    W2 = load_w("W2", DFFN, EMB)

    def load_row(name, n):
        t = cpool.tile([1, n], f32, name=f"r_{name}")
        dma(t, ins[name][:, :])
        return t

    bval = load_row("b_val", EMB)
    boff = load_row("b_off", EMB)
    batt = load_row("b_attn", NH * NL * NPT)
    bout = load_row("b_out", EMB)
    b1r = load_row("b1", DFFN)
    b2r = load_row("b2", EMB)
    onesr = load_row("ones_row", P)

    def load_bc(src_ap, n, name):
        t = cpool.tile([P, n], f32, name=f"b_{name}")
        dma(t, src_ap.to_broadcast([P, n]))
        return t

    ln1g = load_bc(ins["ln1_g"][:, :], EMB, "ln1g")
    ln1b = load_bc(ins["ln1_b"][:, :], EMB, "ln1b")
    ln2g = load_bc(ins["ln2_g"][:, :], EMB, "ln2g")
    ln2b = load_bc(ins["ln2_b"][:, :], EMB, "ln2b")
    c_invn = load_bc(ins["cst_xy"][0:1, :], EMB, "invn")
    c_pixs = load_bc(ins["cst_xy"][1:2, :], EMB, "pixs")
    c_clip = load_bc(ins["cst_xy"][2:3, :], EMB, "clip")
    c_vmax = load_bc(ins["cst_xy"][3:4, :], EMB, "vmax")
    c_W = load_bc(ins["cst_hlp"][0:1, :], P, "cw")
    c_S = load_bc(ins["cst_hlp"][1:2, :], P, "cs")

    ident = cpool.tile([P, P], f32, name="ident")
    dma(ident, ins["ident"][:, :])
    eps_t = cpool.tile([P, 1], f32, name="eps_t")
    nc.vector.memset(eps_t[:, :], 1e-5)

    iota_a = cpool.tile([P, 1], f32, name="iota_a")
    dma(iota_a, ins["iota"][0:P, :])
    iota_b = cpool.tile([L3 - P, 1], f32, name="iota_b")
    dma(iota_b, ins["iota"][P:L3, :])

    refr = cpool.tile([P, NBQ, 2 * NL], f32, name="refr")
    dma(refr, ins["ref_q"].rearrange("(b p) l c -> p b (l c)", p=P))

    # level-3 patch table resident in SBUF: [cell, head, 128]
    t3a = cpool.tile([P, NH, 4 * HD], f16, name="t3a")
    t3b = cpool.tile([L3 - P, NH, 4 * HD], f16, name="t3b")

    def mm(psum_ap, pairs, bias=None):
        seq = list(pairs)
        if bias is not None:
            seq.append((onesr[:1, :psum_ap.shape[0]], bias))
        for i, (lt, rt) in enumerate(seq):
            nc.tensor.matmul(psum_ap, lt, rt,
                             start=(i == 0), stop=(i == len(seq) - 1))

    # ======================= patch-table build (per level) ==============
    def table_dst(l, h):
        if l == 3:
            return tab3, (h * L3) * 4 * HD
        return tabC, (h * CROWS + LVLOFF[l]) * 4 * HD

    def emit_table(l):
        H_, W_ = shapes[l]
        vt = valf[l]
        for h in range(NH):
            tt, tbase = table_dst(l, h)
            for cy in (0, 1):
                for cx in (0, 1):
                    c = cy * 2 + cx
                    src = dap(vt, (cy * W_ + cx) * EMB + h * HD,
                              [[W_ * EMB, H_ - 1], [EMB, W_ - 1], [1, HD]])
                    dst = dap(tt, tbase + c * HD,
                              [[W_ * 4 * HD, H_ - 1], [4 * HD, W_ - 1],
                               [1, HD]])
                    dma(dst, src)
            # fill never-gathered edge records (x=W-1 col, y=H-1 row) so the
            # table contains no uninitialized (possibly non-finite) bytes
            dma(dap(tt, tbase + (W_ - 1) * 4 * HD,
                    [[W_ * 4 * HD, H_], [HD, 4], [1, HD]]),
                dap(vt, (W_ - 1) * EMB + h * HD,
                    [[W_ * EMB, H_], [0, 4], [1, HD]]))
            dma(dap(tt, tbase + ((H_ - 1) * W_) * 4 * HD,
                    [[4 * HD, W_ - 1], [HD, 4], [1, HD]]),
                dap(vt, ((H_ - 1) * W_) * EMB + h * HD,
                    [[EMB, W_ - 1], [0, 4], [1, HD]]))
        if l == 3:
            # SBUF copy for the PE-side gather: [cell, head, 128]
            dma(t3a, tab3.ap().rearrange("(h c) d -> c h d", h=NH)[0:P])
            dma(t3b, tab3.ap().rearrange("(h c) d -> c h d", h=NH)[P:L3])

    # ======================= value projection ===========================
    def emit_value():
        idf16 = cpool.tile([P, P], f16, name="idf16")
        nc.vector.tensor_copy(idf16[:, :], ident[:, :])
        sched = []
        for l in LVL_ORDER:
            for lb in range(pblk[l]):
                sched.append((l, lb, lb == pblk[l] - 1))
        for blk, (l, lb, last) in enumerate(sched):
            fv = apool.tile([P, EMB], f16, name="fv", tag="fv")
            dma(fv, ins["feat_val"][blk * P:(blk + 1) * P, :])
            ftp = ps_tr.tile([P, 2, P], f16, name="ftp", tag="tr")
            nc.tensor.transpose(ftp[:, 0, :], fv[:, 0:P], idf16[:, :])
            nc.tensor.transpose(ftp[:, 1, :], fv[:, P:EMB], idf16[:, :])
            fts = apool.tile([P, 2, P], f16, name="fts", tag="fts")
            nc.vector.tensor_copy(fts[:, :, :], ftp[:, :, :])
            vp = ps_mm.tile([P, EMB], f32, name="vp", tag="mm")
            mm(vp, [(fts[:, 0, :], Wval[:, 0, :]),
                    (fts[:, 1, :], Wval[:, 1, :])], bias=bval[:1, :])
            vf = apool.tile([P, EMB], f16, name="vf", tag="vf")
            nc.vector.tensor_copy(vf[:, :], vp[:, :])
            dma(valf[l].ap()[lb * P:(lb + 1) * P, :], vf)
            if last:
                emit_table(l)

    # ==================== per-block frontend ====================
    def emit_frontend(blk):
        fq = apool.tile([P, EMB], f32, name="fq", tag="fq", bufs=6)
        dma(fq, ins["feat_q"][blk * P:(blk + 1) * P, :])
        pq = apool.tile([P, EMB], f16, name="pq", tag="pq")
        dma(pq, ins["pos_q"][blk * P:(blk + 1) * P, :])
        qb = apool.tile([P, EMB], f32, name="qb", tag="qb")
        nc.vector.tensor_add(qb[:, :], fq[:, :], pq[:, :])

        qtp = ps_tr.tile([P, 2, P], f32, name="qtp", tag="tr")
        nc.tensor.transpose(qtp[:, 0, :], qb[:, 0:P], ident[:, :])
        nc.tensor.transpose(qtp[:, 1, :], qb[:, P:EMB], ident[:, :])
        qts = apool.tile([P, 2, P], f16, name="qts", tag="qts", bufs=3)
        nc.vector.tensor_copy(qts[:, :, :], qtp[:, :, :])

        offp = ps_mm.tile([P, EMB], f32, name="offp", tag="mm")
        mm(offp, [(qts[:, 0, :], Woff[:, 0, :]), (qts[:, 1, :], Woff[:, 1, :])],
           bias=boff[:1, :])
        off = wpool.tile([P, EMB], f32, name="off", tag="off")
        nc.vector.tensor_copy(off[:, :], offp[:, :])

        attp = ps_sm.tile([P, NH * 16], f32, name="attp", tag="sm")
        mm(attp, [(qts[:, 0, :], Watt[:, 0, :]), (qts[:, 1, :], Watt[:, 1, :])],
           bias=batt[:1, :])
        att = wpool.tile([P, NH, 16], f32, name="att", tag="att")
        nc.vector.tensor_copy(att[:, :, :], attp[:, :].rearrange(
            "p (h l) -> p h l", h=NH))

        # softmax over (l,pt) per head
        rmax = wpool.tile([P, NH], f32, name="rmax", tag="rmax")
        nc.vector.reduce_max(rmax[:, :], att[:, :, :], axis=AX.X)
        exv = wpool.tile([P, NH, 16], f32, name="exv", tag="exv")
        rmaxa = rmax[:, :]
        nc.vector.tensor_sub(exv[:, :, :], att[:, :, :],
                             sap(rmaxa, 0, [rmaxa.ap[0], [1, NH], [0, 16]]))
        nc.scalar.activation(exv[:, :, :], exv[:, :, :], act_f.Exp)
        ssum = wpool.tile([P, NH], f32, name="ssum", tag="ssum")
        nc.vector.reduce_sum(ssum[:, :], exv[:, :, :], axis=AX.X)
        rsum = wpool.tile([P, NH], f32, name="rsum", tag="rsum")
        nc.vector.reciprocal(rsum[:, :], ssum[:, :])
        aw = wpool.tile([P, NH, 16], f32, name="aw", tag="aw")
        rsuma = rsum[:, :]
        nc.vector.tensor_mul(aw[:, :, :], exv[:, :, :],
                             sap(rsuma, 0, [rsuma.ap[0], [1, NH], [0, 16]]))

        def wt(name):
            return wpool.tile([P, EMB], f32, name=name, tag=name)

        loc = wt("loc")
        nc.vector.tensor_mul(loc[:, :], off[:, :], c_invn[:, :])
        refa = refr[:, blk, :]
        for xy in (0, 1):
            lvh = sap(loc[:, :], xy, [loc[:, :].ap[0], [32, NH], [8, NL],
                                      [2, NPT]])
            nc.vector.tensor_add(lvh, lvh,
                                 sap(refa, xy, [refa.ap[0], [0, NH], [2, NL],
                                                [0, NPT]]))
        pix = wt("pix")
        nc.vector.tensor_mul(pix[:, :], loc[:, :], c_pixs[:, :])
        nc.vector.tensor_scalar_add(pix[:, :], pix[:, :], -0.5)

        # floor(pix) robust to cast rounding mode
        xi = wpool.tile([P, EMB], mybir.dt.int32, name="xi", tag="xi")
        nc.vector.tensor_copy(xi[:, :], pix[:, :])
        base = wt("base")
        nc.vector.tensor_copy(base[:, :], xi[:, :])
        fixm = wt("fixm")
        nc.vector.tensor_tensor(fixm[:, :], pix[:, :], base[:, :], op=op.is_lt)
        nc.vector.tensor_sub(base[:, :], base[:, :], fixm[:, :])
        wfrac = wt("wfrac")
        nc.vector.tensor_sub(wfrac[:, :], pix[:, :], base[:, :])

        basec = wt("basec")
        nc.vector.tensor_scalar_max(basec[:, :], base[:, :], 0.0)
        nc.vector.tensor_tensor(basec[:, :], basec[:, :], c_clip[:, :],
                                op=op.min)

        v0b = wt("v0b")
        nc.vector.tensor_tensor(v0b[:, :], base[:, :], c_vmax[:, :],
                                op=op.is_le)
        vld0 = wt("vld0")
        nc.vector.scalar_tensor_tensor(vld0[:, :], base[:, :], 0.0, v0b[:, :],
                                       op0=op.is_ge, op1=op.mult)
        v1b = wt("v1b")
        nc.vector.tensor_tensor(v1b[:, :], base[:, :], c_clip[:, :],
                                op=op.is_le)
        vld1 = wt("vld1")
        nc.vector.scalar_tensor_tensor(vld1[:, :], base[:, :], -1.0, v1b[:, :],
                                       op0=op.is_ge, op1=op.mult)

        tsh = wt("tsh")
        nc.vector.tensor_sub(tsh[:, :], base[:, :], basec[:, :])
        e0 = wt("e0")
        nc.vector.tensor_scalar(e0[:, :], tsh[:, :], 0.0, None,
                                op0=op.is_equal)
        em1 = wt("em1")
        nc.vector.tensor_scalar(em1[:, :], tsh[:, :], -1.0, None,
                                op0=op.is_equal)
        ep1 = wt("ep1")
        nc.vector.tensor_scalar(ep1[:, :], tsh[:, :], 1.0, None,
                                op0=op.is_equal)

        u0 = wt("u0")
        nc.vector.tensor_scalar(u0[:, :], wfrac[:, :], -1.0, 1.0, op0=op.mult,
                                op1=op.add)
        nc.vector.tensor_mul(u0[:, :], u0[:, :], vld0[:, :])
        u1 = wt("u1")
        nc.vector.tensor_mul(u1[:, :], wfrac[:, :], vld1[:, :])

        a0 = wt("a0")
        nc.vector.tensor_mul(a0[:, :], u0[:, :], e0[:, :])
        t1 = wt("t1")
        nc.vector.tensor_mul(t1[:, :], u1[:, :], em1[:, :])
        nc.vector.tensor_add(a0[:, :], a0[:, :], t1[:, :])
        a1 = wt("a1")
        nc.vector.tensor_mul(a1[:, :], u0[:, :], ep1[:, :])
        nc.vector.tensor_mul(t1[:, :], u1[:, :], e0[:, :])
        nc.vector.tensor_add(a1[:, :], a1[:, :], t1[:, :])

        def ycols(t):
            return sap(t[:, :], 1, [[t[:, :].ap[0][0], P], [2, P]])

        def xcols(t):
            return sap(t[:, :], 0, [[t[:, :].ap[0][0], P], [2, P]])

        awf = aw.rearrange("p h l -> p (h l)")
        ay0 = wpool.tile([P, P], f32, name="ay0", tag="ay0")
        nc.vector.tensor_mul(ay0[:, :], ycols(a0), awf)
        ay1 = wpool.tile([P, P], f32, name="ay1", tag="ay1")
        nc.vector.tensor_mul(ay1[:, :], ycols(a1), awf)

        w4 = wpool.tile([P, P, 4], f16, name="w4", tag="w4", bufs=6)
        nc.vector.tensor_mul(w4[:, :, 0], ay0[:, :], xcols(a0))
        nc.vector.tensor_mul(w4[:, :, 1], ay0[:, :], xcols(a1))
        nc.vector.tensor_mul(w4[:, :, 2], ay1[:, :], xcols(a0))
        nc.vector.tensor_mul(w4[:, :, 3], ay1[:, :], xcols(a1))

        # cell index within level (+ level offset for the combined table)
        cell = wpool.tile([P, P], f32, name="cell", tag="cell")
        nc.vector.tensor_mul(cell[:, :], ycols(basec), c_W[:, :])
        nc.vector.tensor_add(cell[:, :], cell[:, :], xcols(basec))
        nc.vector.tensor_add(cell[:, :], cell[:, :], c_S[:, :])

        # transpose -> cellT [slot, q]
        ctp = ps_tr.tile([P, P], f32, name="ctp", tag="tr")
        nc.tensor.transpose(ctp[:, :], cell[:, :], ident[:, :])
        cellT = wpool.tile([P, P], f32, name="cellT", tag="cellT", bufs=3)
        nc.vector.tensor_copy(cellT[:, :], ctp[:, :])

        # level-3 index rows [32, q] as f16 (cells <= 168, exact):
        # transpose of the 32 level-3 columns of `cell`.
        cella = cell[:, :]
        c3c = wpool.tile([P, NH * NPT], f32, name="c3c", tag="c3c")
        nc.vector.tensor_copy(
            c3c[:, :], sap(cella, 12, [cella.ap[0], [16, NH], [1, NPT]]))
        c3p = ps_tr.tile([NH * NPT, P], f32, name="c3p", tag="tr3", bufs=1)
        nc.tensor.transpose(c3p[:, :], c3c[:, :], ident[:, :])
        cT3 = wpool.tile([NH * NPT, P], f16, name="cT3", tag="cT3", bufs=3)
        nc.vector.tensor_copy(cT3[:, :], c3p[:, :])
        # bounce through DRAM; broadcast-read happens in emit_l3
        dma(dap(c3d[blk % 8], 0, [[P, NH * NPT], [1, P]]), cT3[:, :])

        # wrapped int16 index layout for dma_gather: positions i = s*128+q
        # live at [i%16, i//16]; build via 8 [128,16]->[16,128] transposes.
        cW0 = wpool.tile([16, NH, 12, 8], i16, name="cW0", tag="cW0", bufs=3)
        for qhi in range(8):
            stp = ps_tr.tile([16, P], f32, name="stp", tag="tr3", bufs=1)
            nc.tensor.transpose(stp[:, :],
                                cellT[:, qhi * 16:(qhi + 1) * 16],
                                ident[:, :])
            pstr = stp[:, :].ap[0][0]
            src = sap(stp[:, :], 0, [[pstr, 16], [16, NH], [1, 12]])
            d0 = cW0[:, :, :, :]
            dst = sap(d0, qhi, [d0.ap[0], [12 * 8, NH], [8, 12]])
            nc.vector.tensor_copy(dst, src)
        # bounce through DRAM, replicating the 16 wrapped partitions x8
        dma(dap(cwd[blk % 8], 0, [[NW, 16], [1, NW]]),
            cW0[:, :, :, :].rearrange("p a b c -> p (a b c)"))
        cW = wpool.tile([P, NH, 12, 8], i16, name="cW", tag="cW", bufs=6)
        dma(cW[:, :, :, :].rearrange("p a b c -> p (a b c)"),
            dap(cwd[blk % 8], 0, [[0, 8], [NW, 16], [1, NW]]))
        return fq, w4, cW

    # ==================== LayerNorm ====================
    def emit_ln(r, gt, bt, pfx):
        nsum = opool.tile([P, 1], f32, name=f"{pfx}ns", tag=f"{pfx}ns")
        nc.vector.tensor_reduce(nsum[:, :], r[:, :], axis=AX.X, op=op.add,
                                negate=True)
        nmean = opool.tile([P, 1], f32, name=f"{pfx}nm", tag=f"{pfx}nm")
        nc.scalar.mul(nmean[:, :], nsum[:, :], 1.0 / EMB)
        c = opool.tile([P, EMB], f32, name=f"{pfx}c", tag=f"{pfx}c")
        nc.vector.tensor_scalar_add(c[:, :], r[:, :], nmean[:, :])
        csq = opool.tile([P, EMB], f32, name=f"{pfx}sq", tag=f"{pfx}sq")
        ssq = opool.tile([P, 1], f32, name=f"{pfx}ssq", tag=f"{pfx}ssq")
        nc.scalar.activation(csq[:, :], c[:, :], act_f.Square,
                             accum_out=ssq[:, :])
        std = opool.tile([P, 1], f32, name=f"{pfx}std", tag=f"{pfx}std")
        nc.scalar.activation(std[:, :], ssq[:, :], act_f.Sqrt,
                             bias=eps_t[:, :], scale=1.0 / EMB)
        rstd = opool.tile([P, 1], f32, name=f"{pfx}rs", tag=f"{pfx}rs")
        nc.vector.reciprocal(rstd[:, :], std[:, :])
        x = opool.tile([P, EMB], f32, name=f"{pfx}x", tag=f"{pfx}x")
        nc.vector.scalar_tensor_tensor(x[:, :], c[:, :], rstd[:, :], gt[:, :],
                                       op0=op.mult, op1=op.mult)
        nc.vector.tensor_add(x[:, :], x[:, :], bt[:, :])
        return x

    # ==================== gathers for one block (levels 0-2) ============
    def emit_gathers(cW):
        gb = gpool.tile([P, NH * 12, 4 * HD], f16, name="gb", tag="gb",
                        bufs=2)
        for h in range(NH):
            nc.gpsimd.dma_gather(
                out_ap=gb[:, h * 12:(h + 1) * 12, :],
                in_ap=tabC.ap()[h * CROWS:(h + 1) * CROWS, :],
                idxs_ap=cW[:, h, :, :],
                num_idxs=12 * P,
                num_idxs_reg=12 * P,
                elem_size=4 * HD,
                single_packet=False,
            )
        return gb

    # ==================== level-3 via PE one-hot ========================
    def emit_l3(blk):
        gb3 = g3pool.tile([P, NH * NPT, 4 * HD], f16, name="gb3", tag="gb3",
                          bufs=2)
        # broadcast all 32 level-3 index rows across cell-partitions
        nidx = NH * NPT * P
        bca = bcpool.tile([P, NH * NPT, P], f16, name="bca", tag="bca")
        dma(bca, c3d[blk % 8].ap().to_broadcast([P, nidx]).rearrange(
            "p (a b) -> p a b", a=NH * NPT))
        bcb = bcpool.tile([L3 - P, NH * NPT, P], f16, name="bcb", tag="bcb")
        dma(bcb, c3d[blk % 8].ap().to_broadcast([L3 - P, nidx]).rearrange(
            "p (a b) -> p a b", a=NH * NPT))
        for h in range(NH):
            ps3 = ps_sm.tile([P, NPT, 4 * HD], f32, name="ps3", tag="sm")
            for pt in range(NPT):
                s3 = h * NPT + pt
                oha = wpool.tile([P, P], f16, name="oha", tag="oha", bufs=2)
                nc.vector.tensor_scalar(oha[:, :], bca[:, s3, :],
                                        iota_a[:, :], None, op0=op.is_equal)
                ohb = wpool.tile([L3 - P, P], f16, name="ohb", tag="ohb",
                                 bufs=2)
                nc.vector.tensor_scalar(ohb[:, :], bcb[:, s3, :],
                                        iota_b[:, :], None, op0=op.is_equal)
                nc.tensor.matmul(ps3[:, pt, :], oha[:, :], t3a[:, h, :],
                                 start=True, stop=False)
                nc.tensor.matmul(ps3[:, pt, :], ohb[:, :], t3b[:, h, :],
                                 start=False, stop=True)
            nc.scalar.mul(gb3[:, h * NPT:(h + 1) * NPT, :], ps3[:, :, :], 1.0)
        return gb3

    # ==================== combine + backend for one block ================
    def emit_backend(blk, fq, w4, gb, gb3):
        acat = kpool.tile([P, EMB], f32, name="acat", tag="acat")
        gba = gb[:, :, :]
        pstr = gba.ap[0][0]
        g3a = gb3[:, :, :]
        p3str = g3a.ap[0][0]
        wstr = w4[:, :, :].ap[0][0]

        def gsl(off, dims):
            return sap(gba, off, [[pstr, P]] + dims)

        def g3l(off, dims):
            return sap(g3a, off, [[p3str, P]] + dims)

        # multiply by bilinear*attention weights (broadcast over head_dim)
        w4g = sap(w4[:, :, :], 0,
                  [[wstr, P], [64, NH], [4, 12], [1, 4], [0, HD]])
        gall = gsl(0, [[12 * 128, NH], [128, 12], [HD, 4], [1, HD]])
        nc.vector.tensor_mul(gall, gall, w4g)
        w43 = sap(w4[:, :, :], 48,
                  [[wstr, P], [64, NH], [4, NPT], [1, 4], [0, HD]])
        g3ll = g3l(0, [[4 * 128, NH], [128, NPT], [HD, 4], [1, HD]])
        nc.vector.tensor_mul(g3ll, g3ll, w43)

        # corner folds: c0+=c1, c2+=c3, c0+=c2
        d2 = [[128, NH * 12], [1, HD]]
        nc.vector.tensor_add(gsl(0, d2), gsl(0, d2), gsl(HD, d2))
        nc.vector.tensor_add(gsl(2 * HD, d2), gsl(2 * HD, d2), gsl(3 * HD, d2))
        nc.vector.tensor_add(gsl(0, d2), gsl(0, d2), gsl(2 * HD, d2))
        d3 = [[128, NH * NPT], [1, HD]]
        nc.vector.tensor_add(g3l(0, d3), g3l(0, d3), g3l(HD, d3))
        nc.vector.tensor_add(g3l(2 * HD, d3), g3l(2 * HD, d3),
                             g3l(3 * HD, d3))
        nc.vector.tensor_add(g3l(0, d3), g3l(0, d3), g3l(2 * HD, d3))

        # level folds within each head: lp[0..4) += lp[4..8), lp[8..12), l3
        dl = [[12 * 128, NH], [128, NPT], [1, HD]]
        nc.vector.tensor_add(gsl(0, dl), gsl(0, dl), gsl(4 * 128, dl))
        nc.vector.tensor_add(gsl(0, dl), gsl(0, dl), gsl(8 * 128, dl))
        d3l = [[4 * 128, NH], [128, NPT], [1, HD]]
        nc.vector.tensor_add(gsl(0, dl), gsl(0, dl), g3l(0, d3l))
        # point folds: 4 -> 2 -> 1 (final fold writes acat slices)
        dp = [[12 * 128, NH], [128, 2], [1, HD]]
        nc.vector.tensor_add(gsl(0, dp), gsl(0, dp), gsl(2 * 128, dp))
        acv = sap(acat[:, :], 0, [[acat[:, :].ap[0][0], P], [HD, NH], [1, HD]])
        dh1 = [[12 * 128, NH], [1, HD]]
        nc.vector.tensor_add(acv, gsl(0, dh1), gsl(128, dh1))

        # ---- output projection + LN + FFN + LN ----
        atp = ps_tr.tile([P, 2, P], f32, name="atp", tag="tr")
        nc.tensor.transpose(atp[:, 0, :], acat[:, 0:P], ident[:, :])
        nc.tensor.transpose(atp[:, 1, :], acat[:, P:EMB], ident[:, :])
        ats = opool.tile([P, 2, P], f16, name="ats", tag="ats")
        nc.vector.tensor_copy(ats[:, :, :], atp[:, :, :])
        oprj = ps_mm.tile([P, EMB], f32, name="oprj", tag="mm")
        mm(oprj, [(ats[:, 0, :], Wout[:, 0, :]),
                  (ats[:, 1, :], Wout[:, 1, :])], bias=bout[:1, :])

        r1 = opool.tile([P, EMB], f32, name="r1", tag="r1")
        nc.vector.tensor_add(r1[:, :], oprj[:, :], fq[:, :])
        x1 = emit_ln(r1, ln1g, ln1b, "la")

        xtp = ps_tr.tile([P, 2, P], f32, name="xtp", tag="tr")
        nc.tensor.transpose(xtp[:, 0, :], x1[:, 0:P], ident[:, :])
        nc.tensor.transpose(xtp[:, 1, :], x1[:, P:EMB], ident[:, :])
        xts = opool.tile([P, 2, P], f16, name="xts", tag="xts")
        nc.vector.tensor_copy(xts[:, :, :], xtp[:, :, :])

        h1s = opool.tile([P, DFFN // P, P], f16, name="h1s", tag="h1s", bufs=1)
        hp = ps_mm.tile([P, DFFN // P, P], f32, name="hp", tag="hpw", bufs=1)
        for mt in range(DFFN // P):
            nc.tensor.matmul(hp[:, mt, :], W1[:, 0, mt * P:(mt + 1) * P],
                             xts[:, 0, :], start=True, stop=False)
            nc.tensor.matmul(hp[:, mt, :], W1[:, 1, mt * P:(mt + 1) * P],
                             xts[:, 1, :], start=False, stop=False)
            nc.tensor.matmul(hp[:, mt, :], b1r[:1, mt * P:(mt + 1) * P],
                             onesr[:1, :], start=False, stop=True)
        nc.scalar.activation(h1s[:, :, :], hp[:, :, :], act_f.Relu)

        yp = ps_mm.tile([P, EMB], f32, name="yp", tag="mm")
        for mt in range(DFFN // P):
            nc.tensor.matmul(yp[:, :], h1s[:, mt, :], W2[:, mt, :],
                             start=(mt == 0), stop=False)
        nc.tensor.matmul(yp[:, :], onesr[:1, :], b2r[:1, :],
                         start=False, stop=True)

        r2 = opool.tile([P, EMB], f32, name="r2", tag="r2")
        nc.vector.tensor_add(r2[:, :], yp[:, :], x1[:, :])
        x2 = emit_ln(r2, ln2g, ln2b, "lb")
        dma(outs["out_q"][blk * P:(blk + 1) * P, :], x2)

    # ==================== top-level schedule ====================
    LOOK = 4
    F = {0: emit_frontend(0), 1: emit_frontend(1)}
    emit_value()
    for j in range(2, LOOK):
        F[j] = emit_frontend(j)
    G = {0: emit_gathers(F[0][2])}
    for g in range(NBQ):
        if g + LOOK < NBQ:
            F[g + LOOK] = emit_frontend(g + LOOK)
        gb3 = emit_l3(g)
        if g + 1 < NBQ:
            G[g + 1] = emit_gathers(F[g + 1][2])
        fq, w4, _ = F.pop(g)
        emit_backend(g, fq, w4, G.pop(g), gb3)

    ctx.close()


# ------------------------------------------------------------ host entry ---

_CACHE = {}


def build_nc(cfg):
    from concourse import bacc, mybir, tile

    nc = bacc.Bacc("TRN2", debug=False)
    f32 = mybir.dt.float32
    f16 = mybir.dt.float16

    def di(name, shape, dt=None):
        return nc.dram_tensor(name, list(shape), dt or f32,
                              kind="ExternalInput").ap()

    HQ, LPAD = cfg["HQ"], cfg["LPAD"]
    ins = dict(
        feat_val=di("feat_val", [LPAD, EMB], f16),
        feat_q=di("feat_q", [HQ, EMB]),
        pos_q=di("pos_q", [HQ, EMB], f16),
        ref_q=di("ref_q", [HQ, NL, 2]),
        W_val=di("W_val", [EMB, EMB]), b_val=di("b_val", [1, EMB]),
        W_off=di("W_off", [EMB, EMB]), b_off=di("b_off", [1, EMB]),
        W_attn=di("W_attn", [EMB, NH * NL * NPT]),
        b_attn=di("b_attn", [1, NH * NL * NPT]),
        W_out=di("W_out", [EMB, EMB]), b_out=di("b_out", [1, EMB]),
        W1=di("W1", [EMB, DFFN]), b1=di("b1", [1, DFFN]),
        W2=di("W2", [DFFN, EMB]), b2=di("b2", [1, EMB]),
        ln1_g=di("ln1_g", [1, EMB]), ln1_b=di("ln1_b", [1, EMB]),
        ln2_g=di("ln2_g", [1, EMB]), ln2_b=di("ln2_b", [1, EMB]),
        cst_xy=di("cst_xy", [4, EMB]),
        cst_hlp=di("cst_hlp", [3, P]),
        ident=di("ident", [P, P]),
        ones_row=di("ones_row", [1, P]),
        iota=di("iota", [L3, 1]),
    )
    outs = dict(
        out_q=nc.dram_tensor("out_q", [HQ, EMB], f32,
                             kind="ExternalOutput").ap(),
    )
    with tile.TileContext(nc) as tc:
        emit_kernel(tc, outs, ins, cfg)
    nc.compile()
    return nc


def make_in_maps(inputs, cfg):
    feats = np.asarray(inputs["features"], np.float32)
    pos = np.asarray(inputs["pos"], np.float32)
    refp = np.asarray(inputs["reference_points"], np.float32)
    B = feats.shape[0]
    HQ, LPAD, L = cfg["HQ"], cfg["LPAD"], cfg["L"]
    hw, vstart = cfg["hw"], cfg["vstart"]
    starts = np.cumsum([0] + hw)[:-1]
    half = L // 2

    consts = host_constants(cfg)
    wkeys = dict(
        W_val=inputs["W_val"], b_val=np.reshape(inputs["b_val"], (1, -1)),
        W_off=inputs["W_off"], b_off=np.reshape(inputs["b_off"], (1, -1)),
        W_attn=inputs["W_attn"], b_attn=np.reshape(inputs["b_attn"], (1, -1)),
        W_out=inputs["W_out"], b_out=np.reshape(inputs["b_out"], (1, -1)),
        W1=inputs["W1"], b1=np.reshape(inputs["b1"], (1, -1)),
        W2=inputs["W2"], b2=np.reshape(inputs["b2"], (1, -1)),
        ln1_g=np.reshape(inputs["ln1_g"], (1, -1)),
        ln1_b=np.reshape(inputs["ln1_b"], (1, -1)),
        ln2_g=np.reshape(inputs["ln2_g"], (1, -1)),
        ln2_b=np.reshape(inputs["ln2_b"], (1, -1)),
    )
    wkeys = {k: np.ascontiguousarray(np.asarray(v, np.float32))
             for k, v in wkeys.items()}

    halves = [(0, half), (half, L)]
    in_maps = []
    for core in range(2 * B):
        b, hf = core // 2, core % 2
        s, e = halves[hf]
        fv = np.zeros((LPAD, EMB), np.float16)
        f16b = feats[b].astype(np.float16)
        for l in range(NL):
            fv[vstart[l]:vstart[l] + hw[l]] = \
                f16b[starts[l]:starts[l] + hw[l]]
        fq = np.zeros((HQ, EMB), np.float32)
        fq[:e - s] = feats[b, s:e]
        pq = np.zeros((HQ, EMB), np.float16)
        pq[:e - s] = pos[b, s:e].astype(np.float16)
        rq = np.zeros((HQ, NL, 2), np.float32)
        rq[:e - s] = refp[b, s:e]
        m = dict(feat_val=fv, feat_q=fq, pos_q=pq, ref_q=rq)
        m.update(wkeys)
        m.update({k: np.ascontiguousarray(np.asarray(v, np.float32))
                  for k, v in consts.items()})
        in_maps.append(m)
    return in_maps, halves


def kernel(**inputs):
    from concourse import bass_utils

    cfg = CFG_FULL
    in_maps, halves = make_in_maps(inputs, cfg)
    B = np.asarray(inputs["features"]).shape[0]
    L = cfg["L"]

    if "nc" not in _CACHE:
        _CACHE["nc"] = build_nc(cfg)
    nc = _CACHE["nc"]

    res = bass_utils.run_bass_kernel_spmd(nc, in_maps,
                                          core_ids=list(range(2 * B)))
    out = np.zeros((B, L, EMB), np.float32)
    for core in range(2 * B):
        b, hf = core // 2, core % 2
        s, e = halves[hf]
        out[b, s:e] = res.results[core]["out_q"][:e - s]
    return out
